# revision 8
# baseline (speedup 1.0000x reference)
"""Block-sparse MoE (softmax top-k routing + silu-gated FFN) on 8 Trainium2 cores.

Sharding: expert-pair x FFN-half. The router runs on host; each expert's
token list is sorted by router weight and split at THETA: high-weight
pairs run in bf16, low-weight pairs in fp8 e4m3 (DoubleRow, 2x tensor
rate) — the output error each fp8 token contributes is proportional to
its (small) router weight, keeping total rel err ~1.2e-2 vs the 2e-2
gate. Segments are LPT-packed onto 4 core-pairs (fp8 cost 0.5/col) and
each pair of cores splits the FFN dim in half (1792 of 3584), so all 8
cores run the same slot-size program (SPMD) with balanced cycles.

Per slot on a core (S tokens, ND=16 d-tiles, NFh=14 f-tiles):
  phase1: h.T [128 f, chunk] = w1h.T @ x (contract D); silu(g)*u -> aT
  phase3: y[d, tok] = sum_fi w2h[f,d].T @ aT[:, fi, :] (contract F/2)
          d on PSUM partitions, tokens moving => exact columns, no pad.
  fp8 slots: x*16, w1*32 -> psum = 512*h; silu via ACT scale 1/512;
  u16 = psum/32; a8 = e4m3(silu*u16) = 16a; w2*64 -> y*1024 (host /1024).

Moving chunks are equalized (~420-510) so LDWEIGHTS (~97-116ns) hides
under the moving phase. Slot0's x arrives dt-major in two column chunks
consumed dt-outer across 8 PSUM banks (PE starts ~12us in). w1 prefetch
depth 4 covers the ~6us pair-tile transfer. All bulk traffic rides the
SWDGE queue with >=2KB lines; later slots' x and w1 overlap prior ph3.
Host sums the two F-halves, applies router weight, scatter-adds (f32).
"""

import numpy as np


def _ensure_concourse_on_path():
    try:
        import concourse  # noqa: F401
    except ImportError:
        import sys

        for p in ("/opt/trn_rl_repo", "/root/.axon_site/_ro/trn_rl_repo"):
            if p not in sys.path:
                sys.path.insert(0, p)


_ensure_concourse_on_path()

P = 128
CHUNK_MAX = 512   # PSUM bank free-dim limit (f32)
SLOT_CAP = 2560   # max tokens per slot (SBUF-bound)
THETA = 0.33      # router-weight cutoff: below -> fp8 path
X8S, W1S, W2S = 16.0, 32.0, 64.0
Y8S = X8S * W2S   # fp8-slot output scale (host divides)

_BASS_CACHE: dict = {}
last_run = None  # BassKernelResults of the most recent kernel() call (for test.py)


def _legalize_sync(nc, max_waits: int = 1):
    """Split multi-wait sync_info into preceding EventSemaphore instructions.

    The walrus build in this container lowers every instruction with capacity
    for a single sync-wait command and errors with "Too many sync wait
    commands" otherwise, while Tile attaches up to 3 waits per instruction.
    A wait carried by an EventSemaphore on the same engine immediately before
    the instruction is semantically identical. For DMAs, keep the own-lane
    FIFO wait on the instruction itself so the in-queue wait doesn't stall
    the sequencer.
    """
    import concourse.mybir as mybir

    fn = nc.m.functions[0]
    for blk in fn.blocks:
        new_insts = []
        for inst in blk.instructions:
            si = inst.sync_info
            if si is not None and si.on_wait is not None and len(si.on_wait) > max_waits:
                ow = list(si.on_wait)
                upd_ids = {u.id for u in (si.on_update or [])}
                keep = [w for w in ow if w.id in upd_ids][:1]
                if not keep:
                    keep = [ow[-1]]
                for j, w in enumerate(ow):
                    if w is keep[0]:
                        continue
                    new_insts.append(
                        mybir.InstEventSemaphore(
                            name=f"{inst.name}-ws{j}",
                            opcode="EventSemaphore",
                            engine=inst.engine,
                            sync_info=mybir.SyncInfo(on_wait=[w], on_update=[]),
                        )
                    )
                si.on_wait = keep
            new_insts.append(inst)
        blk.instructions = new_insts


def _chunk_list(S: int):
    """Equalized moving chunks: k = ceil(S/512), base = ceil(S/k) rounded to
    a multiple of 4, so every chunk is large enough that LDWEIGHTS hides
    under the moving phase."""
    k = -(-S // CHUNK_MAX)
    base = -(-S // k)
    base = -(-base // 4) * 4
    chunks = []
    t0 = 0
    while t0 < S:
        c = min(base, S - t0)
        chunks.append((t0, c))
        t0 += c
    return chunks


def _build_bass(slot_spec: tuple, D: int, Fh: int):
    """Bass program: sequence of expert slots, each (S tokens, ty) with
    ty 'b' (bf16) or 'f' (fp8 e4m3 DoubleRow). Per-slot inputs x{j}, w1{j},
    w2{j}; outputs y{j}."""
    import concourse.bass as bass
    import concourse.mybir as mybir
    import concourse.tile as tile

    ND = D // P           # 16 contraction tiles (phase 1)
    NF = Fh // P          # 14 f-tiles per half
    NPR = NF // 2         # 7 w1 pairs
    NDP = ND // 2         # 8 dt-pair groups
    assert NF % 2 == 0 and ND % 2 == 0
    S1b = max((S for S, ty in slot_spec if ty == "b"), default=8)
    S1f = max((S for S, ty in slot_spec if ty == "f"), default=8)

    bf16 = mybir.dt.bfloat16
    f8 = mybir.dt.float8e4
    f32 = mybir.dt.float32
    DR = mybir.MatmulPerfMode.DoubleRow

    nc = bass.Bass(name="moe_pair_ffn", num_swdge_queues=4)
    xs_d, w1s_d, w2s_d, ys_d = [], [], [], []
    for j, (S, ty) in enumerate(slot_spec):
        if ty == "b":
            xs_d.append(nc.dram_tensor(f"x{j}", [ND, P, S], bf16, kind="ExternalInput"))
            w1s_d.append(
                nc.dram_tensor(
                    f"w1{j}", [2, NPR, P, 2, ND, P], bf16, kind="ExternalInput"
                )
            )
            w2s_d.append(
                nc.dram_tensor(f"w2{j}", [NDP, P, 2, NF, P], bf16, kind="ExternalInput")
            )
        else:
            xs_d.append(nc.dram_tensor(f"x{j}", [ND, P, S], f8, kind="ExternalInput"))
            w1s_d.append(
                nc.dram_tensor(
                    f"w1{j}", [2, NPR, P, 2, NDP, 2, P], f8, kind="ExternalInput"
                )
            )
            w2s_d.append(
                nc.dram_tensor(
                    f"w2{j}", [NDP, P, 2, NPR, 2, P], f8, kind="ExternalInput"
                )
            )
        ys_d.append(nc.dram_tensor(f"y{j}", [ND, P, S], bf16, kind="ExternalOutput"))

    act_silu = mybir.ActivationFunctionType.Silu
    act_copy = mybir.ActivationFunctionType.Copy

    with tile.TileContext(nc) as tc:
        with (
            tc.tile_pool(name="xp", bufs=1) as xpool,
            tc.tile_pool(name="ap", bufs=1) as apool,
            tc.tile_pool(name="w1p", bufs=4) as w1pool,
            tc.tile_pool(name="w2p", bufs=3) as w2pool,
            tc.tile_pool(name="hp", bufs=4) as hpool,
            tc.tile_pool(name="yp", bufs=3) as ypool,
            tc.tile_pool(name="ps", bufs=8, space="PSUM") as psum,
        ):
            x_sb = xpool.tile([P, ND, S1b], bf16)
            aT = apool.tile([P, NF, S1b], bf16)
            has_f8 = any(ty == "f" for _, ty in slot_spec)
            if has_f8:
                x8_sb = xpool.tile([P, ND, S1f], f8)
                aT8 = apool.tile([P, NF, S1f], f8)

            for j, (S, ty) in enumerate(slot_spec):
                chunks = _chunk_list(S)
                x_d, w1_d, w2_d, y_d = xs_d[j], w1s_d[j], w2s_d[j], ys_d[j]

                if ty == "b":
                    # ---------------- bf16 slot ----------------
                    xsplit = -(-S // 8) * 4
                    startup = (
                        j == 0
                        and len(chunks) >= 3
                        and chunks[1][0] + chunks[1][1] <= xsplit
                    )
                    w1g0 = w1pool.tile([P, 2, ND, P], bf16, tag="w1", name="w1g0")
                    w1u0 = w1pool.tile([P, 2, ND, P], bf16, tag="w1", name="w1u0")
                    if startup:
                        # queue order: 0.5MB of w1 (gate/jj0), x chunk0
                        # dt-major, rest of pair0, x chunk1 — the dt-outer
                        # sub-phases below consume in exactly this order.
                        nc.gpsimd.dma_start(w1g0[:, 0], w1_d[0, 0, :, 0])
                        for dt in range(ND):
                            nc.gpsimd.dma_start(
                                x_sb[:, dt, :xsplit], x_d[dt, :, :xsplit]
                            )
                        nc.gpsimd.dma_start(w1g0[:, 1], w1_d[0, 0, :, 1])
                        nc.gpsimd.dma_start(w1u0[:, 0], w1_d[1, 0, :, 0])
                        nc.gpsimd.dma_start(w1u0[:, 1], w1_d[1, 0, :, 1])
                        for dt in range(ND):
                            nc.gpsimd.dma_start(
                                x_sb[:, dt, xsplit:S], x_d[dt, :, xsplit:]
                            )
                    else:
                        nc.gpsimd.dma_start(w1g0, w1_d[0, 0])
                        nc.gpsimd.dma_start(w1u0, w1_d[1, 0])
                        for dt in range(ND):
                            nc.gpsimd.dma_start(x_sb[:, dt, :S], x_d[dt])

                    def ph1_chain(w1t, jj, t0, csz, start, stop, ps):
                        for dt in range(ND):
                            nc.tensor.matmul(
                                ps[:, :csz],
                                w1t[:, jj, dt, :],
                                x_sb[:, dt, t0 : t0 + csz],
                                start=start and dt == 0,
                                stop=stop and dt == ND - 1,
                            )

                    def ph1_evac(fi, t0, csz, ps_g, ps_u):
                        hg = hpool.tile([P, CHUNK_MAX], bf16, tag="h", name="hg")
                        nc.scalar.activation(hg[:, :csz], ps_g[:, :csz], act_silu)
                        nc.vector.tensor_mul(
                            aT[:, fi, t0 : t0 + csz], hg[:, :csz], ps_u[:, :csz]
                        )

                    pr_start = 0
                    if startup:
                        sc = chunks[:2]
                        banks = {}
                        for gu in range(2):
                            for jj in range(2):
                                for ci in range(len(sc)):
                                    banks[(jj, ci, gu)] = psum.tile(
                                        [P, CHUNK_MAX], f32, tag="ps",
                                        name=f"ps_s{jj}{ci}{gu}",
                                    )
                        for gu, jj in ((0, 0), (0, 1), (1, 0), (1, 1)):
                            w1t = w1g0 if gu == 0 else w1u0
                            for dt in range(ND):
                                for ci, (t0, csz) in enumerate(sc):
                                    nc.tensor.matmul(
                                        banks[(jj, ci, gu)][:, :csz],
                                        w1t[:, jj, dt, :],
                                        x_sb[:, dt, t0 : t0 + csz],
                                        start=(dt == 0),
                                        stop=(dt == ND - 1),
                                    )
                            if gu == 1:
                                for ci, (t0, csz) in enumerate(sc):
                                    ph1_evac(
                                        jj, t0, csz,
                                        banks[(jj, ci, 0)], banks[(jj, ci, 1)],
                                    )
                        for jj in range(2):
                            for t0, csz in chunks[2:]:
                                ps_g = psum.tile(
                                    [P, CHUNK_MAX], f32, tag="ps", name="ps_g"
                                )
                                ph1_chain(w1g0, jj, t0, csz, True, True, ps_g)
                                ps_u = psum.tile(
                                    [P, CHUNK_MAX], f32, tag="ps", name="ps_u"
                                )
                                ph1_chain(w1u0, jj, t0, csz, True, True, ps_u)
                                ph1_evac(jj, t0, csz, ps_g, ps_u)
                        pr_start = 1

                    for pr in range(pr_start, NPR):
                        if pr == 0:
                            w1g, w1u = w1g0, w1u0
                        else:
                            w1g = w1pool.tile([P, 2, ND, P], bf16, tag="w1", name="w1g")
                            nc.gpsimd.dma_start(w1g, w1_d[0, pr])
                            w1u = w1pool.tile([P, 2, ND, P], bf16, tag="w1", name="w1u")
                            nc.gpsimd.dma_start(w1u, w1_d[1, pr])
                        for jj in range(2):
                            for t0, csz in chunks:
                                ps_g = psum.tile(
                                    [P, CHUNK_MAX], f32, tag="ps", name="ps_g"
                                )
                                ph1_chain(w1g, jj, t0, csz, True, True, ps_g)
                                ps_u = psum.tile(
                                    [P, CHUNK_MAX], f32, tag="ps", name="ps_u"
                                )
                                ph1_chain(w1u, jj, t0, csz, True, True, ps_u)
                                ph1_evac(2 * pr + jj, t0, csz, ps_g, ps_u)

                    for dp in range(NDP):
                        w2sb = w2pool.tile([P, 2, NF, P], bf16, tag="w2", name="w2sb")
                        nc.gpsimd.dma_start(w2sb, w2_d[dp])
                        for dj in range(2):
                            dt = 2 * dp + dj
                            y_st = ypool.tile([P, S1b], bf16, tag="y", name="y_st")
                            for t0, csz in chunks:
                                ps_y = psum.tile(
                                    [P, CHUNK_MAX], f32, tag="ps", name="ps_y"
                                )
                                for fi in range(NF):
                                    nc.tensor.matmul(
                                        ps_y[:, :csz],
                                        w2sb[:, dj, fi, :],
                                        aT[:, fi, t0 : t0 + csz],
                                        start=(fi == 0),
                                        stop=(fi == NF - 1),
                                    )
                                nc.scalar.activation(
                                    y_st[:, t0 : t0 + csz], ps_y[:, :csz], act_copy
                                )
                            nc.gpsimd.dma_start(y_d[dt], y_st[:, :S])
                else:
                    # ---------------- fp8 slot (DoubleRow) ----------------
                    for dt in range(ND):
                        nc.gpsimd.dma_start(x8_sb[:, dt, :S], x_d[dt])

                    for pr in range(NPR):
                        w1g = w1pool.tile(
                            [P, 2, NDP, 2, P], f8, tag="w18", name="w18g"
                        )
                        nc.gpsimd.dma_start(w1g, w1_d[0, pr])
                        w1u = w1pool.tile(
                            [P, 2, NDP, 2, P], f8, tag="w18", name="w18u"
                        )
                        nc.gpsimd.dma_start(w1u, w1_d[1, pr])
                        for jj in range(2):
                            fi = 2 * pr + jj
                            for t0, csz in chunks:
                                ps_g = psum.tile(
                                    [P, CHUNK_MAX], f32, tag="ps", name="ps_g"
                                )
                                for dp in range(NDP):
                                    nc.tensor.matmul(
                                        ps_g[:, :csz],
                                        w1g[:, jj, dp],
                                        x8_sb[:, 2 * dp : 2 * dp + 2, t0 : t0 + csz],
                                        start=(dp == 0),
                                        stop=(dp == NDP - 1),
                                        perf_mode=DR,
                                    )
                                ps_u = psum.tile(
                                    [P, CHUNK_MAX], f32, tag="ps", name="ps_u"
                                )
                                for dp in range(NDP):
                                    nc.tensor.matmul(
                                        ps_u[:, :csz],
                                        w1u[:, jj, dp],
                                        x8_sb[:, 2 * dp : 2 * dp + 2, t0 : t0 + csz],
                                        start=(dp == 0),
                                        stop=(dp == NDP - 1),
                                        perf_mode=DR,
                                    )
                                hg = hpool.tile([P, CHUNK_MAX], bf16, tag="h", name="hg")
                                nc.scalar.activation(
                                    hg[:, :csz], ps_g[:, :csz], act_silu,
                                    scale=1.0 / (X8S * W1S),
                                )
                                hu = hpool.tile([P, CHUNK_MAX], bf16, tag="h", name="hu")
                                nc.scalar.activation(
                                    hu[:, :csz], ps_u[:, :csz], act_copy,
                                    scale=X8S / (X8S * W1S),
                                )
                                nc.vector.tensor_mul(
                                    aT8[:, fi, t0 : t0 + csz], hg[:, :csz], hu[:, :csz]
                                )

                    for dp in range(NDP):
                        w2sb = w2pool.tile(
                            [P, 2, NPR, 2, P], f8, tag="w28", name="w28sb"
                        )
                        nc.gpsimd.dma_start(w2sb, w2_d[dp])
                        for dj in range(2):
                            dt = 2 * dp + dj
                            y_st = ypool.tile([P, S1b], bf16, tag="y", name="y_st")
                            for t0, csz in chunks:
                                ps_y = psum.tile(
                                    [P, CHUNK_MAX], f32, tag="ps", name="ps_y"
                                )
                                for q in range(NPR):
                                    nc.tensor.matmul(
                                        ps_y[:, :csz],
                                        w2sb[:, dj, q],
                                        aT8[:, 2 * q : 2 * q + 2, t0 : t0 + csz],
                                        start=(q == 0),
                                        stop=(q == NPR - 1),
                                        perf_mode=DR,
                                    )
                                nc.scalar.activation(
                                    y_st[:, t0 : t0 + csz], ps_y[:, :csz], act_copy
                                )
                            nc.gpsimd.dma_start(y_d[dt], y_st[:, :S])

    _legalize_sync(nc)
    return nc


def _ensure_ntff_hook():
    """Register the axon NTFF-profile hook if the image's antenv lacks
    ``axon_hooks`` (the hook impl ships in trn_agent_boot). Best-effort."""
    import sys
    import types

    try:
        from antenv.axon_hooks import get_axon_ntff_profile_hook  # noqa: F401

        return
    except ImportError:
        pass
    try:
        import antenv

        mod = types.ModuleType("antenv.axon_hooks")
        mod._hook = None

        def set_axon_ntff_profile_hook(h):
            mod._hook = h

        def get_axon_ntff_profile_hook():
            return mod._hook

        mod.set_axon_ntff_profile_hook = set_axon_ntff_profile_hook
        mod.get_axon_ntff_profile_hook = get_axon_ntff_profile_hook
        sys.modules["antenv.axon_hooks"] = mod
        antenv.axon_hooks = mod

        from trn_agent_boot.trn_boot import _ntff_profile_via_ctypes

        so_path = "/opt/axon/libaxon_pjrt.so"
        hook = _ntff_profile_via_ctypes(so_path)
        if hook is not None:
            mod._hook = hook
    except Exception:
        pass


def _route(x, gate_w, top_k):
    """Replicates the reference router in numpy fp32 (renormalized top-k
    softmax == softmax over the top-k logits)."""
    logits = x.astype(np.float32) @ gate_w.astype(np.float32).T  # [T, E]
    k = int(top_k)
    idx = np.argpartition(-logits, k - 1, axis=1)[:, :k]
    lv = np.take_along_axis(logits, idx, axis=1)
    m = lv.max(axis=1, keepdims=True)
    ew = np.exp(lv - m)
    wts = ew / ew.sum(axis=1, keepdims=True)
    return idx, wts.astype(np.float32)


def _make_slots(hi_counts, lo_counts):
    """Cut experts into <=SLOT_CAP segments per precision; LPT-pack onto 4
    core pairs (fp8 cost 0.5/col); return per-pair slot lists
    [(expert, lo, hi, ty), ...] (bf16 slots first) and the shared slot-spec
    tuple ((S, ty), ...) with S = max across pairs, padded to mult of 4."""
    segs = []
    for e, c in enumerate(hi_counts):
        t = 0
        while t < c:
            s = min(SLOT_CAP, c - t)
            segs.append((s, 1.0, e, t, t + s, "b"))
            t += s
    for e, c in enumerate(lo_counts):
        base = hi_counts[e]
        t = 0
        while t < c:
            s = min(SLOT_CAP, c - t)
            segs.append((s, 0.5, e, base + t, base + t + s, "f"))
            t += s
    # bf16 first (descending), then fp8 (descending), onto least-loaded pair
    segs.sort(key=lambda g: (g[5], -g[0]))
    loads = [0.0] * 4
    pair_slots = [[] for _ in range(4)]
    for s, cost, e, lo, hi, ty in segs:
        p = min(range(4), key=lambda i: loads[i])
        loads[p] += s * cost
        pair_slots[p].append((e, lo, hi, ty))
    for sl in pair_slots:
        sl.sort(key=lambda t: (t[3], -(t[2] - t[1])))
    kb = max(sum(1 for t in sl if t[3] == "b") for sl in pair_slots)
    kf = max(sum(1 for t in sl if t[3] == "f") for sl in pair_slots)
    # normalize: every pair gets kb bf slots then kf f8 slots (dummies empty)
    spec = []
    norm = [[] for _ in range(4)]
    for ji in range(kb + kf):
        ty = "b" if ji < kb else "f"
        m = 8
        for p in range(4):
            mine = [t for t in pair_slots[p] if t[3] == ty]
            i = ji if ty == "b" else ji - kb
            if i < len(mine):
                norm[p].append(mine[i])
                m = max(m, mine[i][2] - mine[i][1])
            else:
                norm[p].append(None)
        spec.append((-(-m // 4) * 4, ty))
    return norm, tuple(spec)


def kernel(x, gate_w, wv1, w2, top_k):
    import ml_dtypes

    from concourse.bass_utils import run_bass_kernel_spmd

    x = np.asarray(x)
    gate_w = np.asarray(gate_w)
    wv1 = np.asarray(wv1)
    w2 = np.asarray(w2)

    T, D = x.shape
    E, F2, _ = wv1.shape
    F = F2 // 2
    Fh = F // 2
    ND = D // P
    NF = Fh // P
    NPR = NF // 2
    NDP = ND // 2
    n_cores = 8

    idx, wts = _route(x, gate_w, top_k)

    rows_l, w_l, hi_n = [], [], []
    for e in range(E):
        rows, cols = np.nonzero(idx == e)
        w_e = wts[rows, cols]
        order = np.argsort(-w_e, kind="stable")
        rows_l.append(rows[order])
        w_l.append(w_e[order])
        hi_n.append(int((w_e >= THETA).sum()))
    counts = [len(r) for r in rows_l]
    lo_n = [c - h for c, h in zip(counts, hi_n)]

    pair_slots, slot_spec = _make_slots(hi_n, lo_n)

    key = (slot_spec, D, Fh)
    if key not in _BASS_CACHE:
        _BASS_CACHE[key] = _build_bass(slot_spec, D, Fh)
    nc = _BASS_CACHE[key]

    bf16 = ml_dtypes.bfloat16
    f8 = ml_dtypes.float8_e4m3
    x_bf = x.astype(bf16)
    x_f8 = np.clip(x * X8S, -240, 240).astype(f8)
    w1_bf = wv1.astype(bf16)
    w2_bf = w2.astype(bf16)

    w1_cache: dict = {}
    w2_cache: dict = {}

    def w1_pack(e, h, ty):
        if (e, h, ty) not in w1_cache:
            if ty == "b":
                gsl = w1_bf[e][h * Fh : (h + 1) * Fh]
                usl = w1_bf[e][F + h * Fh : F + (h + 1) * Fh]
                both = np.stack([gsl, usl])  # [2, Fh, D]
                w1_cache[(e, h, ty)] = np.ascontiguousarray(
                    both.reshape(2, NPR, 2, P, ND, P).transpose(0, 1, 5, 2, 4, 3)
                )
            else:
                gsl = wv1[e][h * Fh : (h + 1) * Fh]
                usl = wv1[e][F + h * Fh : F + (h + 1) * Fh]
                both = np.clip(np.stack([gsl, usl]) * W1S, -240, 240).astype(f8)
                w1_cache[(e, h, ty)] = np.ascontiguousarray(
                    both.reshape(2, NPR, 2, P, NDP, 2, P).transpose(0, 1, 6, 2, 4, 5, 3)
                )
        return w1_cache[(e, h, ty)]

    def w2_pack(e, h, ty):
        if (e, h, ty) not in w2_cache:
            if ty == "b":
                sl = w2_bf[e][:, h * Fh : (h + 1) * Fh]  # [D, Fh]
                w2_cache[(e, h, ty)] = np.ascontiguousarray(
                    sl.reshape(NDP, 2, P, NF, P).transpose(0, 4, 1, 3, 2)
                )
            else:
                sl = np.clip(w2[e][:, h * Fh : (h + 1) * Fh] * W2S, -240, 240).astype(f8)
                w2_cache[(e, h, ty)] = np.ascontiguousarray(
                    sl.reshape(NDP, 2, P, NPR, 2, P).transpose(0, 5, 1, 3, 4, 2)
                )
        return w2_cache[(e, h, ty)]

    in_maps = []
    for p in range(4):
        slots = pair_slots[p]
        for h in range(2):
            im = {}
            for ji, (S, ty) in enumerate(slot_spec):
                slot = slots[ji]
                if ty == "b":
                    xq, w1z, w2z = (
                        np.zeros((D, S), dtype=bf16),
                        np.zeros((2, NPR, P, 2, ND, P), dtype=bf16),
                        np.zeros((NDP, P, 2, NF, P), dtype=bf16),
                    )
                else:
                    xq, w1z, w2z = (
                        np.zeros((D, S), dtype=f8),
                        np.zeros((2, NPR, P, 2, NDP, 2, P), dtype=f8),
                        np.zeros((NDP, P, 2, NPR, 2, P), dtype=f8),
                    )
                if slot is not None:
                    e, lo, hi, _ = slot
                    seg = rows_l[e][lo:hi]
                    src = x_bf if ty == "b" else x_f8
                    xq[:, : hi - lo] = src[seg].T
                    w1z = w1_pack(e, h, ty)
                    w2z = w2_pack(e, h, ty)
                im[f"x{ji}"] = np.ascontiguousarray(xq.reshape(ND, P, S))
                im[f"w1{ji}"] = w1z
                im[f"w2{ji}"] = w2z
            in_maps.append(im)

    _ensure_ntff_hook()
    res = run_bass_kernel_spmd(nc, in_maps, core_ids=list(range(n_cores)))
    global last_run
    last_run = res

    out = np.zeros((T, D), dtype=np.float32)
    for p in range(4):
        for ji, (S, ty) in enumerate(slot_spec):
            slot = pair_slots[p][ji]
            if slot is None:
                continue
            e, lo, hi, _ = slot
            n = hi - lo
            y0 = res.results[2 * p][f"y{ji}"].reshape(D, -1)[:, :n]
            y1 = res.results[2 * p + 1][f"y{ji}"].reshape(D, -1)[:, :n]
            ysum = y0.astype(np.float32) + y1.astype(np.float32)
            if ty == "f":
                ysum *= 1.0 / Y8S
            seg = rows_l[e][lo:hi]
            out[seg] += w_l[e][lo:hi, None] * ysum.T
    return out.astype(x.dtype, copy=False)


# revision 9
# speedup vs baseline: 1.2264x; 1.2264x over previous
"""Block-sparse MoE (softmax top-k routing + silu-gated FFN) on 8 Trainium2 cores.

Sharding: expert-pair x FFN-half. The router runs on host; each expert's
token list is sorted by router weight and split at THETA: high-weight
pairs run in bf16, low-weight pairs in fp8 e4m3 (DoubleRow, 2x tensor
rate) — the output error each fp8 token contributes is proportional to
its (small) router weight, keeping total rel err ~1.2e-2 vs the 2e-2
gate. Segments are LPT-packed onto 4 core-pairs (fp8 cost 0.5/col) and
each pair of cores splits the FFN dim in half (1792 of 3584), so all 8
cores run the same slot-size program (SPMD) with balanced cycles.

Per slot on a core (S tokens, ND=16 d-tiles, NFh=14 f-tiles):
  phase1: h.T [128 f, chunk] = w1h.T @ x (contract D); silu(g)*u -> aT
  phase3: y[d, tok] = sum_fi w2h[f,d].T @ aT[:, fi, :] (contract F/2)
          d on PSUM partitions, tokens moving => exact columns, no pad.
  fp8 slots: x*16, w1*32 -> psum = 512*h; silu via ACT scale 1/512;
  u16 = psum/32; a8 = e4m3(silu*u16) = 16a; w2*64 -> y*1024 (host /1024).

Moving chunks are equalized (~420-510) so LDWEIGHTS (~97-116ns) hides
under the moving phase. Slot0's x arrives dt-major in two column chunks
consumed dt-outer across 8 PSUM banks (PE starts ~12us in). w1 prefetch
depth 4 covers the ~6us pair-tile transfer. All bulk traffic rides the
SWDGE queue with >=2KB lines; later slots' x and w1 overlap prior ph3.
Host sums the two F-halves, applies router weight, scatter-adds (f32).
"""

import numpy as np


def _ensure_concourse_on_path():
    try:
        import concourse  # noqa: F401
    except ImportError:
        import sys

        for p in ("/opt/trn_rl_repo", "/root/.axon_site/_ro/trn_rl_repo"):
            if p not in sys.path:
                sys.path.insert(0, p)


_ensure_concourse_on_path()

P = 128
CHUNK_MAX = 512   # PSUM bank free-dim limit (f32)
SLOT_CAP = 2560   # max tokens per slot (SBUF-bound)
THETA = 0.33      # router-weight cutoff: below -> fp8 path
X8S, W1S, W2S = 16.0, 32.0, 64.0
Y8S = X8S * W2S   # fp8-slot output scale (host divides)

_BASS_CACHE: dict = {}
last_run = None  # BassKernelResults of the most recent kernel() call (for test.py)


def _legalize_sync(nc, max_waits: int = 1):
    """Split multi-wait sync_info into preceding EventSemaphore instructions.

    The walrus build in this container lowers every instruction with capacity
    for a single sync-wait command and errors with "Too many sync wait
    commands" otherwise, while Tile attaches up to 3 waits per instruction.
    A wait carried by an EventSemaphore on the same engine immediately before
    the instruction is semantically identical. For DMAs, keep the own-lane
    FIFO wait on the instruction itself so the in-queue wait doesn't stall
    the sequencer.
    """
    import concourse.mybir as mybir

    fn = nc.m.functions[0]
    for blk in fn.blocks:
        new_insts = []
        for inst in blk.instructions:
            si = inst.sync_info
            if si is not None and si.on_wait is not None and len(si.on_wait) > max_waits:
                ow = list(si.on_wait)
                upd_ids = {u.id for u in (si.on_update or [])}
                keep = [w for w in ow if w.id in upd_ids][:1]
                if not keep:
                    keep = [ow[-1]]
                for j, w in enumerate(ow):
                    if w is keep[0]:
                        continue
                    new_insts.append(
                        mybir.InstEventSemaphore(
                            name=f"{inst.name}-ws{j}",
                            opcode="EventSemaphore",
                            engine=inst.engine,
                            sync_info=mybir.SyncInfo(on_wait=[w], on_update=[]),
                        )
                    )
                si.on_wait = keep
            new_insts.append(inst)
        blk.instructions = new_insts


def _chunk_list(S: int):
    """Equalized moving chunks: k = ceil(S/512), base = ceil(S/k) rounded to
    a multiple of 4, so every chunk is large enough that LDWEIGHTS hides
    under the moving phase."""
    k = -(-S // CHUNK_MAX)
    base = -(-S // k)
    base = -(-base // 4) * 4
    chunks = []
    t0 = 0
    while t0 < S:
        c = min(base, S - t0)
        chunks.append((t0, c))
        t0 += c
    return chunks


def _build_bass(slot_spec: tuple, D: int, Fh: int):
    """Bass program: sequence of expert slots, each (S tokens, ty) with
    ty 'b' (bf16) or 'f' (fp8 e4m3 DoubleRow). Per-slot inputs x{j}, w1{j},
    w2{j}; outputs y{j}."""
    import concourse.bass as bass
    import concourse.mybir as mybir
    import concourse.tile as tile

    ND = D // P           # 16 contraction tiles (phase 1)
    NF = Fh // P          # 14 f-tiles per half
    NPR = NF // 2         # 7 w1 pairs
    NDP = ND // 2         # 8 dt-pair groups
    assert NF % 2 == 0 and ND % 2 == 0
    S1b = max((S for S, ty in slot_spec if ty == "b"), default=8)
    S1f = max((S for S, ty in slot_spec if ty == "f"), default=8)

    bf16 = mybir.dt.bfloat16
    f8 = mybir.dt.float8e4
    f32 = mybir.dt.float32
    DR = mybir.MatmulPerfMode.DoubleRow

    nc = bass.Bass(name="moe_pair_ffn", num_swdge_queues=4)
    xs_d, w1s_d, w2s_d, ys_d = [], [], [], []
    for j, (S, ty) in enumerate(slot_spec):
        if ty == "b":
            xs_d.append(nc.dram_tensor(f"x{j}", [ND, P, S], bf16, kind="ExternalInput"))
            w1s_d.append(
                nc.dram_tensor(
                    f"w1{j}", [2, NPR, P, 2, ND, P], bf16, kind="ExternalInput"
                )
            )
            w2s_d.append(
                nc.dram_tensor(f"w2{j}", [NDP, P, 2, NF, P], bf16, kind="ExternalInput")
            )
        else:
            xs_d.append(nc.dram_tensor(f"x{j}", [ND, P, S], f8, kind="ExternalInput"))
            w1s_d.append(
                nc.dram_tensor(
                    f"w1{j}", [2, NPR, P, 2, NDP, 2, P], f8, kind="ExternalInput"
                )
            )
            w2s_d.append(
                nc.dram_tensor(
                    f"w2{j}", [NDP, P, 2, NPR, 2, P], f8, kind="ExternalInput"
                )
            )
        ys_d.append(nc.dram_tensor(f"y{j}", [ND, P, S], bf16, kind="ExternalOutput"))

    act_silu = mybir.ActivationFunctionType.Silu
    act_copy = mybir.ActivationFunctionType.Copy

    with tile.TileContext(nc) as tc:
        with (
            tc.tile_pool(name="xp", bufs=1) as xpool,
            tc.tile_pool(name="ap", bufs=1) as apool,
            tc.tile_pool(name="w1p", bufs=3) as w1pool,
            tc.tile_pool(name="w2p", bufs=3) as w2pool,
            tc.tile_pool(name="hp", bufs=4) as hpool,
            tc.tile_pool(name="yp", bufs=3) as ypool,
            tc.tile_pool(name="x8p", bufs=1) as x8pool,
            tc.tile_pool(name="w18p", bufs=3) as w18pool,
            tc.tile_pool(name="w28p", bufs=3) as w28pool,
            tc.tile_pool(name="ps", bufs=8, space="PSUM") as psum,
        ):
            # fp8 tiles live in their own pools, declared after the bf16
            # pools so the bf16 phase keeps the measured conflict-free
            # SBUF layout (shifting pool bases cost +35ns/matmul once).
            x_sb = xpool.tile([P, ND, S1b], bf16)
            aT = apool.tile([P, NF, S1b], bf16)
            has_f8 = any(ty == "f" for _, ty in slot_spec)
            if has_f8:
                x8_sb = x8pool.tile([P, ND, S1f], f8)
                aT8 = x8pool.tile([P, NF, S1f], f8)

            for j, (S, ty) in enumerate(slot_spec):
                chunks = _chunk_list(S)
                x_d, w1_d, w2_d, y_d = xs_d[j], w1s_d[j], w2s_d[j], ys_d[j]

                if ty == "b":
                    # ---------------- bf16 slot ----------------
                    xsplit = -(-S // 8) * 4
                    startup = (
                        j == 0
                        and len(chunks) >= 3
                        and chunks[1][0] + chunks[1][1] <= xsplit
                    )
                    w1g0 = w1pool.tile([P, 2, ND, P], bf16, tag="w1", name="w1g0")
                    w1u0 = w1pool.tile([P, 2, ND, P], bf16, tag="w1", name="w1u0")
                    if startup:
                        # queue order: 0.5MB of w1 (gate/jj0), x chunk0
                        # dt-major, rest of pair0, x chunk1 — the dt-outer
                        # sub-phases below consume in exactly this order.
                        nc.gpsimd.dma_start(w1g0[:, 0], w1_d[0, 0, :, 0])
                        for dt in range(ND):
                            nc.gpsimd.dma_start(
                                x_sb[:, dt, :xsplit], x_d[dt, :, :xsplit]
                            )
                        nc.gpsimd.dma_start(w1g0[:, 1], w1_d[0, 0, :, 1])
                        nc.gpsimd.dma_start(w1u0[:, 0], w1_d[1, 0, :, 0])
                        nc.gpsimd.dma_start(w1u0[:, 1], w1_d[1, 0, :, 1])
                        for dt in range(ND):
                            nc.gpsimd.dma_start(
                                x_sb[:, dt, xsplit:S], x_d[dt, :, xsplit:]
                            )
                    else:
                        nc.gpsimd.dma_start(w1g0, w1_d[0, 0])
                        nc.gpsimd.dma_start(w1u0, w1_d[1, 0])
                        for dt in range(ND):
                            nc.gpsimd.dma_start(x_sb[:, dt, :S], x_d[dt])

                    def ph1_chain(w1t, jj, t0, csz, start, stop, ps):
                        for dt in range(ND):
                            nc.tensor.matmul(
                                ps[:, :csz],
                                w1t[:, jj, dt, :],
                                x_sb[:, dt, t0 : t0 + csz],
                                start=start and dt == 0,
                                stop=stop and dt == ND - 1,
                            )

                    def ph1_evac(fi, t0, csz, ps_g, ps_u):
                        hg = hpool.tile([P, CHUNK_MAX], bf16, tag="h", name="hg")
                        nc.scalar.activation(hg[:, :csz], ps_g[:, :csz], act_silu)
                        nc.vector.tensor_mul(
                            aT[:, fi, t0 : t0 + csz], hg[:, :csz], ps_u[:, :csz]
                        )

                    pr_start = 0
                    if startup:
                        sc = chunks[:2]
                        banks = {}
                        for gu in range(2):
                            for jj in range(2):
                                for ci in range(len(sc)):
                                    banks[(jj, ci, gu)] = psum.tile(
                                        [P, CHUNK_MAX], f32, tag="ps",
                                        name=f"ps_s{jj}{ci}{gu}",
                                    )
                        for gu, jj in ((0, 0), (0, 1), (1, 0), (1, 1)):
                            w1t = w1g0 if gu == 0 else w1u0
                            for dt in range(ND):
                                for ci, (t0, csz) in enumerate(sc):
                                    nc.tensor.matmul(
                                        banks[(jj, ci, gu)][:, :csz],
                                        w1t[:, jj, dt, :],
                                        x_sb[:, dt, t0 : t0 + csz],
                                        start=(dt == 0),
                                        stop=(dt == ND - 1),
                                    )
                            if gu == 1:
                                for ci, (t0, csz) in enumerate(sc):
                                    ph1_evac(
                                        jj, t0, csz,
                                        banks[(jj, ci, 0)], banks[(jj, ci, 1)],
                                    )
                        for jj in range(2):
                            for t0, csz in chunks[2:]:
                                ps_g = psum.tile(
                                    [P, CHUNK_MAX], f32, tag="ps", name="ps_g"
                                )
                                ph1_chain(w1g0, jj, t0, csz, True, True, ps_g)
                                ps_u = psum.tile(
                                    [P, CHUNK_MAX], f32, tag="ps", name="ps_u"
                                )
                                ph1_chain(w1u0, jj, t0, csz, True, True, ps_u)
                                ph1_evac(jj, t0, csz, ps_g, ps_u)
                        pr_start = 1

                    for pr in range(pr_start, NPR):
                        if pr == 0:
                            w1g, w1u = w1g0, w1u0
                        else:
                            w1g = w1pool.tile([P, 2, ND, P], bf16, tag="w1", name="w1g")
                            nc.gpsimd.dma_start(w1g, w1_d[0, pr])
                            w1u = w1pool.tile([P, 2, ND, P], bf16, tag="w1", name="w1u")
                            nc.gpsimd.dma_start(w1u, w1_d[1, pr])
                        for jj in range(2):
                            for t0, csz in chunks:
                                ps_g = psum.tile(
                                    [P, CHUNK_MAX], f32, tag="ps", name="ps_g"
                                )
                                ph1_chain(w1g, jj, t0, csz, True, True, ps_g)
                                ps_u = psum.tile(
                                    [P, CHUNK_MAX], f32, tag="ps", name="ps_u"
                                )
                                ph1_chain(w1u, jj, t0, csz, True, True, ps_u)
                                ph1_evac(2 * pr + jj, t0, csz, ps_g, ps_u)

                    for dp in range(NDP):
                        w2sb = w2pool.tile([P, 2, NF, P], bf16, tag="w2", name="w2sb")
                        nc.gpsimd.dma_start(w2sb, w2_d[dp])
                        for dj in range(2):
                            dt = 2 * dp + dj
                            y_st = ypool.tile([P, S1b], bf16, tag="y", name="y_st")
                            for t0, csz in chunks:
                                ps_y = psum.tile(
                                    [P, CHUNK_MAX], f32, tag="ps", name="ps_y"
                                )
                                for fi in range(NF):
                                    nc.tensor.matmul(
                                        ps_y[:, :csz],
                                        w2sb[:, dj, fi, :],
                                        aT[:, fi, t0 : t0 + csz],
                                        start=(fi == 0),
                                        stop=(fi == NF - 1),
                                    )
                                nc.scalar.activation(
                                    y_st[:, t0 : t0 + csz], ps_y[:, :csz], act_copy
                                )
                            nc.gpsimd.dma_start(y_d[dt], y_st[:, :S])
                else:
                    # ---------------- fp8 slot (DoubleRow) ----------------
                    for dt in range(ND):
                        nc.gpsimd.dma_start(x8_sb[:, dt, :S], x_d[dt])

                    for pr in range(NPR):
                        w1g = w18pool.tile(
                            [P, 2, NDP, 2, P], f8, tag="w18", name="w18g"
                        )
                        nc.gpsimd.dma_start(w1g, w1_d[0, pr])
                        w1u = w18pool.tile(
                            [P, 2, NDP, 2, P], f8, tag="w18", name="w18u"
                        )
                        nc.gpsimd.dma_start(w1u, w1_d[1, pr])
                        for jj in range(2):
                            fi = 2 * pr + jj
                            for t0, csz in chunks:
                                ps_g = psum.tile(
                                    [P, CHUNK_MAX], f32, tag="ps", name="ps_g"
                                )
                                for dp in range(NDP):
                                    nc.tensor.matmul(
                                        ps_g[:, :csz],
                                        w1g[:, jj, dp],
                                        x8_sb[:, 2 * dp : 2 * dp + 2, t0 : t0 + csz],
                                        start=(dp == 0),
                                        stop=(dp == NDP - 1),
                                        perf_mode=DR,
                                    )
                                ps_u = psum.tile(
                                    [P, CHUNK_MAX], f32, tag="ps", name="ps_u"
                                )
                                for dp in range(NDP):
                                    nc.tensor.matmul(
                                        ps_u[:, :csz],
                                        w1u[:, jj, dp],
                                        x8_sb[:, 2 * dp : 2 * dp + 2, t0 : t0 + csz],
                                        start=(dp == 0),
                                        stop=(dp == NDP - 1),
                                        perf_mode=DR,
                                    )
                                hg = hpool.tile([P, CHUNK_MAX], bf16, tag="h", name="hg")
                                nc.scalar.activation(
                                    hg[:, :csz], ps_g[:, :csz], act_silu,
                                    scale=1.0 / (X8S * W1S),
                                )
                                hu = hpool.tile([P, CHUNK_MAX], bf16, tag="h", name="hu")
                                nc.scalar.activation(
                                    hu[:, :csz], ps_u[:, :csz], act_copy,
                                    scale=X8S / (X8S * W1S),
                                )
                                nc.vector.tensor_mul(
                                    aT8[:, fi, t0 : t0 + csz], hg[:, :csz], hu[:, :csz]
                                )

                    for dp in range(NDP):
                        w2sb = w28pool.tile(
                            [P, 2, NPR, 2, P], f8, tag="w28", name="w28sb"
                        )
                        nc.gpsimd.dma_start(w2sb, w2_d[dp])
                        for dj in range(2):
                            dt = 2 * dp + dj
                            y_st = ypool.tile([P, S1b], bf16, tag="y", name="y_st")
                            for t0, csz in chunks:
                                ps_y = psum.tile(
                                    [P, CHUNK_MAX], f32, tag="ps", name="ps_y"
                                )
                                for q in range(NPR):
                                    nc.tensor.matmul(
                                        ps_y[:, :csz],
                                        w2sb[:, dj, q],
                                        aT8[:, 2 * q : 2 * q + 2, t0 : t0 + csz],
                                        start=(q == 0),
                                        stop=(q == NPR - 1),
                                        perf_mode=DR,
                                    )
                                nc.scalar.activation(
                                    y_st[:, t0 : t0 + csz], ps_y[:, :csz], act_copy
                                )
                            nc.gpsimd.dma_start(y_d[dt], y_st[:, :S])

    _legalize_sync(nc)
    return nc


def _ensure_ntff_hook():
    """Register the axon NTFF-profile hook if the image's antenv lacks
    ``axon_hooks`` (the hook impl ships in trn_agent_boot). Best-effort."""
    import sys
    import types

    try:
        from antenv.axon_hooks import get_axon_ntff_profile_hook  # noqa: F401

        return
    except ImportError:
        pass
    try:
        import antenv

        mod = types.ModuleType("antenv.axon_hooks")
        mod._hook = None

        def set_axon_ntff_profile_hook(h):
            mod._hook = h

        def get_axon_ntff_profile_hook():
            return mod._hook

        mod.set_axon_ntff_profile_hook = set_axon_ntff_profile_hook
        mod.get_axon_ntff_profile_hook = get_axon_ntff_profile_hook
        sys.modules["antenv.axon_hooks"] = mod
        antenv.axon_hooks = mod

        from trn_agent_boot.trn_boot import _ntff_profile_via_ctypes

        so_path = "/opt/axon/libaxon_pjrt.so"
        hook = _ntff_profile_via_ctypes(so_path)
        if hook is not None:
            mod._hook = hook
    except Exception:
        pass


def _route(x, gate_w, top_k):
    """Replicates the reference router in numpy fp32 (renormalized top-k
    softmax == softmax over the top-k logits)."""
    logits = x.astype(np.float32) @ gate_w.astype(np.float32).T  # [T, E]
    k = int(top_k)
    idx = np.argpartition(-logits, k - 1, axis=1)[:, :k]
    lv = np.take_along_axis(logits, idx, axis=1)
    m = lv.max(axis=1, keepdims=True)
    ew = np.exp(lv - m)
    wts = ew / ew.sum(axis=1, keepdims=True)
    return idx, wts.astype(np.float32)


def _make_slots(hi_counts, lo_counts):
    """Cut experts into <=SLOT_CAP segments per precision; LPT-pack onto 4
    core pairs (fp8 cost 0.5/col); return per-pair slot lists
    [(expert, lo, hi, ty), ...] (bf16 slots first) and the shared slot-spec
    tuple ((S, ty), ...) with S = max across pairs, padded to mult of 4."""
    segs = []
    for e, c in enumerate(hi_counts):
        t = 0
        while t < c:
            s = min(SLOT_CAP, c - t)
            segs.append((s, 1.0, e, t, t + s, "b"))
            t += s
    for e, c in enumerate(lo_counts):
        base = hi_counts[e]
        t = 0
        while t < c:
            s = min(SLOT_CAP, c - t)
            segs.append((s, 0.5, e, base + t, base + t + s, "f"))
            t += s
    # bf16 first (descending), then fp8 (descending), onto least-loaded pair
    segs.sort(key=lambda g: (g[5], -g[0]))
    loads = [0.0] * 4
    pair_slots = [[] for _ in range(4)]
    for s, cost, e, lo, hi, ty in segs:
        p = min(range(4), key=lambda i: loads[i])
        loads[p] += s * cost
        pair_slots[p].append((e, lo, hi, ty))
    for sl in pair_slots:
        sl.sort(key=lambda t: (t[3], -(t[2] - t[1])))
    kb = max(sum(1 for t in sl if t[3] == "b") for sl in pair_slots)
    kf = max(sum(1 for t in sl if t[3] == "f") for sl in pair_slots)
    # normalize: every pair gets kb bf slots then kf f8 slots (dummies empty)
    spec = []
    norm = [[] for _ in range(4)]
    for ji in range(kb + kf):
        ty = "b" if ji < kb else "f"
        m = 8
        for p in range(4):
            mine = [t for t in pair_slots[p] if t[3] == ty]
            i = ji if ty == "b" else ji - kb
            if i < len(mine):
                norm[p].append(mine[i])
                m = max(m, mine[i][2] - mine[i][1])
            else:
                norm[p].append(None)
        spec.append((-(-m // 4) * 4, ty))
    return norm, tuple(spec)


def kernel(x, gate_w, wv1, w2, top_k):
    import ml_dtypes

    from concourse.bass_utils import run_bass_kernel_spmd

    x = np.asarray(x)
    gate_w = np.asarray(gate_w)
    wv1 = np.asarray(wv1)
    w2 = np.asarray(w2)

    T, D = x.shape
    E, F2, _ = wv1.shape
    F = F2 // 2
    Fh = F // 2
    ND = D // P
    NF = Fh // P
    NPR = NF // 2
    NDP = ND // 2
    n_cores = 8

    idx, wts = _route(x, gate_w, top_k)

    rows_l, w_l, hi_n = [], [], []
    for e in range(E):
        rows, cols = np.nonzero(idx == e)
        w_e = wts[rows, cols]
        order = np.argsort(-w_e, kind="stable")
        rows_l.append(rows[order])
        w_l.append(w_e[order])
        hi_n.append(int((w_e >= THETA).sum()))
    counts = [len(r) for r in rows_l]
    lo_n = [c - h for c, h in zip(counts, hi_n)]

    pair_slots, slot_spec = _make_slots(hi_n, lo_n)

    key = (slot_spec, D, Fh)
    if key not in _BASS_CACHE:
        _BASS_CACHE[key] = _build_bass(slot_spec, D, Fh)
    nc = _BASS_CACHE[key]

    bf16 = ml_dtypes.bfloat16
    f8 = ml_dtypes.float8_e4m3
    x_bf = x.astype(bf16)
    x_f8 = np.clip(x * X8S, -240, 240).astype(f8)
    w1_bf = wv1.astype(bf16)
    w2_bf = w2.astype(bf16)

    w1_cache: dict = {}
    w2_cache: dict = {}

    def w1_pack(e, h, ty):
        if (e, h, ty) not in w1_cache:
            if ty == "b":
                gsl = w1_bf[e][h * Fh : (h + 1) * Fh]
                usl = w1_bf[e][F + h * Fh : F + (h + 1) * Fh]
                both = np.stack([gsl, usl])  # [2, Fh, D]
                w1_cache[(e, h, ty)] = np.ascontiguousarray(
                    both.reshape(2, NPR, 2, P, ND, P).transpose(0, 1, 5, 2, 4, 3)
                )
            else:
                gsl = wv1[e][h * Fh : (h + 1) * Fh]
                usl = wv1[e][F + h * Fh : F + (h + 1) * Fh]
                both = np.clip(np.stack([gsl, usl]) * W1S, -240, 240).astype(f8)
                w1_cache[(e, h, ty)] = np.ascontiguousarray(
                    both.reshape(2, NPR, 2, P, NDP, 2, P).transpose(0, 1, 6, 2, 4, 5, 3)
                )
        return w1_cache[(e, h, ty)]

    def w2_pack(e, h, ty):
        if (e, h, ty) not in w2_cache:
            if ty == "b":
                sl = w2_bf[e][:, h * Fh : (h + 1) * Fh]  # [D, Fh]
                w2_cache[(e, h, ty)] = np.ascontiguousarray(
                    sl.reshape(NDP, 2, P, NF, P).transpose(0, 4, 1, 3, 2)
                )
            else:
                sl = np.clip(w2[e][:, h * Fh : (h + 1) * Fh] * W2S, -240, 240).astype(f8)
                w2_cache[(e, h, ty)] = np.ascontiguousarray(
                    sl.reshape(NDP, 2, P, NPR, 2, P).transpose(0, 5, 1, 3, 4, 2)
                )
        return w2_cache[(e, h, ty)]

    in_maps = []
    for p in range(4):
        slots = pair_slots[p]
        for h in range(2):
            im = {}
            for ji, (S, ty) in enumerate(slot_spec):
                slot = slots[ji]
                if ty == "b":
                    xq, w1z, w2z = (
                        np.zeros((D, S), dtype=bf16),
                        np.zeros((2, NPR, P, 2, ND, P), dtype=bf16),
                        np.zeros((NDP, P, 2, NF, P), dtype=bf16),
                    )
                else:
                    xq, w1z, w2z = (
                        np.zeros((D, S), dtype=f8),
                        np.zeros((2, NPR, P, 2, NDP, 2, P), dtype=f8),
                        np.zeros((NDP, P, 2, NPR, 2, P), dtype=f8),
                    )
                if slot is not None:
                    e, lo, hi, _ = slot
                    seg = rows_l[e][lo:hi]
                    src = x_bf if ty == "b" else x_f8
                    xq[:, : hi - lo] = src[seg].T
                    w1z = w1_pack(e, h, ty)
                    w2z = w2_pack(e, h, ty)
                im[f"x{ji}"] = np.ascontiguousarray(xq.reshape(ND, P, S))
                im[f"w1{ji}"] = w1z
                im[f"w2{ji}"] = w2z
            in_maps.append(im)

    _ensure_ntff_hook()
    res = run_bass_kernel_spmd(nc, in_maps, core_ids=list(range(n_cores)))
    global last_run
    last_run = res

    out = np.zeros((T, D), dtype=np.float32)
    for p in range(4):
        for ji, (S, ty) in enumerate(slot_spec):
            slot = pair_slots[p][ji]
            if slot is None:
                continue
            e, lo, hi, _ = slot
            n = hi - lo
            y0 = res.results[2 * p][f"y{ji}"].reshape(D, -1)[:, :n]
            y1 = res.results[2 * p + 1][f"y{ji}"].reshape(D, -1)[:, :n]
            ysum = y0.astype(np.float32) + y1.astype(np.float32)
            if ty == "f":
                ysum *= 1.0 / Y8S
            seg = rows_l[e][lo:hi]
            out[seg] += w_l[e][lo:hi, None] * ysum.T
    return out.astype(x.dtype, copy=False)


# revision 11
# speedup vs baseline: 1.2875x; 1.0498x over previous
"""Block-sparse MoE (softmax top-k routing + silu-gated FFN) on 8 Trainium2 cores.

Sharding: expert-pair x FFN-half. The router runs on host; each expert's
token list is sorted by router weight and split at THETA: high-weight
pairs run in bf16, low-weight pairs in fp8 e4m3 (DoubleRow, 2x tensor
rate) — the output error each fp8 token contributes is proportional to
its (small) router weight, keeping total rel err ~1.2e-2 vs the 2e-2
gate. Segments are LPT-packed onto 4 core-pairs (fp8 cost 0.5/col) and
each pair of cores splits the FFN dim in half (1792 of 3584), so all 8
cores run the same slot-size program (SPMD) with balanced cycles.

Per slot on a core (S tokens, ND=16 d-tiles, NFh=14 f-tiles):
  phase1: h.T [128 f, chunk] = w1h.T @ x (contract D); silu(g)*u -> aT
  phase3: y[d, tok] = sum_fi w2h[f,d].T @ aT[:, fi, :] (contract F/2)
          d on PSUM partitions, tokens moving => exact columns, no pad.
  fp8 slots: x*16, w1*32 -> psum = 512*h; silu via ACT scale 1/512;
  u16 = psum/32; a8 = e4m3(silu*u16) = 16a; w2*64 -> y*1024 (host /1024).

Moving chunks are equalized (~420-510) so LDWEIGHTS (~97-116ns) hides
under the moving phase. Slot0's x arrives dt-major in two column chunks
consumed dt-outer across 8 PSUM banks (PE starts ~12us in). w1 prefetch
depth 4 covers the ~6us pair-tile transfer. All bulk traffic rides the
SWDGE queue with >=2KB lines; later slots' x and w1 overlap prior ph3.
Host sums the two F-halves, applies router weight, scatter-adds (f32).
"""

import numpy as np


def _ensure_concourse_on_path():
    try:
        import concourse  # noqa: F401
    except ImportError:
        import sys

        for p in ("/opt/trn_rl_repo", "/root/.axon_site/_ro/trn_rl_repo"):
            if p not in sys.path:
                sys.path.insert(0, p)


_ensure_concourse_on_path()

P = 128
CHUNK_MAX = 512   # PSUM bank free-dim limit (f32)
SLOT_CAP = 2560   # max tokens per slot (SBUF-bound)
THETA = 0.35      # router-weight cutoff: below -> fp8 path
X8S, W1S, W2S = 16.0, 32.0, 64.0
Y8S = X8S * W2S   # fp8-slot output scale (host divides)

_BASS_CACHE: dict = {}
last_run = None  # BassKernelResults of the most recent kernel() call (for test.py)


def _legalize_sync(nc, max_waits: int = 1):
    """Split multi-wait sync_info into preceding EventSemaphore instructions.

    The walrus build in this container lowers every instruction with capacity
    for a single sync-wait command and errors with "Too many sync wait
    commands" otherwise, while Tile attaches up to 3 waits per instruction.
    A wait carried by an EventSemaphore on the same engine immediately before
    the instruction is semantically identical. For DMAs, keep the own-lane
    FIFO wait on the instruction itself so the in-queue wait doesn't stall
    the sequencer.
    """
    import concourse.mybir as mybir

    fn = nc.m.functions[0]
    for blk in fn.blocks:
        new_insts = []
        for inst in blk.instructions:
            si = inst.sync_info
            if si is not None and si.on_wait is not None and len(si.on_wait) > max_waits:
                ow = list(si.on_wait)
                upd_ids = {u.id for u in (si.on_update or [])}
                keep = [w for w in ow if w.id in upd_ids][:1]
                if not keep:
                    keep = [ow[-1]]
                for j, w in enumerate(ow):
                    if w is keep[0]:
                        continue
                    new_insts.append(
                        mybir.InstEventSemaphore(
                            name=f"{inst.name}-ws{j}",
                            opcode="EventSemaphore",
                            engine=inst.engine,
                            sync_info=mybir.SyncInfo(on_wait=[w], on_update=[]),
                        )
                    )
                si.on_wait = keep
            new_insts.append(inst)
        blk.instructions = new_insts


def _chunk_list(S: int):
    """Equalized moving chunks: k = ceil(S/512), base = ceil(S/k) rounded to
    a multiple of 4, so every chunk is large enough that LDWEIGHTS hides
    under the moving phase."""
    k = -(-S // CHUNK_MAX)
    base = -(-S // k)
    base = -(-base // 4) * 4
    chunks = []
    t0 = 0
    while t0 < S:
        c = min(base, S - t0)
        chunks.append((t0, c))
        t0 += c
    return chunks


def _build_bass(slot_spec: tuple, D: int, Fh: int):
    """Bass program: sequence of expert slots, each (S tokens, ty) with
    ty 'b' (bf16) or 'f' (fp8 e4m3 DoubleRow). Per-slot inputs x{j}, w1{j},
    w2{j}; outputs y{j}."""
    import concourse.bass as bass
    import concourse.mybir as mybir
    import concourse.tile as tile

    ND = D // P           # 16 contraction tiles (phase 1)
    NF = Fh // P          # 14 f-tiles per half
    NPR = NF // 2         # 7 w1 pairs
    NDP = ND // 2         # 8 dt-pair groups
    assert NF % 2 == 0 and ND % 2 == 0
    S1b = max((S for S, ty in slot_spec if ty == "b"), default=8)
    S1f = max((S for S, ty in slot_spec if ty == "f"), default=8)

    bf16 = mybir.dt.bfloat16
    f8 = mybir.dt.float8e4
    f32 = mybir.dt.float32
    DR = mybir.MatmulPerfMode.DoubleRow

    nc = bass.Bass(name="moe_pair_ffn", num_swdge_queues=4)
    xs_d, w1s_d, w2s_d, ys_d = [], [], [], []
    for j, (S, ty) in enumerate(slot_spec):
        if ty == "b":
            xs_d.append(nc.dram_tensor(f"x{j}", [ND, P, S], bf16, kind="ExternalInput"))
            w1s_d.append(
                nc.dram_tensor(
                    f"w1{j}", [2, NPR, P, 2, ND, P], bf16, kind="ExternalInput"
                )
            )
            w2s_d.append(
                nc.dram_tensor(f"w2{j}", [NDP, P, 2, NF, P], bf16, kind="ExternalInput")
            )
        else:
            xs_d.append(nc.dram_tensor(f"x{j}", [ND, P, S], f8, kind="ExternalInput"))
            w1s_d.append(
                nc.dram_tensor(
                    f"w1{j}", [2, NPR, P, 2, NDP, 2, P], f8, kind="ExternalInput"
                )
            )
            w2s_d.append(
                nc.dram_tensor(
                    f"w2{j}", [NDP, P, 2, NPR, 2, P], f8, kind="ExternalInput"
                )
            )
        ys_d.append(nc.dram_tensor(f"y{j}", [ND, P, S], bf16, kind="ExternalOutput"))

    act_silu = mybir.ActivationFunctionType.Silu
    act_copy = mybir.ActivationFunctionType.Copy

    with tile.TileContext(nc) as tc:
        with (
            tc.tile_pool(name="xp", bufs=1) as xpool,
            tc.tile_pool(name="ap", bufs=1) as apool,
            tc.tile_pool(name="w1p", bufs=3) as w1pool,
            tc.tile_pool(name="w2p", bufs=3) as w2pool,
            tc.tile_pool(name="hp", bufs=4) as hpool,
            tc.tile_pool(name="yp", bufs=3) as ypool,
            tc.tile_pool(name="x8p", bufs=1) as x8pool,
            tc.tile_pool(name="w18p", bufs=5) as w18pool,
            tc.tile_pool(name="w28p", bufs=4) as w28pool,
            tc.tile_pool(name="ps", bufs=8, space="PSUM") as psum,
        ):
            # fp8 tiles live in their own pools, declared after the bf16
            # pools so the bf16 phase keeps the measured conflict-free
            # SBUF layout (shifting pool bases cost +35ns/matmul once).
            x_sb = xpool.tile([P, ND, S1b], bf16)
            aT = apool.tile([P, NF, S1b], bf16)
            has_f8 = any(ty == "f" for _, ty in slot_spec)
            if has_f8:
                x8_sb = x8pool.tile([P, ND, S1f], f8)
                aT8 = x8pool.tile([P, NF, S1f], f8)

            for j, (S, ty) in enumerate(slot_spec):
                chunks = _chunk_list(S)
                x_d, w1_d, w2_d, y_d = xs_d[j], w1s_d[j], w2s_d[j], ys_d[j]

                if ty == "b":
                    # ---------------- bf16 slot ----------------
                    xsplit = -(-S // 8) * 4
                    startup = (
                        j == 0
                        and len(chunks) >= 3
                        and chunks[1][0] + chunks[1][1] <= xsplit
                    )
                    w1g0 = w1pool.tile([P, 2, ND, P], bf16, tag="w1", name="w1g0")
                    w1u0 = w1pool.tile([P, 2, ND, P], bf16, tag="w1", name="w1u0")
                    if startup:
                        # queue order: 0.5MB of w1 (gate/jj0), x chunk0
                        # dt-major, rest of pair0, x chunk1 — the dt-outer
                        # sub-phases below consume in exactly this order.
                        nc.gpsimd.dma_start(w1g0[:, 0], w1_d[0, 0, :, 0])
                        for dt in range(ND):
                            nc.gpsimd.dma_start(
                                x_sb[:, dt, :xsplit], x_d[dt, :, :xsplit]
                            )
                        nc.gpsimd.dma_start(w1g0[:, 1], w1_d[0, 0, :, 1])
                        nc.gpsimd.dma_start(w1u0[:, 0], w1_d[1, 0, :, 0])
                        nc.gpsimd.dma_start(w1u0[:, 1], w1_d[1, 0, :, 1])
                        for dt in range(ND):
                            nc.gpsimd.dma_start(
                                x_sb[:, dt, xsplit:S], x_d[dt, :, xsplit:]
                            )
                    else:
                        nc.gpsimd.dma_start(w1g0, w1_d[0, 0])
                        nc.gpsimd.dma_start(w1u0, w1_d[1, 0])
                        for dt in range(ND):
                            nc.gpsimd.dma_start(x_sb[:, dt, :S], x_d[dt])

                    def ph1_chain(w1t, jj, t0, csz, start, stop, ps):
                        for dt in range(ND):
                            nc.tensor.matmul(
                                ps[:, :csz],
                                w1t[:, jj, dt, :],
                                x_sb[:, dt, t0 : t0 + csz],
                                start=start and dt == 0,
                                stop=stop and dt == ND - 1,
                            )

                    def ph1_evac(fi, t0, csz, ps_g, ps_u):
                        hg = hpool.tile([P, CHUNK_MAX], bf16, tag="h", name="hg")
                        nc.scalar.activation(hg[:, :csz], ps_g[:, :csz], act_silu)
                        nc.vector.tensor_mul(
                            aT[:, fi, t0 : t0 + csz], hg[:, :csz], ps_u[:, :csz]
                        )

                    pr_start = 0
                    if startup:
                        sc = chunks[:2]
                        banks = {}
                        for gu in range(2):
                            for jj in range(2):
                                for ci in range(len(sc)):
                                    banks[(jj, ci, gu)] = psum.tile(
                                        [P, CHUNK_MAX], f32, tag="ps",
                                        name=f"ps_s{jj}{ci}{gu}",
                                    )
                        for gu, jj in ((0, 0), (0, 1), (1, 0), (1, 1)):
                            w1t = w1g0 if gu == 0 else w1u0
                            for dt in range(ND):
                                for ci, (t0, csz) in enumerate(sc):
                                    nc.tensor.matmul(
                                        banks[(jj, ci, gu)][:, :csz],
                                        w1t[:, jj, dt, :],
                                        x_sb[:, dt, t0 : t0 + csz],
                                        start=(dt == 0),
                                        stop=(dt == ND - 1),
                                    )
                            if gu == 1:
                                for ci, (t0, csz) in enumerate(sc):
                                    ph1_evac(
                                        jj, t0, csz,
                                        banks[(jj, ci, 0)], banks[(jj, ci, 1)],
                                    )
                        for jj in range(2):
                            for t0, csz in chunks[2:]:
                                ps_g = psum.tile(
                                    [P, CHUNK_MAX], f32, tag="ps", name="ps_g"
                                )
                                ph1_chain(w1g0, jj, t0, csz, True, True, ps_g)
                                ps_u = psum.tile(
                                    [P, CHUNK_MAX], f32, tag="ps", name="ps_u"
                                )
                                ph1_chain(w1u0, jj, t0, csz, True, True, ps_u)
                                ph1_evac(jj, t0, csz, ps_g, ps_u)
                        pr_start = 1

                    for pr in range(pr_start, NPR):
                        if pr == 0:
                            w1g, w1u = w1g0, w1u0
                        else:
                            w1g = w1pool.tile([P, 2, ND, P], bf16, tag="w1", name="w1g")
                            nc.gpsimd.dma_start(w1g, w1_d[0, pr])
                            w1u = w1pool.tile([P, 2, ND, P], bf16, tag="w1", name="w1u")
                            nc.gpsimd.dma_start(w1u, w1_d[1, pr])
                        for jj in range(2):
                            for t0, csz in chunks:
                                ps_g = psum.tile(
                                    [P, CHUNK_MAX], f32, tag="ps", name="ps_g"
                                )
                                ph1_chain(w1g, jj, t0, csz, True, True, ps_g)
                                ps_u = psum.tile(
                                    [P, CHUNK_MAX], f32, tag="ps", name="ps_u"
                                )
                                ph1_chain(w1u, jj, t0, csz, True, True, ps_u)
                                ph1_evac(2 * pr + jj, t0, csz, ps_g, ps_u)

                    for dp in range(NDP):
                        w2sb = w2pool.tile([P, 2, NF, P], bf16, tag="w2", name="w2sb")
                        nc.gpsimd.dma_start(w2sb, w2_d[dp])
                        for dj in range(2):
                            dt = 2 * dp + dj
                            y_st = ypool.tile([P, S1b], bf16, tag="y", name="y_st")
                            for t0, csz in chunks:
                                ps_y = psum.tile(
                                    [P, CHUNK_MAX], f32, tag="ps", name="ps_y"
                                )
                                for fi in range(NF):
                                    nc.tensor.matmul(
                                        ps_y[:, :csz],
                                        w2sb[:, dj, fi, :],
                                        aT[:, fi, t0 : t0 + csz],
                                        start=(fi == 0),
                                        stop=(fi == NF - 1),
                                    )
                                nc.scalar.activation(
                                    y_st[:, t0 : t0 + csz], ps_y[:, :csz], act_copy
                                )
                            nc.gpsimd.dma_start(y_d[dt], y_st[:, :S])
                else:
                    # ---------------- fp8 slot (DoubleRow) ----------------
                    for dt in range(ND):
                        nc.gpsimd.dma_start(x8_sb[:, dt, :S], x_d[dt])

                    for pr in range(NPR):
                        w1g = w18pool.tile(
                            [P, 2, NDP, 2, P], f8, tag="w18", name="w18g"
                        )
                        nc.gpsimd.dma_start(w1g, w1_d[0, pr])
                        w1u = w18pool.tile(
                            [P, 2, NDP, 2, P], f8, tag="w18", name="w18u"
                        )
                        nc.gpsimd.dma_start(w1u, w1_d[1, pr])
                        for jj in range(2):
                            fi = 2 * pr + jj
                            for t0, csz in chunks:
                                ps_g = psum.tile(
                                    [P, CHUNK_MAX], f32, tag="ps", name="ps_g"
                                )
                                for dp in range(NDP):
                                    nc.tensor.matmul(
                                        ps_g[:, :csz],
                                        w1g[:, jj, dp],
                                        x8_sb[:, 2 * dp : 2 * dp + 2, t0 : t0 + csz],
                                        start=(dp == 0),
                                        stop=(dp == NDP - 1),
                                        perf_mode=DR,
                                    )
                                ps_u = psum.tile(
                                    [P, CHUNK_MAX], f32, tag="ps", name="ps_u"
                                )
                                for dp in range(NDP):
                                    nc.tensor.matmul(
                                        ps_u[:, :csz],
                                        w1u[:, jj, dp],
                                        x8_sb[:, 2 * dp : 2 * dp + 2, t0 : t0 + csz],
                                        start=(dp == 0),
                                        stop=(dp == NDP - 1),
                                        perf_mode=DR,
                                    )
                                hg = hpool.tile([P, CHUNK_MAX], bf16, tag="h", name="hg")
                                nc.scalar.activation(
                                    hg[:, :csz], ps_g[:, :csz], act_silu,
                                    scale=1.0 / (X8S * W1S),
                                )
                                hu = hpool.tile([P, CHUNK_MAX], bf16, tag="h", name="hu")
                                nc.scalar.activation(
                                    hu[:, :csz], ps_u[:, :csz], act_copy,
                                    scale=X8S / (X8S * W1S),
                                )
                                nc.vector.tensor_mul(
                                    aT8[:, fi, t0 : t0 + csz], hg[:, :csz], hu[:, :csz]
                                )

                    for dp in range(NDP):
                        w2sb = w28pool.tile(
                            [P, 2, NPR, 2, P], f8, tag="w28", name="w28sb"
                        )
                        nc.gpsimd.dma_start(w2sb, w2_d[dp])
                        for dj in range(2):
                            dt = 2 * dp + dj
                            y_st = ypool.tile([P, S1b], bf16, tag="y", name="y_st")
                            for t0, csz in chunks:
                                ps_y = psum.tile(
                                    [P, CHUNK_MAX], f32, tag="ps", name="ps_y"
                                )
                                for q in range(NPR):
                                    nc.tensor.matmul(
                                        ps_y[:, :csz],
                                        w2sb[:, dj, q],
                                        aT8[:, 2 * q : 2 * q + 2, t0 : t0 + csz],
                                        start=(q == 0),
                                        stop=(q == NPR - 1),
                                        perf_mode=DR,
                                    )
                                nc.scalar.activation(
                                    y_st[:, t0 : t0 + csz], ps_y[:, :csz], act_copy
                                )
                            nc.gpsimd.dma_start(y_d[dt], y_st[:, :S])

    _legalize_sync(nc)
    return nc


def _ensure_ntff_hook():
    """Register the axon NTFF-profile hook if the image's antenv lacks
    ``axon_hooks`` (the hook impl ships in trn_agent_boot). Best-effort."""
    import sys
    import types

    try:
        from antenv.axon_hooks import get_axon_ntff_profile_hook  # noqa: F401

        return
    except ImportError:
        pass
    try:
        import antenv

        mod = types.ModuleType("antenv.axon_hooks")
        mod._hook = None

        def set_axon_ntff_profile_hook(h):
            mod._hook = h

        def get_axon_ntff_profile_hook():
            return mod._hook

        mod.set_axon_ntff_profile_hook = set_axon_ntff_profile_hook
        mod.get_axon_ntff_profile_hook = get_axon_ntff_profile_hook
        sys.modules["antenv.axon_hooks"] = mod
        antenv.axon_hooks = mod

        from trn_agent_boot.trn_boot import _ntff_profile_via_ctypes

        so_path = "/opt/axon/libaxon_pjrt.so"
        hook = _ntff_profile_via_ctypes(so_path)
        if hook is not None:
            mod._hook = hook
    except Exception:
        pass


def _route(x, gate_w, top_k):
    """Replicates the reference router in numpy fp32 (renormalized top-k
    softmax == softmax over the top-k logits)."""
    logits = x.astype(np.float32) @ gate_w.astype(np.float32).T  # [T, E]
    k = int(top_k)
    idx = np.argpartition(-logits, k - 1, axis=1)[:, :k]
    lv = np.take_along_axis(logits, idx, axis=1)
    m = lv.max(axis=1, keepdims=True)
    ew = np.exp(lv - m)
    wts = ew / ew.sum(axis=1, keepdims=True)
    return idx, wts.astype(np.float32)


def _make_slots(hi_counts, lo_counts):
    """Cut experts into <=SLOT_CAP segments per precision; LPT-pack onto 4
    core pairs (fp8 cost 0.5/col); return per-pair slot lists
    [(expert, lo, hi, ty), ...] (bf16 slots first) and the shared slot-spec
    tuple ((S, ty), ...) with S = max across pairs, padded to mult of 4."""
    segs = []
    for e, c in enumerate(hi_counts):
        t = 0
        while t < c:
            s = min(SLOT_CAP, c - t)
            segs.append((s, 1.0, e, t, t + s, "b"))
            t += s
    for e, c in enumerate(lo_counts):
        base = hi_counts[e]
        t = 0
        while t < c:
            s = min(SLOT_CAP, c - t)
            segs.append((s, 0.5, e, base + t, base + t + s, "f"))
            t += s
    # bf16 first (descending), then fp8 (descending), onto least-loaded pair
    segs.sort(key=lambda g: (g[5], -g[0]))
    loads = [0.0] * 4
    pair_slots = [[] for _ in range(4)]
    for s, cost, e, lo, hi, ty in segs:
        p = min(range(4), key=lambda i: loads[i])
        loads[p] += s * cost
        pair_slots[p].append((e, lo, hi, ty))
    for sl in pair_slots:
        sl.sort(key=lambda t: (t[3], -(t[2] - t[1])))
    kb = max(sum(1 for t in sl if t[3] == "b") for sl in pair_slots)
    kf = max(sum(1 for t in sl if t[3] == "f") for sl in pair_slots)
    # normalize: every pair gets kb bf slots then kf f8 slots (dummies empty)
    spec = []
    norm = [[] for _ in range(4)]
    for ji in range(kb + kf):
        ty = "b" if ji < kb else "f"
        m = 8
        for p in range(4):
            mine = [t for t in pair_slots[p] if t[3] == ty]
            i = ji if ty == "b" else ji - kb
            if i < len(mine):
                norm[p].append(mine[i])
                m = max(m, mine[i][2] - mine[i][1])
            else:
                norm[p].append(None)
        spec.append((-(-m // 4) * 4, ty))
    return norm, tuple(spec)


def kernel(x, gate_w, wv1, w2, top_k):
    import ml_dtypes

    from concourse.bass_utils import run_bass_kernel_spmd

    x = np.asarray(x)
    gate_w = np.asarray(gate_w)
    wv1 = np.asarray(wv1)
    w2 = np.asarray(w2)

    T, D = x.shape
    E, F2, _ = wv1.shape
    F = F2 // 2
    Fh = F // 2
    ND = D // P
    NF = Fh // P
    NPR = NF // 2
    NDP = ND // 2
    n_cores = 8

    idx, wts = _route(x, gate_w, top_k)

    rows_l, w_l, hi_n = [], [], []
    for e in range(E):
        rows, cols = np.nonzero(idx == e)
        w_e = wts[rows, cols]
        order = np.argsort(-w_e, kind="stable")
        rows_l.append(rows[order])
        w_l.append(w_e[order])
        hi_n.append(int((w_e >= THETA).sum()))
    counts = [len(r) for r in rows_l]
    lo_n = [c - h for c, h in zip(counts, hi_n)]

    pair_slots, slot_spec = _make_slots(hi_n, lo_n)

    key = (slot_spec, D, Fh)
    if key not in _BASS_CACHE:
        _BASS_CACHE[key] = _build_bass(slot_spec, D, Fh)
    nc = _BASS_CACHE[key]

    bf16 = ml_dtypes.bfloat16
    f8 = ml_dtypes.float8_e4m3
    x_bf = x.astype(bf16)
    x_f8 = np.clip(x * X8S, -240, 240).astype(f8)
    w1_bf = wv1.astype(bf16)
    w2_bf = w2.astype(bf16)

    w1_cache: dict = {}
    w2_cache: dict = {}

    def w1_pack(e, h, ty):
        if (e, h, ty) not in w1_cache:
            if ty == "b":
                gsl = w1_bf[e][h * Fh : (h + 1) * Fh]
                usl = w1_bf[e][F + h * Fh : F + (h + 1) * Fh]
                both = np.stack([gsl, usl])  # [2, Fh, D]
                w1_cache[(e, h, ty)] = np.ascontiguousarray(
                    both.reshape(2, NPR, 2, P, ND, P).transpose(0, 1, 5, 2, 4, 3)
                )
            else:
                gsl = wv1[e][h * Fh : (h + 1) * Fh]
                usl = wv1[e][F + h * Fh : F + (h + 1) * Fh]
                both = np.clip(np.stack([gsl, usl]) * W1S, -240, 240).astype(f8)
                w1_cache[(e, h, ty)] = np.ascontiguousarray(
                    both.reshape(2, NPR, 2, P, NDP, 2, P).transpose(0, 1, 6, 2, 4, 5, 3)
                )
        return w1_cache[(e, h, ty)]

    def w2_pack(e, h, ty):
        if (e, h, ty) not in w2_cache:
            if ty == "b":
                sl = w2_bf[e][:, h * Fh : (h + 1) * Fh]  # [D, Fh]
                w2_cache[(e, h, ty)] = np.ascontiguousarray(
                    sl.reshape(NDP, 2, P, NF, P).transpose(0, 4, 1, 3, 2)
                )
            else:
                sl = np.clip(w2[e][:, h * Fh : (h + 1) * Fh] * W2S, -240, 240).astype(f8)
                w2_cache[(e, h, ty)] = np.ascontiguousarray(
                    sl.reshape(NDP, 2, P, NPR, 2, P).transpose(0, 5, 1, 3, 4, 2)
                )
        return w2_cache[(e, h, ty)]

    in_maps = []
    for p in range(4):
        slots = pair_slots[p]
        for h in range(2):
            im = {}
            for ji, (S, ty) in enumerate(slot_spec):
                slot = slots[ji]
                if ty == "b":
                    xq, w1z, w2z = (
                        np.zeros((D, S), dtype=bf16),
                        np.zeros((2, NPR, P, 2, ND, P), dtype=bf16),
                        np.zeros((NDP, P, 2, NF, P), dtype=bf16),
                    )
                else:
                    xq, w1z, w2z = (
                        np.zeros((D, S), dtype=f8),
                        np.zeros((2, NPR, P, 2, NDP, 2, P), dtype=f8),
                        np.zeros((NDP, P, 2, NPR, 2, P), dtype=f8),
                    )
                if slot is not None:
                    e, lo, hi, _ = slot
                    seg = rows_l[e][lo:hi]
                    src = x_bf if ty == "b" else x_f8
                    xq[:, : hi - lo] = src[seg].T
                    w1z = w1_pack(e, h, ty)
                    w2z = w2_pack(e, h, ty)
                im[f"x{ji}"] = np.ascontiguousarray(xq.reshape(ND, P, S))
                im[f"w1{ji}"] = w1z
                im[f"w2{ji}"] = w2z
            in_maps.append(im)

    _ensure_ntff_hook()
    res = run_bass_kernel_spmd(nc, in_maps, core_ids=list(range(n_cores)))
    global last_run
    last_run = res

    out = np.zeros((T, D), dtype=np.float32)
    for p in range(4):
        for ji, (S, ty) in enumerate(slot_spec):
            slot = pair_slots[p][ji]
            if slot is None:
                continue
            e, lo, hi, _ = slot
            n = hi - lo
            y0 = res.results[2 * p][f"y{ji}"].reshape(D, -1)[:, :n]
            y1 = res.results[2 * p + 1][f"y{ji}"].reshape(D, -1)[:, :n]
            ysum = y0.astype(np.float32) + y1.astype(np.float32)
            if ty == "f":
                ysum *= 1.0 / Y8S
            seg = rows_l[e][lo:hi]
            out[seg] += w_l[e][lo:hi, None] * ysum.T
    return out.astype(x.dtype, copy=False)


# revision 12
# speedup vs baseline: 1.2969x; 1.0073x over previous
"""Block-sparse MoE (softmax top-k routing + silu-gated FFN) on 8 Trainium2 cores.

Sharding: expert-pair x FFN-half. The router runs on host; each expert's
token list is sorted by router weight and split at THETA: high-weight
pairs run in bf16, low-weight pairs in fp8 e4m3 (DoubleRow, 2x tensor
rate) — the output error each fp8 token contributes is proportional to
its (small) router weight, keeping total rel err ~1.2e-2 vs the 2e-2
gate. Segments are LPT-packed onto 4 core-pairs (fp8 cost 0.5/col) and
each pair of cores splits the FFN dim in half (1792 of 3584), so all 8
cores run the same slot-size program (SPMD) with balanced cycles.

Per slot on a core (S tokens, ND=16 d-tiles, NFh=14 f-tiles):
  phase1: h.T [128 f, chunk] = w1h.T @ x (contract D); silu(g)*u -> aT
  phase3: y[d, tok] = sum_fi w2h[f,d].T @ aT[:, fi, :] (contract F/2)
          d on PSUM partitions, tokens moving => exact columns, no pad.
  fp8 slots: x*16, w1*32 -> psum = 512*h; silu via ACT scale 1/512;
  u16 = psum/32; a8 = e4m3(silu*u16) = 16a; w2*64 -> y*1024 (host /1024).

Moving chunks are equalized (~420-510) so LDWEIGHTS (~97-116ns) hides
under the moving phase. Slot0's x arrives dt-major in two column chunks
consumed dt-outer across 8 PSUM banks (PE starts ~12us in). w1 prefetch
depth 4 covers the ~6us pair-tile transfer. All bulk traffic rides the
SWDGE queue with >=2KB lines; later slots' x and w1 overlap prior ph3.
Host sums the two F-halves, applies router weight, scatter-adds (f32).
"""

import numpy as np


def _ensure_concourse_on_path():
    try:
        import concourse  # noqa: F401
    except ImportError:
        import sys

        for p in ("/opt/trn_rl_repo", "/root/.axon_site/_ro/trn_rl_repo"):
            if p not in sys.path:
                sys.path.insert(0, p)


_ensure_concourse_on_path()

P = 128
CHUNK_MAX = 512   # PSUM bank free-dim limit (f32)
SLOT_CAP = 2560   # max tokens per slot (SBUF-bound)
THETA = 0.35      # router-weight cutoff: below -> fp8 path
X8S, W1S, W2S = 16.0, 32.0, 64.0
Y8S = X8S * W2S   # fp8-slot output scale (host divides)

_BASS_CACHE: dict = {}
last_run = None  # BassKernelResults of the most recent kernel() call (for test.py)


def _legalize_sync(nc, max_waits: int = 1):
    """Split multi-wait sync_info into preceding EventSemaphore instructions.

    The walrus build in this container lowers every instruction with capacity
    for a single sync-wait command and errors with "Too many sync wait
    commands" otherwise, while Tile attaches up to 3 waits per instruction.
    A wait carried by an EventSemaphore on the same engine immediately before
    the instruction is semantically identical. For DMAs, keep the own-lane
    FIFO wait on the instruction itself so the in-queue wait doesn't stall
    the sequencer.
    """
    import concourse.mybir as mybir

    fn = nc.m.functions[0]
    for blk in fn.blocks:
        new_insts = []
        for inst in blk.instructions:
            si = inst.sync_info
            if si is not None and si.on_wait is not None and len(si.on_wait) > max_waits:
                ow = list(si.on_wait)
                upd_ids = {u.id for u in (si.on_update or [])}
                keep = [w for w in ow if w.id in upd_ids][:1]
                if not keep:
                    keep = [ow[-1]]
                for j, w in enumerate(ow):
                    if w is keep[0]:
                        continue
                    new_insts.append(
                        mybir.InstEventSemaphore(
                            name=f"{inst.name}-ws{j}",
                            opcode="EventSemaphore",
                            engine=inst.engine,
                            sync_info=mybir.SyncInfo(on_wait=[w], on_update=[]),
                        )
                    )
                si.on_wait = keep
            new_insts.append(inst)
        blk.instructions = new_insts


def _chunk_list(S: int):
    """Equalized moving chunks: k = ceil(S/512), base = ceil(S/k) rounded to
    a multiple of 4, so every chunk is large enough that LDWEIGHTS hides
    under the moving phase."""
    k = -(-S // CHUNK_MAX)
    base = -(-S // k)
    base = -(-base // 4) * 4
    chunks = []
    t0 = 0
    while t0 < S:
        c = min(base, S - t0)
        chunks.append((t0, c))
        t0 += c
    return chunks


def _build_bass(slot_spec: tuple, D: int, Fh: int):
    """Bass program: sequence of expert slots, each (S tokens, ty) with
    ty 'b' (bf16) or 'f' (fp8 e4m3 DoubleRow). Per-slot inputs x{j}, w1{j},
    w2{j}; outputs y{j}."""
    import concourse.bass as bass
    import concourse.mybir as mybir
    import concourse.tile as tile

    ND = D // P           # 16 contraction tiles (phase 1)
    NF = Fh // P          # 14 f-tiles per half
    NPR = NF // 2         # 7 w1 pairs
    NDP = ND // 2         # 8 dt-pair groups
    assert NF % 2 == 0 and ND % 2 == 0
    S1b = max((S for S, ty in slot_spec if ty == "b"), default=8)
    S1f = max((S for S, ty in slot_spec if ty == "f"), default=8)

    bf16 = mybir.dt.bfloat16
    f8 = mybir.dt.float8e4
    f32 = mybir.dt.float32
    DR = mybir.MatmulPerfMode.DoubleRow

    nc = bass.Bass(name="moe_pair_ffn", num_swdge_queues=4)
    xs_d, w1s_d, w2s_d, ys_d = [], [], [], []
    for j, (S, ty) in enumerate(slot_spec):
        if ty == "b":
            xs_d.append(nc.dram_tensor(f"x{j}", [ND, P, S], bf16, kind="ExternalInput"))
            w1s_d.append(
                nc.dram_tensor(
                    f"w1{j}", [2, NPR, P, 2, ND, P], bf16, kind="ExternalInput"
                )
            )
            w2s_d.append(
                nc.dram_tensor(f"w2{j}", [NDP, P, 2, NF, P], bf16, kind="ExternalInput")
            )
        else:
            xs_d.append(nc.dram_tensor(f"x{j}", [ND, P, S], f8, kind="ExternalInput"))
            w1s_d.append(
                nc.dram_tensor(
                    f"w1{j}", [2, NPR, P, 2, NDP, 2, P], f8, kind="ExternalInput"
                )
            )
            w2s_d.append(
                nc.dram_tensor(
                    f"w2{j}", [NDP, P, 2, NPR, 2, P], f8, kind="ExternalInput"
                )
            )
        if ty == "b":
            ys_d.append(
                nc.dram_tensor(f"y{j}", [ND, P, S], bf16, kind="ExternalOutput")
            )
        else:
            ys_d.append(
                nc.dram_tensor(
                    f"y{j}", [ND // 4, P, 4, S], bf16, kind="ExternalOutput"
                )
            )

    act_silu = mybir.ActivationFunctionType.Silu
    act_copy = mybir.ActivationFunctionType.Copy

    with tile.TileContext(nc) as tc:
        with (
            tc.tile_pool(name="xp", bufs=1) as xpool,
            tc.tile_pool(name="ap", bufs=1) as apool,
            tc.tile_pool(name="w1p", bufs=3) as w1pool,
            tc.tile_pool(name="w2p", bufs=3) as w2pool,
            tc.tile_pool(name="hp", bufs=3) as hpool,
            tc.tile_pool(name="yp", bufs=3) as ypool,
            tc.tile_pool(name="x8p", bufs=1) as x8pool,
            tc.tile_pool(name="w18p", bufs=4) as w18pool,
            tc.tile_pool(name="w28p", bufs=4) as w28pool,
            tc.tile_pool(name="y4p", bufs=2) as y4pool,
            tc.tile_pool(name="ps", bufs=8, space="PSUM") as psum,
        ):
            # fp8 tiles live in their own pools, declared after the bf16
            # pools so the bf16 phase keeps the measured conflict-free
            # SBUF layout (shifting pool bases cost +35ns/matmul once).
            x_sb = xpool.tile([P, ND, S1b], bf16)
            aT = apool.tile([P, NF, S1b], bf16)
            has_f8 = any(ty == "f" for _, ty in slot_spec)
            if has_f8:
                x8_sb = x8pool.tile([P, ND, S1f], f8)
                aT8 = x8pool.tile([P, NF, S1f], f8)

            x8_hoisted: set = set()
            w18_hoisted: dict = {}
            for j, (S, ty) in enumerate(slot_spec):
                chunks = _chunk_list(S)
                x_d, w1_d, w2_d, y_d = xs_d[j], w1s_d[j], w2s_d[j], ys_d[j]

                if ty == "b":
                    # ---------------- bf16 slot ----------------
                    xsplit = -(-S // 8) * 4
                    startup = (
                        j == 0
                        and len(chunks) >= 3
                        and chunks[1][0] + chunks[1][1] <= xsplit
                    )
                    w1g0 = w1pool.tile([P, 2, ND, P], bf16, tag="w1", name="w1g0")
                    w1u0 = w1pool.tile([P, 2, ND, P], bf16, tag="w1", name="w1u0")
                    if startup:
                        # queue order: 0.5MB of w1 (gate/jj0), x chunk0
                        # dt-major, rest of pair0, x chunk1 — the dt-outer
                        # sub-phases below consume in exactly this order.
                        nc.gpsimd.dma_start(w1g0[:, 0], w1_d[0, 0, :, 0])
                        for dt in range(ND):
                            nc.gpsimd.dma_start(
                                x_sb[:, dt, :xsplit], x_d[dt, :, :xsplit]
                            )
                        nc.gpsimd.dma_start(w1g0[:, 1], w1_d[0, 0, :, 1])
                        nc.gpsimd.dma_start(w1u0[:, 0], w1_d[1, 0, :, 0])
                        nc.gpsimd.dma_start(w1u0[:, 1], w1_d[1, 0, :, 1])
                        for dt in range(ND):
                            nc.gpsimd.dma_start(
                                x_sb[:, dt, xsplit:S], x_d[dt, :, xsplit:]
                            )
                    else:
                        nc.gpsimd.dma_start(w1g0, w1_d[0, 0])
                        nc.gpsimd.dma_start(w1u0, w1_d[1, 0])
                        for dt in range(ND):
                            nc.gpsimd.dma_start(x_sb[:, dt, :S], x_d[dt])

                    def ph1_chain(w1t, jj, t0, csz, start, stop, ps):
                        for dt in range(ND):
                            nc.tensor.matmul(
                                ps[:, :csz],
                                w1t[:, jj, dt, :],
                                x_sb[:, dt, t0 : t0 + csz],
                                start=start and dt == 0,
                                stop=stop and dt == ND - 1,
                            )

                    def ph1_evac(fi, t0, csz, ps_g, ps_u):
                        hg = hpool.tile([P, CHUNK_MAX], bf16, tag="h", name="hg")
                        nc.scalar.activation(hg[:, :csz], ps_g[:, :csz], act_silu)
                        nc.vector.tensor_mul(
                            aT[:, fi, t0 : t0 + csz], hg[:, :csz], ps_u[:, :csz]
                        )

                    pr_start = 0
                    if startup:
                        sc = chunks[:2]
                        banks = {}
                        for gu in range(2):
                            for jj in range(2):
                                for ci in range(len(sc)):
                                    banks[(jj, ci, gu)] = psum.tile(
                                        [P, CHUNK_MAX], f32, tag="ps",
                                        name=f"ps_s{jj}{ci}{gu}",
                                    )
                        for gu, jj in ((0, 0), (0, 1), (1, 0), (1, 1)):
                            w1t = w1g0 if gu == 0 else w1u0
                            for dt in range(ND):
                                for ci, (t0, csz) in enumerate(sc):
                                    nc.tensor.matmul(
                                        banks[(jj, ci, gu)][:, :csz],
                                        w1t[:, jj, dt, :],
                                        x_sb[:, dt, t0 : t0 + csz],
                                        start=(dt == 0),
                                        stop=(dt == ND - 1),
                                    )
                            if gu == 1:
                                for ci, (t0, csz) in enumerate(sc):
                                    ph1_evac(
                                        jj, t0, csz,
                                        banks[(jj, ci, 0)], banks[(jj, ci, 1)],
                                    )
                        for jj in range(2):
                            for t0, csz in chunks[2:]:
                                ps_g = psum.tile(
                                    [P, CHUNK_MAX], f32, tag="ps", name="ps_g"
                                )
                                ph1_chain(w1g0, jj, t0, csz, True, True, ps_g)
                                ps_u = psum.tile(
                                    [P, CHUNK_MAX], f32, tag="ps", name="ps_u"
                                )
                                ph1_chain(w1u0, jj, t0, csz, True, True, ps_u)
                                ph1_evac(jj, t0, csz, ps_g, ps_u)
                        pr_start = 1

                    for pr in range(pr_start, NPR):
                        if pr == 0:
                            w1g, w1u = w1g0, w1u0
                        else:
                            w1g = w1pool.tile([P, 2, ND, P], bf16, tag="w1", name="w1g")
                            nc.gpsimd.dma_start(w1g, w1_d[0, pr])
                            w1u = w1pool.tile([P, 2, ND, P], bf16, tag="w1", name="w1u")
                            nc.gpsimd.dma_start(w1u, w1_d[1, pr])
                        for jj in range(2):
                            for t0, csz in chunks:
                                ps_g = psum.tile(
                                    [P, CHUNK_MAX], f32, tag="ps", name="ps_g"
                                )
                                ph1_chain(w1g, jj, t0, csz, True, True, ps_g)
                                ps_u = psum.tile(
                                    [P, CHUNK_MAX], f32, tag="ps", name="ps_u"
                                )
                                ph1_chain(w1u, jj, t0, csz, True, True, ps_u)
                                ph1_evac(2 * pr + jj, t0, csz, ps_g, ps_u)

                    for dp in range(NDP):
                        w2sb = w2pool.tile([P, 2, NF, P], bf16, tag="w2", name="w2sb")
                        nc.gpsimd.dma_start(w2sb, w2_d[dp])
                        for dj in range(2):
                            dt = 2 * dp + dj
                            y_st = ypool.tile([P, S1b], bf16, tag="y", name="y_st")
                            for t0, csz in chunks:
                                ps_y = psum.tile(
                                    [P, CHUNK_MAX], f32, tag="ps", name="ps_y"
                                )
                                for fi in range(NF):
                                    nc.tensor.matmul(
                                        ps_y[:, :csz],
                                        w2sb[:, dj, fi, :],
                                        aT[:, fi, t0 : t0 + csz],
                                        start=(fi == 0),
                                        stop=(fi == NF - 1),
                                    )
                                nc.scalar.activation(
                                    y_st[:, t0 : t0 + csz], ps_y[:, :csz], act_copy
                                )
                            (nc.sync if dt % 2 == 0 else nc.gpsimd).dma_start(
                                y_d[dt], y_st[:, :S]
                            )
                    # prefetch the next (fp8) slot's x8 + first w18 pair ahead
                    # of this ph3's throttled y DMAs so they aren't
                    # head-blocked on the SWDGE queue.
                    if j + 1 < len(slot_spec) and slot_spec[j + 1][1] == "f":
                        Sn = slot_spec[j + 1][0]
                        for dt in range(ND):
                            nc.gpsimd.dma_start(
                                x8_sb[:, dt, :Sn], xs_d[j + 1][dt]
                            )
                        x8_hoisted.add(j + 1)
                        hg0 = w18pool.tile(
                            [P, 2, NDP, 2, P], f8, tag="w18", name="w18hg"
                        )
                        nc.gpsimd.dma_start(hg0, w1s_d[j + 1][0, 0])
                        hu0 = w18pool.tile(
                            [P, 2, NDP, 2, P], f8, tag="w18", name="w18hu"
                        )
                        nc.gpsimd.dma_start(hu0, w1s_d[j + 1][1, 0])
                        w18_hoisted[j + 1] = (hg0, hu0)
                else:
                    # ---------------- fp8 slot (DoubleRow) ----------------
                    if j not in x8_hoisted:
                        for dt in range(ND):
                            nc.gpsimd.dma_start(x8_sb[:, dt, :S], x_d[dt])

                    for pr in range(NPR):
                        if pr == 0 and j in w18_hoisted:
                            w1g, w1u = w18_hoisted[j]
                        else:
                            w1g = w18pool.tile(
                                [P, 2, NDP, 2, P], f8, tag="w18", name="w18g"
                            )
                            nc.gpsimd.dma_start(w1g, w1_d[0, pr])
                            w1u = w18pool.tile(
                                [P, 2, NDP, 2, P], f8, tag="w18", name="w18u"
                            )
                            nc.gpsimd.dma_start(w1u, w1_d[1, pr])
                        for jj in range(2):
                            fi = 2 * pr + jj
                            for t0, csz in chunks:
                                ps_g = psum.tile(
                                    [P, CHUNK_MAX], f32, tag="ps", name="ps_g"
                                )
                                for dp in range(NDP):
                                    nc.tensor.matmul(
                                        ps_g[:, :csz],
                                        w1g[:, jj, dp],
                                        x8_sb[:, 2 * dp : 2 * dp + 2, t0 : t0 + csz],
                                        start=(dp == 0),
                                        stop=(dp == NDP - 1),
                                        perf_mode=DR,
                                    )
                                ps_u = psum.tile(
                                    [P, CHUNK_MAX], f32, tag="ps", name="ps_u"
                                )
                                for dp in range(NDP):
                                    nc.tensor.matmul(
                                        ps_u[:, :csz],
                                        w1u[:, jj, dp],
                                        x8_sb[:, 2 * dp : 2 * dp + 2, t0 : t0 + csz],
                                        start=(dp == 0),
                                        stop=(dp == NDP - 1),
                                        perf_mode=DR,
                                    )
                                hg = hpool.tile([P, CHUNK_MAX], bf16, tag="h", name="hg")
                                nc.scalar.activation(
                                    hg[:, :csz], ps_g[:, :csz], act_silu,
                                    scale=1.0 / (X8S * W1S),
                                )
                                hu = hpool.tile([P, CHUNK_MAX], bf16, tag="h", name="hu")
                                nc.scalar.activation(
                                    hu[:, :csz], ps_u[:, :csz], act_copy,
                                    scale=X8S / (X8S * W1S),
                                )
                                nc.vector.tensor_mul(
                                    aT8[:, fi, t0 : t0 + csz], hg[:, :csz], hu[:, :csz]
                                )

                    y4 = None
                    for dp in range(NDP):
                        w2sb = w28pool.tile(
                            [P, 2, NPR, 2, P], f8, tag="w28", name="w28sb"
                        )
                        nc.gpsimd.dma_start(w2sb, w2_d[dp])
                        for dj in range(2):
                            dt = 2 * dp + dj
                            if dt % 4 == 0:
                                y4 = y4pool.tile(
                                    [P, 4, S1f], bf16, tag="y4", name="y4"
                                )
                            for t0, csz in chunks:
                                ps_y = psum.tile(
                                    [P, CHUNK_MAX], f32, tag="ps", name="ps_y"
                                )
                                for q in range(NPR):
                                    nc.tensor.matmul(
                                        ps_y[:, :csz],
                                        w2sb[:, dj, q],
                                        aT8[:, 2 * q : 2 * q + 2, t0 : t0 + csz],
                                        start=(q == 0),
                                        stop=(q == NPR - 1),
                                        perf_mode=DR,
                                    )
                                nc.scalar.activation(
                                    y4[:, dt % 4, t0 : t0 + csz],
                                    ps_y[:, :csz],
                                    act_copy,
                                )
                            if dt % 4 == 3:
                                nc.gpsimd.dma_start(
                                    y_d[dt // 4], y4[:, :, :S]
                                )

    _legalize_sync(nc)
    return nc


def _ensure_ntff_hook():
    """Register the axon NTFF-profile hook if the image's antenv lacks
    ``axon_hooks`` (the hook impl ships in trn_agent_boot). Best-effort."""
    import sys
    import types

    try:
        from antenv.axon_hooks import get_axon_ntff_profile_hook  # noqa: F401

        return
    except ImportError:
        pass
    try:
        import antenv

        mod = types.ModuleType("antenv.axon_hooks")
        mod._hook = None

        def set_axon_ntff_profile_hook(h):
            mod._hook = h

        def get_axon_ntff_profile_hook():
            return mod._hook

        mod.set_axon_ntff_profile_hook = set_axon_ntff_profile_hook
        mod.get_axon_ntff_profile_hook = get_axon_ntff_profile_hook
        sys.modules["antenv.axon_hooks"] = mod
        antenv.axon_hooks = mod

        from trn_agent_boot.trn_boot import _ntff_profile_via_ctypes

        so_path = "/opt/axon/libaxon_pjrt.so"
        hook = _ntff_profile_via_ctypes(so_path)
        if hook is not None:
            mod._hook = hook
    except Exception:
        pass


def _route(x, gate_w, top_k):
    """Replicates the reference router in numpy fp32 (renormalized top-k
    softmax == softmax over the top-k logits)."""
    logits = x.astype(np.float32) @ gate_w.astype(np.float32).T  # [T, E]
    k = int(top_k)
    idx = np.argpartition(-logits, k - 1, axis=1)[:, :k]
    lv = np.take_along_axis(logits, idx, axis=1)
    m = lv.max(axis=1, keepdims=True)
    ew = np.exp(lv - m)
    wts = ew / ew.sum(axis=1, keepdims=True)
    return idx, wts.astype(np.float32)


def _make_slots(hi_counts, lo_counts):
    """Cut experts into <=SLOT_CAP segments per precision; LPT-pack onto 4
    core pairs (fp8 cost 0.5/col); return per-pair slot lists
    [(expert, lo, hi, ty), ...] (bf16 slots first) and the shared slot-spec
    tuple ((S, ty), ...) with S = max across pairs, padded to mult of 4."""
    segs = []
    for e, c in enumerate(hi_counts):
        t = 0
        while t < c:
            s = min(SLOT_CAP, c - t)
            segs.append((s, 1.0, e, t, t + s, "b"))
            t += s
    for e, c in enumerate(lo_counts):
        base = hi_counts[e]
        t = 0
        while t < c:
            s = min(SLOT_CAP, c - t)
            segs.append((s, 0.5, e, base + t, base + t + s, "f"))
            t += s
    # bf16 first (descending), then fp8 (descending), onto least-loaded pair
    segs.sort(key=lambda g: (g[5], -g[0]))
    loads = [0.0] * 4
    pair_slots = [[] for _ in range(4)]
    for s, cost, e, lo, hi, ty in segs:
        p = min(range(4), key=lambda i: loads[i])
        loads[p] += s * cost
        pair_slots[p].append((e, lo, hi, ty))
    for sl in pair_slots:
        sl.sort(key=lambda t: (t[3], -(t[2] - t[1])))
    kb = max(sum(1 for t in sl if t[3] == "b") for sl in pair_slots)
    kf = max(sum(1 for t in sl if t[3] == "f") for sl in pair_slots)
    # normalize: every pair gets kb bf slots then kf f8 slots (dummies empty)
    spec = []
    norm = [[] for _ in range(4)]
    for ji in range(kb + kf):
        ty = "b" if ji < kb else "f"
        m = 8
        for p in range(4):
            mine = [t for t in pair_slots[p] if t[3] == ty]
            i = ji if ty == "b" else ji - kb
            if i < len(mine):
                norm[p].append(mine[i])
                m = max(m, mine[i][2] - mine[i][1])
            else:
                norm[p].append(None)
        spec.append((-(-m // 4) * 4, ty))
    return norm, tuple(spec)


def kernel(x, gate_w, wv1, w2, top_k):
    import ml_dtypes

    from concourse.bass_utils import run_bass_kernel_spmd

    x = np.asarray(x)
    gate_w = np.asarray(gate_w)
    wv1 = np.asarray(wv1)
    w2 = np.asarray(w2)

    T, D = x.shape
    E, F2, _ = wv1.shape
    F = F2 // 2
    Fh = F // 2
    ND = D // P
    NF = Fh // P
    NPR = NF // 2
    NDP = ND // 2
    n_cores = 8

    idx, wts = _route(x, gate_w, top_k)

    rows_l, w_l, hi_n = [], [], []
    for e in range(E):
        rows, cols = np.nonzero(idx == e)
        w_e = wts[rows, cols]
        order = np.argsort(-w_e, kind="stable")
        rows_l.append(rows[order])
        w_l.append(w_e[order])
        hi_n.append(int((w_e >= THETA).sum()))
    counts = [len(r) for r in rows_l]
    lo_n = [c - h for c, h in zip(counts, hi_n)]

    pair_slots, slot_spec = _make_slots(hi_n, lo_n)

    key = (slot_spec, D, Fh)
    if key not in _BASS_CACHE:
        _BASS_CACHE[key] = _build_bass(slot_spec, D, Fh)
    nc = _BASS_CACHE[key]

    bf16 = ml_dtypes.bfloat16
    f8 = ml_dtypes.float8_e4m3
    x_bf = x.astype(bf16)
    x_f8 = np.clip(x * X8S, -240, 240).astype(f8)
    w1_bf = wv1.astype(bf16)
    w2_bf = w2.astype(bf16)

    w1_cache: dict = {}
    w2_cache: dict = {}

    def w1_pack(e, h, ty):
        if (e, h, ty) not in w1_cache:
            if ty == "b":
                gsl = w1_bf[e][h * Fh : (h + 1) * Fh]
                usl = w1_bf[e][F + h * Fh : F + (h + 1) * Fh]
                both = np.stack([gsl, usl])  # [2, Fh, D]
                w1_cache[(e, h, ty)] = np.ascontiguousarray(
                    both.reshape(2, NPR, 2, P, ND, P).transpose(0, 1, 5, 2, 4, 3)
                )
            else:
                gsl = wv1[e][h * Fh : (h + 1) * Fh]
                usl = wv1[e][F + h * Fh : F + (h + 1) * Fh]
                both = np.clip(np.stack([gsl, usl]) * W1S, -240, 240).astype(f8)
                w1_cache[(e, h, ty)] = np.ascontiguousarray(
                    both.reshape(2, NPR, 2, P, NDP, 2, P).transpose(0, 1, 6, 2, 4, 5, 3)
                )
        return w1_cache[(e, h, ty)]

    def w2_pack(e, h, ty):
        if (e, h, ty) not in w2_cache:
            if ty == "b":
                sl = w2_bf[e][:, h * Fh : (h + 1) * Fh]  # [D, Fh]
                w2_cache[(e, h, ty)] = np.ascontiguousarray(
                    sl.reshape(NDP, 2, P, NF, P).transpose(0, 4, 1, 3, 2)
                )
            else:
                sl = np.clip(w2[e][:, h * Fh : (h + 1) * Fh] * W2S, -240, 240).astype(f8)
                w2_cache[(e, h, ty)] = np.ascontiguousarray(
                    sl.reshape(NDP, 2, P, NPR, 2, P).transpose(0, 5, 1, 3, 4, 2)
                )
        return w2_cache[(e, h, ty)]

    in_maps = []
    for p in range(4):
        slots = pair_slots[p]
        for h in range(2):
            im = {}
            for ji, (S, ty) in enumerate(slot_spec):
                slot = slots[ji]
                if ty == "b":
                    xq, w1z, w2z = (
                        np.zeros((D, S), dtype=bf16),
                        np.zeros((2, NPR, P, 2, ND, P), dtype=bf16),
                        np.zeros((NDP, P, 2, NF, P), dtype=bf16),
                    )
                else:
                    xq, w1z, w2z = (
                        np.zeros((D, S), dtype=f8),
                        np.zeros((2, NPR, P, 2, NDP, 2, P), dtype=f8),
                        np.zeros((NDP, P, 2, NPR, 2, P), dtype=f8),
                    )
                if slot is not None:
                    e, lo, hi, _ = slot
                    seg = rows_l[e][lo:hi]
                    src = x_bf if ty == "b" else x_f8
                    xq[:, : hi - lo] = src[seg].T
                    w1z = w1_pack(e, h, ty)
                    w2z = w2_pack(e, h, ty)
                im[f"x{ji}"] = np.ascontiguousarray(xq.reshape(ND, P, S))
                im[f"w1{ji}"] = w1z
                im[f"w2{ji}"] = w2z
            in_maps.append(im)

    _ensure_ntff_hook()
    res = run_bass_kernel_spmd(nc, in_maps, core_ids=list(range(n_cores)))
    global last_run
    last_run = res

    out = np.zeros((T, D), dtype=np.float32)
    for p in range(4):
        for ji, (S, ty) in enumerate(slot_spec):
            slot = pair_slots[p][ji]
            if slot is None:
                continue
            e, lo, hi, _ = slot
            n = hi - lo
            if ty == "b":
                y0 = res.results[2 * p][f"y{ji}"].reshape(D, -1)[:, :n]
                y1 = res.results[2 * p + 1][f"y{ji}"].reshape(D, -1)[:, :n]
            else:
                y0 = (
                    res.results[2 * p][f"y{ji}"]
                    .transpose(0, 2, 1, 3)
                    .reshape(D, -1)[:, :n]
                )
                y1 = (
                    res.results[2 * p + 1][f"y{ji}"]
                    .transpose(0, 2, 1, 3)
                    .reshape(D, -1)[:, :n]
                )
            ysum = y0.astype(np.float32) + y1.astype(np.float32)
            if ty == "f":
                ysum *= 1.0 / Y8S
            seg = rows_l[e][lo:hi]
            out[seg] += w_l[e][lo:hi, None] * ysum.T
    return out.astype(x.dtype, copy=False)


# revision 13
# speedup vs baseline: 1.3145x; 1.0136x over previous
"""Block-sparse MoE (softmax top-k routing + silu-gated FFN) on 8 Trainium2 cores.

Sharding: expert-pair x FFN-half. The router runs on host; each expert's
token list is sorted by router weight and split at THETA: high-weight
pairs run in bf16, low-weight pairs in fp8 e4m3 (DoubleRow, 2x tensor
rate) — the output error each fp8 token contributes is proportional to
its (small) router weight, keeping total rel err ~1.2e-2 vs the 2e-2
gate. Segments are LPT-packed onto 4 core-pairs (fp8 cost 0.5/col) and
each pair of cores splits the FFN dim in half (1792 of 3584), so all 8
cores run the same slot-size program (SPMD) with balanced cycles.

Per slot on a core (S tokens, ND=16 d-tiles, NFh=14 f-tiles):
  phase1: h.T [128 f, chunk] = w1h.T @ x (contract D); silu(g)*u -> aT
  phase3: y[d, tok] = sum_fi w2h[f,d].T @ aT[:, fi, :] (contract F/2)
          d on PSUM partitions, tokens moving => exact columns, no pad.
  fp8 slots: x*16, w1*32 -> psum = 512*h; silu via ACT scale 1/512;
  u16 = psum/32; a8 = e4m3(silu*u16) = 16a; w2*64 -> y*1024 (host /1024).

Moving chunks are equalized (~420-510) so LDWEIGHTS (~97-116ns) hides
under the moving phase. Slot0's x arrives dt-major in two column chunks
consumed dt-outer across 8 PSUM banks (PE starts ~12us in). w1 prefetch
depth 4 covers the ~6us pair-tile transfer. All bulk traffic rides the
SWDGE queue with >=2KB lines; later slots' x and w1 overlap prior ph3.
Host sums the two F-halves, applies router weight, scatter-adds (f32).
"""

import numpy as np


def _ensure_concourse_on_path():
    try:
        import concourse  # noqa: F401
    except ImportError:
        import sys

        for p in ("/opt/trn_rl_repo", "/root/.axon_site/_ro/trn_rl_repo"):
            if p not in sys.path:
                sys.path.insert(0, p)


_ensure_concourse_on_path()

P = 128
CHUNK_MAX = 512   # PSUM bank free-dim limit (f32)
SLOT_CAP = 2560   # max tokens per slot (SBUF-bound)
THETA = 0.37      # router-weight cutoff: below -> fp8 path
X8S, W1S, W2S = 16.0, 32.0, 64.0
Y8S = X8S * W2S   # fp8-slot output scale (host divides)

_BASS_CACHE: dict = {}
last_run = None  # BassKernelResults of the most recent kernel() call (for test.py)


def _legalize_sync(nc, max_waits: int = 1):
    """Split multi-wait sync_info into preceding EventSemaphore instructions.

    The walrus build in this container lowers every instruction with capacity
    for a single sync-wait command and errors with "Too many sync wait
    commands" otherwise, while Tile attaches up to 3 waits per instruction.
    A wait carried by an EventSemaphore on the same engine immediately before
    the instruction is semantically identical. For DMAs, keep the own-lane
    FIFO wait on the instruction itself so the in-queue wait doesn't stall
    the sequencer.
    """
    import concourse.mybir as mybir

    fn = nc.m.functions[0]
    for blk in fn.blocks:
        new_insts = []
        for inst in blk.instructions:
            si = inst.sync_info
            if si is not None and si.on_wait is not None and len(si.on_wait) > max_waits:
                ow = list(si.on_wait)
                upd_ids = {u.id for u in (si.on_update or [])}
                keep = [w for w in ow if w.id in upd_ids][:1]
                if not keep:
                    keep = [ow[-1]]
                for j, w in enumerate(ow):
                    if w is keep[0]:
                        continue
                    new_insts.append(
                        mybir.InstEventSemaphore(
                            name=f"{inst.name}-ws{j}",
                            opcode="EventSemaphore",
                            engine=inst.engine,
                            sync_info=mybir.SyncInfo(on_wait=[w], on_update=[]),
                        )
                    )
                si.on_wait = keep
            new_insts.append(inst)
        blk.instructions = new_insts


def _chunk_list(S: int):
    """Equalized moving chunks: k = ceil(S/512), base = ceil(S/k) rounded to
    a multiple of 4, so every chunk is large enough that LDWEIGHTS hides
    under the moving phase."""
    k = -(-S // CHUNK_MAX)
    base = -(-S // k)
    base = -(-base // 4) * 4
    chunks = []
    t0 = 0
    while t0 < S:
        c = min(base, S - t0)
        chunks.append((t0, c))
        t0 += c
    return chunks


def _build_bass(slot_spec: tuple, D: int, Fh: int):
    """Bass program: sequence of expert slots, each (S tokens, ty) with
    ty 'b' (bf16) or 'f' (fp8 e4m3 DoubleRow). Per-slot inputs x{j}, w1{j},
    w2{j}; outputs y{j}."""
    import concourse.bass as bass
    import concourse.mybir as mybir
    import concourse.tile as tile

    ND = D // P           # 16 contraction tiles (phase 1)
    NF = Fh // P          # 14 f-tiles per half
    NPR = NF // 2         # 7 w1 pairs
    NDP = ND // 2         # 8 dt-pair groups
    assert NF % 2 == 0 and ND % 2 == 0
    S1b = max((S for S, ty in slot_spec if ty == "b"), default=8)
    S1f = max((S for S, ty in slot_spec if ty == "f"), default=8)

    bf16 = mybir.dt.bfloat16
    f8 = mybir.dt.float8e4
    f32 = mybir.dt.float32
    DR = mybir.MatmulPerfMode.DoubleRow

    nc = bass.Bass(name="moe_pair_ffn", num_swdge_queues=4)
    xs_d, w1s_d, w2s_d, ys_d = [], [], [], []
    for j, (S, ty) in enumerate(slot_spec):
        if ty == "b":
            xs_d.append(nc.dram_tensor(f"x{j}", [ND, P, S], bf16, kind="ExternalInput"))
            w1s_d.append(
                nc.dram_tensor(
                    f"w1{j}", [2, NPR, P, 2, ND, P], bf16, kind="ExternalInput"
                )
            )
            w2s_d.append(
                nc.dram_tensor(f"w2{j}", [NDP, P, 2, NF, P], bf16, kind="ExternalInput")
            )
        else:
            xs_d.append(nc.dram_tensor(f"x{j}", [ND, P, S], f8, kind="ExternalInput"))
            w1s_d.append(
                nc.dram_tensor(
                    f"w1{j}", [2, NPR, P, 2, NDP, 2, P], f8, kind="ExternalInput"
                )
            )
            w2s_d.append(
                nc.dram_tensor(
                    f"w2{j}", [NDP, P, 2, NPR, 2, P], f8, kind="ExternalInput"
                )
            )
        if ty == "b":
            ys_d.append(
                nc.dram_tensor(f"y{j}", [ND, P, S], bf16, kind="ExternalOutput")
            )
        else:
            ys_d.append(
                nc.dram_tensor(
                    f"y{j}", [ND // 4, P, 4, S], bf16, kind="ExternalOutput"
                )
            )

    act_silu = mybir.ActivationFunctionType.Silu
    act_copy = mybir.ActivationFunctionType.Copy

    with tile.TileContext(nc) as tc:
        with (
            tc.tile_pool(name="xp", bufs=1) as xpool,
            tc.tile_pool(name="ap", bufs=1) as apool,
            tc.tile_pool(name="w1p", bufs=3) as w1pool,
            tc.tile_pool(name="w2p", bufs=3) as w2pool,
            tc.tile_pool(name="hp", bufs=3) as hpool,
            tc.tile_pool(name="yp", bufs=3) as ypool,
            tc.tile_pool(name="x8p", bufs=1) as x8pool,
            tc.tile_pool(name="w18p", bufs=4) as w18pool,
            tc.tile_pool(name="w28p", bufs=4) as w28pool,
            tc.tile_pool(name="y4p", bufs=2) as y4pool,
            tc.tile_pool(name="ps", bufs=8, space="PSUM") as psum,
        ):
            # fp8 tiles live in their own pools, declared after the bf16
            # pools so the bf16 phase keeps the measured conflict-free
            # SBUF layout (shifting pool bases cost +35ns/matmul once).
            x_sb = xpool.tile([P, ND, S1b], bf16)
            aT = apool.tile([P, NF, S1b], bf16)
            has_f8 = any(ty == "f" for _, ty in slot_spec)
            if has_f8:
                x8_sb = x8pool.tile([P, ND, S1f], f8)
                aT8 = x8pool.tile([P, NF, S1f], f8)

            x8_hoisted: set = set()
            w18_hoisted: dict = {}
            for j, (S, ty) in enumerate(slot_spec):
                chunks = _chunk_list(S)
                x_d, w1_d, w2_d, y_d = xs_d[j], w1s_d[j], w2s_d[j], ys_d[j]

                if ty == "b":
                    # ---------------- bf16 slot ----------------
                    xsplit = -(-S // 8) * 4
                    startup = (
                        j == 0
                        and len(chunks) >= 3
                        and chunks[1][0] + chunks[1][1] <= xsplit
                    )
                    w1g0 = w1pool.tile([P, 2, ND, P], bf16, tag="w1", name="w1g0")
                    w1u0 = w1pool.tile([P, 2, ND, P], bf16, tag="w1", name="w1u0")
                    if startup:
                        # queue order: 0.5MB of w1 (gate/jj0), x chunk0
                        # dt-major, rest of pair0, x chunk1 — the dt-outer
                        # sub-phases below consume in exactly this order.
                        nc.gpsimd.dma_start(w1g0[:, 0], w1_d[0, 0, :, 0])
                        for dt in range(ND):
                            nc.gpsimd.dma_start(
                                x_sb[:, dt, :xsplit], x_d[dt, :, :xsplit]
                            )
                        nc.gpsimd.dma_start(w1g0[:, 1], w1_d[0, 0, :, 1])
                        nc.gpsimd.dma_start(w1u0[:, 0], w1_d[1, 0, :, 0])
                        for dt in range(ND):
                            nc.gpsimd.dma_start(
                                x_sb[:, dt, xsplit:S], x_d[dt, :, xsplit:]
                            )
                        nc.gpsimd.dma_start(w1u0[:, 1], w1_d[1, 0, :, 1])
                    else:
                        nc.gpsimd.dma_start(w1g0, w1_d[0, 0])
                        nc.gpsimd.dma_start(w1u0, w1_d[1, 0])
                        for dt in range(ND):
                            nc.gpsimd.dma_start(x_sb[:, dt, :S], x_d[dt])

                    def ph1_chain(w1t, jj, t0, csz, start, stop, ps):
                        for dt in range(ND):
                            nc.tensor.matmul(
                                ps[:, :csz],
                                w1t[:, jj, dt, :],
                                x_sb[:, dt, t0 : t0 + csz],
                                start=start and dt == 0,
                                stop=stop and dt == ND - 1,
                            )

                    def ph1_evac(fi, t0, csz, ps_g, ps_u):
                        hg = hpool.tile([P, CHUNK_MAX], bf16, tag="h", name="hg")
                        nc.scalar.activation(hg[:, :csz], ps_g[:, :csz], act_silu)
                        nc.vector.tensor_mul(
                            aT[:, fi, t0 : t0 + csz], hg[:, :csz], ps_u[:, :csz]
                        )

                    pr_start = 0
                    if startup:
                        sc = chunks[:2]
                        banks = {}
                        for gu in range(2):
                            for jj in range(2):
                                for ci in range(len(sc)):
                                    banks[(jj, ci, gu)] = psum.tile(
                                        [P, CHUNK_MAX], f32, tag="ps",
                                        name=f"ps_s{jj}{ci}{gu}",
                                    )
                        for gu, jj in ((0, 0), (0, 1), (1, 0), (1, 1)):
                            w1t = w1g0 if gu == 0 else w1u0
                            for dt in range(ND):
                                for ci, (t0, csz) in enumerate(sc):
                                    nc.tensor.matmul(
                                        banks[(jj, ci, gu)][:, :csz],
                                        w1t[:, jj, dt, :],
                                        x_sb[:, dt, t0 : t0 + csz],
                                        start=(dt == 0),
                                        stop=(dt == ND - 1),
                                    )
                            if gu == 1:
                                for ci, (t0, csz) in enumerate(sc):
                                    ph1_evac(
                                        jj, t0, csz,
                                        banks[(jj, ci, 0)], banks[(jj, ci, 1)],
                                    )
                        for jj in range(2):
                            for t0, csz in chunks[2:]:
                                ps_g = psum.tile(
                                    [P, CHUNK_MAX], f32, tag="ps", name="ps_g"
                                )
                                ph1_chain(w1g0, jj, t0, csz, True, True, ps_g)
                                ps_u = psum.tile(
                                    [P, CHUNK_MAX], f32, tag="ps", name="ps_u"
                                )
                                ph1_chain(w1u0, jj, t0, csz, True, True, ps_u)
                                ph1_evac(jj, t0, csz, ps_g, ps_u)
                        pr_start = 1

                    for pr in range(pr_start, NPR):
                        if pr == 0:
                            w1g, w1u = w1g0, w1u0
                        else:
                            w1g = w1pool.tile([P, 2, ND, P], bf16, tag="w1", name="w1g")
                            nc.gpsimd.dma_start(w1g, w1_d[0, pr])
                            w1u = w1pool.tile([P, 2, ND, P], bf16, tag="w1", name="w1u")
                            nc.gpsimd.dma_start(w1u, w1_d[1, pr])
                        for jj in range(2):
                            for t0, csz in chunks:
                                ps_g = psum.tile(
                                    [P, CHUNK_MAX], f32, tag="ps", name="ps_g"
                                )
                                ph1_chain(w1g, jj, t0, csz, True, True, ps_g)
                                ps_u = psum.tile(
                                    [P, CHUNK_MAX], f32, tag="ps", name="ps_u"
                                )
                                ph1_chain(w1u, jj, t0, csz, True, True, ps_u)
                                ph1_evac(2 * pr + jj, t0, csz, ps_g, ps_u)

                    for dp in range(NDP):
                        w2sb = w2pool.tile([P, 2, NF, P], bf16, tag="w2", name="w2sb")
                        nc.gpsimd.dma_start(w2sb, w2_d[dp])
                        for dj in range(2):
                            dt = 2 * dp + dj
                            y_st = ypool.tile([P, S1b], bf16, tag="y", name="y_st")
                            for t0, csz in chunks:
                                ps_y = psum.tile(
                                    [P, CHUNK_MAX], f32, tag="ps", name="ps_y"
                                )
                                for fi in range(NF):
                                    nc.tensor.matmul(
                                        ps_y[:, :csz],
                                        w2sb[:, dj, fi, :],
                                        aT[:, fi, t0 : t0 + csz],
                                        start=(fi == 0),
                                        stop=(fi == NF - 1),
                                    )
                                nc.scalar.activation(
                                    y_st[:, t0 : t0 + csz], ps_y[:, :csz], act_copy
                                )
                            (nc.sync if dt % 2 == 0 else nc.gpsimd).dma_start(
                                y_d[dt], y_st[:, :S]
                            )
                    # prefetch the next (fp8) slot's x8 + first w18 pair ahead
                    # of this ph3's throttled y DMAs so they aren't
                    # head-blocked on the SWDGE queue.
                    if j + 1 < len(slot_spec) and slot_spec[j + 1][1] == "f":
                        Sn = slot_spec[j + 1][0]
                        for dt in range(ND):
                            nc.gpsimd.dma_start(
                                x8_sb[:, dt, :Sn], xs_d[j + 1][dt]
                            )
                        x8_hoisted.add(j + 1)
                        hg0 = w18pool.tile(
                            [P, 2, NDP, 2, P], f8, tag="w18", name="w18hg"
                        )
                        nc.gpsimd.dma_start(hg0, w1s_d[j + 1][0, 0])
                        hu0 = w18pool.tile(
                            [P, 2, NDP, 2, P], f8, tag="w18", name="w18hu"
                        )
                        nc.gpsimd.dma_start(hu0, w1s_d[j + 1][1, 0])
                        hg1 = w18pool.tile(
                            [P, 2, NDP, 2, P], f8, tag="w18", name="w18hg1"
                        )
                        nc.gpsimd.dma_start(hg1, w1s_d[j + 1][0, 1])
                        hu1 = w18pool.tile(
                            [P, 2, NDP, 2, P], f8, tag="w18", name="w18hu1"
                        )
                        nc.gpsimd.dma_start(hu1, w1s_d[j + 1][1, 1])
                        w18_hoisted[j + 1] = ((hg0, hu0), (hg1, hu1))
                else:
                    # ---------------- fp8 slot (DoubleRow) ----------------
                    if j not in x8_hoisted:
                        for dt in range(ND):
                            nc.gpsimd.dma_start(x8_sb[:, dt, :S], x_d[dt])

                    for pr in range(NPR):
                        if pr < 2 and j in w18_hoisted:
                            w1g, w1u = w18_hoisted[j][pr]
                        else:
                            w1g = w18pool.tile(
                                [P, 2, NDP, 2, P], f8, tag="w18", name="w18g"
                            )
                            nc.gpsimd.dma_start(w1g, w1_d[0, pr])
                            w1u = w18pool.tile(
                                [P, 2, NDP, 2, P], f8, tag="w18", name="w18u"
                            )
                            nc.gpsimd.dma_start(w1u, w1_d[1, pr])
                        for jj in range(2):
                            fi = 2 * pr + jj
                            for t0, csz in chunks:
                                ps_g = psum.tile(
                                    [P, CHUNK_MAX], f32, tag="ps", name="ps_g"
                                )
                                for dp in range(NDP):
                                    nc.tensor.matmul(
                                        ps_g[:, :csz],
                                        w1g[:, jj, dp],
                                        x8_sb[:, 2 * dp : 2 * dp + 2, t0 : t0 + csz],
                                        start=(dp == 0),
                                        stop=(dp == NDP - 1),
                                        perf_mode=DR,
                                    )
                                ps_u = psum.tile(
                                    [P, CHUNK_MAX], f32, tag="ps", name="ps_u"
                                )
                                for dp in range(NDP):
                                    nc.tensor.matmul(
                                        ps_u[:, :csz],
                                        w1u[:, jj, dp],
                                        x8_sb[:, 2 * dp : 2 * dp + 2, t0 : t0 + csz],
                                        start=(dp == 0),
                                        stop=(dp == NDP - 1),
                                        perf_mode=DR,
                                    )
                                hg = hpool.tile([P, CHUNK_MAX], bf16, tag="h", name="hg")
                                nc.scalar.activation(
                                    hg[:, :csz], ps_g[:, :csz], act_silu,
                                    scale=1.0 / (X8S * W1S),
                                )
                                hu = hpool.tile([P, CHUNK_MAX], bf16, tag="h", name="hu")
                                nc.scalar.activation(
                                    hu[:, :csz], ps_u[:, :csz], act_copy,
                                    scale=X8S / (X8S * W1S),
                                )
                                nc.vector.tensor_mul(
                                    aT8[:, fi, t0 : t0 + csz], hg[:, :csz], hu[:, :csz]
                                )

                    y4 = None
                    for dp in range(NDP):
                        w2sb = w28pool.tile(
                            [P, 2, NPR, 2, P], f8, tag="w28", name="w28sb"
                        )
                        nc.gpsimd.dma_start(w2sb, w2_d[dp])
                        for dj in range(2):
                            dt = 2 * dp + dj
                            if dt % 4 == 0:
                                y4 = y4pool.tile(
                                    [P, 4, S1f], bf16, tag="y4", name="y4"
                                )
                            for t0, csz in chunks:
                                ps_y = psum.tile(
                                    [P, CHUNK_MAX], f32, tag="ps", name="ps_y"
                                )
                                for q in range(NPR):
                                    nc.tensor.matmul(
                                        ps_y[:, :csz],
                                        w2sb[:, dj, q],
                                        aT8[:, 2 * q : 2 * q + 2, t0 : t0 + csz],
                                        start=(q == 0),
                                        stop=(q == NPR - 1),
                                        perf_mode=DR,
                                    )
                                nc.scalar.activation(
                                    y4[:, dt % 4, t0 : t0 + csz],
                                    ps_y[:, :csz],
                                    act_copy,
                                )
                            if dt % 4 == 3:
                                nc.gpsimd.dma_start(
                                    y_d[dt // 4], y4[:, :, :S]
                                )

    _legalize_sync(nc)
    return nc


def _ensure_ntff_hook():
    """Register the axon NTFF-profile hook if the image's antenv lacks
    ``axon_hooks`` (the hook impl ships in trn_agent_boot). Best-effort."""
    import sys
    import types

    try:
        from antenv.axon_hooks import get_axon_ntff_profile_hook  # noqa: F401

        return
    except ImportError:
        pass
    try:
        import antenv

        mod = types.ModuleType("antenv.axon_hooks")
        mod._hook = None

        def set_axon_ntff_profile_hook(h):
            mod._hook = h

        def get_axon_ntff_profile_hook():
            return mod._hook

        mod.set_axon_ntff_profile_hook = set_axon_ntff_profile_hook
        mod.get_axon_ntff_profile_hook = get_axon_ntff_profile_hook
        sys.modules["antenv.axon_hooks"] = mod
        antenv.axon_hooks = mod

        from trn_agent_boot.trn_boot import _ntff_profile_via_ctypes

        so_path = "/opt/axon/libaxon_pjrt.so"
        hook = _ntff_profile_via_ctypes(so_path)
        if hook is not None:
            mod._hook = hook
    except Exception:
        pass


def _route(x, gate_w, top_k):
    """Replicates the reference router in numpy fp32 (renormalized top-k
    softmax == softmax over the top-k logits)."""
    logits = x.astype(np.float32) @ gate_w.astype(np.float32).T  # [T, E]
    k = int(top_k)
    idx = np.argpartition(-logits, k - 1, axis=1)[:, :k]
    lv = np.take_along_axis(logits, idx, axis=1)
    m = lv.max(axis=1, keepdims=True)
    ew = np.exp(lv - m)
    wts = ew / ew.sum(axis=1, keepdims=True)
    return idx, wts.astype(np.float32)


def _make_slots(hi_counts, lo_counts):
    """Cut experts into <=SLOT_CAP segments per precision; LPT-pack onto 4
    core pairs (fp8 cost 0.5/col); return per-pair slot lists
    [(expert, lo, hi, ty), ...] (bf16 slots first) and the shared slot-spec
    tuple ((S, ty), ...) with S = max across pairs, padded to mult of 4."""
    segs = []
    for e, c in enumerate(hi_counts):
        t = 0
        while t < c:
            s = min(SLOT_CAP, c - t)
            segs.append((s, 1.0, e, t, t + s, "b"))
            t += s
    for e, c in enumerate(lo_counts):
        base = hi_counts[e]
        t = 0
        while t < c:
            s = min(SLOT_CAP, c - t)
            segs.append((s, 0.5, e, base + t, base + t + s, "f"))
            t += s
    # bf16 first (descending), then fp8 (descending), onto least-loaded pair
    segs.sort(key=lambda g: (g[5], -g[0]))
    loads = [0.0] * 4
    pair_slots = [[] for _ in range(4)]
    for s, cost, e, lo, hi, ty in segs:
        p = min(range(4), key=lambda i: loads[i])
        loads[p] += s * cost
        pair_slots[p].append((e, lo, hi, ty))
    for sl in pair_slots:
        sl.sort(key=lambda t: (t[3], -(t[2] - t[1])))
    kb = max(sum(1 for t in sl if t[3] == "b") for sl in pair_slots)
    kf = max(sum(1 for t in sl if t[3] == "f") for sl in pair_slots)
    # normalize: every pair gets kb bf slots then kf f8 slots (dummies empty)
    spec = []
    norm = [[] for _ in range(4)]
    for ji in range(kb + kf):
        ty = "b" if ji < kb else "f"
        m = 8
        for p in range(4):
            mine = [t for t in pair_slots[p] if t[3] == ty]
            i = ji if ty == "b" else ji - kb
            if i < len(mine):
                norm[p].append(mine[i])
                m = max(m, mine[i][2] - mine[i][1])
            else:
                norm[p].append(None)
        spec.append((-(-m // 4) * 4, ty))
    return norm, tuple(spec)


def kernel(x, gate_w, wv1, w2, top_k):
    import ml_dtypes

    from concourse.bass_utils import run_bass_kernel_spmd

    x = np.asarray(x)
    gate_w = np.asarray(gate_w)
    wv1 = np.asarray(wv1)
    w2 = np.asarray(w2)

    T, D = x.shape
    E, F2, _ = wv1.shape
    F = F2 // 2
    Fh = F // 2
    ND = D // P
    NF = Fh // P
    NPR = NF // 2
    NDP = ND // 2
    n_cores = 8

    idx, wts = _route(x, gate_w, top_k)

    rows_l, w_l, hi_n = [], [], []
    for e in range(E):
        rows, cols = np.nonzero(idx == e)
        w_e = wts[rows, cols]
        order = np.argsort(-w_e, kind="stable")
        rows_l.append(rows[order])
        w_l.append(w_e[order])
        hi_n.append(int((w_e >= THETA).sum()))
    counts = [len(r) for r in rows_l]
    lo_n = [c - h for c, h in zip(counts, hi_n)]

    pair_slots, slot_spec = _make_slots(hi_n, lo_n)

    key = (slot_spec, D, Fh)
    if key not in _BASS_CACHE:
        _BASS_CACHE[key] = _build_bass(slot_spec, D, Fh)
    nc = _BASS_CACHE[key]

    bf16 = ml_dtypes.bfloat16
    f8 = ml_dtypes.float8_e4m3
    x_bf = x.astype(bf16)
    x_f8 = np.clip(x * X8S, -240, 240).astype(f8)
    w1_bf = wv1.astype(bf16)
    w2_bf = w2.astype(bf16)

    w1_cache: dict = {}
    w2_cache: dict = {}

    def w1_pack(e, h, ty):
        if (e, h, ty) not in w1_cache:
            if ty == "b":
                gsl = w1_bf[e][h * Fh : (h + 1) * Fh]
                usl = w1_bf[e][F + h * Fh : F + (h + 1) * Fh]
                both = np.stack([gsl, usl])  # [2, Fh, D]
                w1_cache[(e, h, ty)] = np.ascontiguousarray(
                    both.reshape(2, NPR, 2, P, ND, P).transpose(0, 1, 5, 2, 4, 3)
                )
            else:
                gsl = wv1[e][h * Fh : (h + 1) * Fh]
                usl = wv1[e][F + h * Fh : F + (h + 1) * Fh]
                both = np.clip(np.stack([gsl, usl]) * W1S, -240, 240).astype(f8)
                w1_cache[(e, h, ty)] = np.ascontiguousarray(
                    both.reshape(2, NPR, 2, P, NDP, 2, P).transpose(0, 1, 6, 2, 4, 5, 3)
                )
        return w1_cache[(e, h, ty)]

    def w2_pack(e, h, ty):
        if (e, h, ty) not in w2_cache:
            if ty == "b":
                sl = w2_bf[e][:, h * Fh : (h + 1) * Fh]  # [D, Fh]
                w2_cache[(e, h, ty)] = np.ascontiguousarray(
                    sl.reshape(NDP, 2, P, NF, P).transpose(0, 4, 1, 3, 2)
                )
            else:
                sl = np.clip(w2[e][:, h * Fh : (h + 1) * Fh] * W2S, -240, 240).astype(f8)
                w2_cache[(e, h, ty)] = np.ascontiguousarray(
                    sl.reshape(NDP, 2, P, NPR, 2, P).transpose(0, 5, 1, 3, 4, 2)
                )
        return w2_cache[(e, h, ty)]

    in_maps = []
    for p in range(4):
        slots = pair_slots[p]
        for h in range(2):
            im = {}
            for ji, (S, ty) in enumerate(slot_spec):
                slot = slots[ji]
                if ty == "b":
                    xq, w1z, w2z = (
                        np.zeros((D, S), dtype=bf16),
                        np.zeros((2, NPR, P, 2, ND, P), dtype=bf16),
                        np.zeros((NDP, P, 2, NF, P), dtype=bf16),
                    )
                else:
                    xq, w1z, w2z = (
                        np.zeros((D, S), dtype=f8),
                        np.zeros((2, NPR, P, 2, NDP, 2, P), dtype=f8),
                        np.zeros((NDP, P, 2, NPR, 2, P), dtype=f8),
                    )
                if slot is not None:
                    e, lo, hi, _ = slot
                    seg = rows_l[e][lo:hi]
                    src = x_bf if ty == "b" else x_f8
                    xq[:, : hi - lo] = src[seg].T
                    w1z = w1_pack(e, h, ty)
                    w2z = w2_pack(e, h, ty)
                im[f"x{ji}"] = np.ascontiguousarray(xq.reshape(ND, P, S))
                im[f"w1{ji}"] = w1z
                im[f"w2{ji}"] = w2z
            in_maps.append(im)

    _ensure_ntff_hook()
    res = run_bass_kernel_spmd(nc, in_maps, core_ids=list(range(n_cores)))
    global last_run
    last_run = res

    out = np.zeros((T, D), dtype=np.float32)
    for p in range(4):
        for ji, (S, ty) in enumerate(slot_spec):
            slot = pair_slots[p][ji]
            if slot is None:
                continue
            e, lo, hi, _ = slot
            n = hi - lo
            if ty == "b":
                y0 = res.results[2 * p][f"y{ji}"].reshape(D, -1)[:, :n]
                y1 = res.results[2 * p + 1][f"y{ji}"].reshape(D, -1)[:, :n]
            else:
                y0 = (
                    res.results[2 * p][f"y{ji}"]
                    .transpose(0, 2, 1, 3)
                    .reshape(D, -1)[:, :n]
                )
                y1 = (
                    res.results[2 * p + 1][f"y{ji}"]
                    .transpose(0, 2, 1, 3)
                    .reshape(D, -1)[:, :n]
                )
            ysum = y0.astype(np.float32) + y1.astype(np.float32)
            if ty == "f":
                ysum *= 1.0 / Y8S
            seg = rows_l[e][lo:hi]
            out[seg] += w_l[e][lo:hi, None] * ysum.T
    return out.astype(x.dtype, copy=False)


# revision 14
# speedup vs baseline: 1.3182x; 1.0028x over previous
"""Block-sparse MoE (softmax top-k routing + silu-gated FFN) on 8 Trainium2 cores.

Sharding: expert-pair x FFN-half. The router runs on host; each expert's
token list is sorted by router weight and split at THETA: high-weight
pairs run in bf16, low-weight pairs in fp8 e4m3 (DoubleRow, 2x tensor
rate) — the output error each fp8 token contributes is proportional to
its (small) router weight, keeping total rel err ~1.2e-2 vs the 2e-2
gate. Segments are LPT-packed onto 4 core-pairs (fp8 cost 0.5/col) and
each pair of cores splits the FFN dim in half (1792 of 3584), so all 8
cores run the same slot-size program (SPMD) with balanced cycles.

Per slot on a core (S tokens, ND=16 d-tiles, NFh=14 f-tiles):
  phase1: h.T [128 f, chunk] = w1h.T @ x (contract D); silu(g)*u -> aT
  phase3: y[d, tok] = sum_fi w2h[f,d].T @ aT[:, fi, :] (contract F/2)
          d on PSUM partitions, tokens moving => exact columns, no pad.
  fp8 slots: x*16, w1*32 -> psum = 512*h; silu via ACT scale 1/512;
  u16 = psum/32; a8 = e4m3(silu*u16) = 16a; w2*64 -> y*1024 (host /1024).

Moving chunks are equalized (~420-510) so LDWEIGHTS (~97-116ns) hides
under the moving phase. Slot0's x arrives dt-major in two column chunks
consumed dt-outer across 8 PSUM banks (PE starts ~12us in). w1 prefetch
depth 4 covers the ~6us pair-tile transfer. All bulk traffic rides the
SWDGE queue with >=2KB lines; later slots' x and w1 overlap prior ph3.
Host sums the two F-halves, applies router weight, scatter-adds (f32).
"""

import numpy as np


def _ensure_concourse_on_path():
    try:
        import concourse  # noqa: F401
    except ImportError:
        import sys

        for p in ("/opt/trn_rl_repo", "/root/.axon_site/_ro/trn_rl_repo"):
            if p not in sys.path:
                sys.path.insert(0, p)


_ensure_concourse_on_path()

P = 128
CHUNK_MAX = 512   # PSUM bank free-dim limit (f32)
SLOT_CAP = 2560   # max tokens per slot (SBUF-bound)
THETA = 0.37      # router-weight cutoff: below -> fp8 path
X8S, W1S, W2S = 16.0, 32.0, 64.0
Y8S = X8S * W2S   # fp8-slot output scale (host divides)

_BASS_CACHE: dict = {}
last_run = None  # BassKernelResults of the most recent kernel() call (for test.py)


def _legalize_sync(nc, max_waits: int = 1):
    """Split multi-wait sync_info into preceding EventSemaphore instructions.

    The walrus build in this container lowers every instruction with capacity
    for a single sync-wait command and errors with "Too many sync wait
    commands" otherwise, while Tile attaches up to 3 waits per instruction.
    A wait carried by an EventSemaphore on the same engine immediately before
    the instruction is semantically identical. For DMAs, keep the own-lane
    FIFO wait on the instruction itself so the in-queue wait doesn't stall
    the sequencer.
    """
    import concourse.mybir as mybir

    fn = nc.m.functions[0]
    for blk in fn.blocks:
        new_insts = []
        for inst in blk.instructions:
            si = inst.sync_info
            if si is not None and si.on_wait is not None and len(si.on_wait) > max_waits:
                ow = list(si.on_wait)
                upd_ids = {u.id for u in (si.on_update or [])}
                keep = [w for w in ow if w.id in upd_ids][:1]
                if not keep:
                    keep = [ow[-1]]
                for j, w in enumerate(ow):
                    if w is keep[0]:
                        continue
                    new_insts.append(
                        mybir.InstEventSemaphore(
                            name=f"{inst.name}-ws{j}",
                            opcode="EventSemaphore",
                            engine=inst.engine,
                            sync_info=mybir.SyncInfo(on_wait=[w], on_update=[]),
                        )
                    )
                si.on_wait = keep
            new_insts.append(inst)
        blk.instructions = new_insts


def _chunk_list(S: int):
    """Equalized moving chunks: k = ceil(S/512), base = ceil(S/k) rounded to
    a multiple of 4, so every chunk is large enough that LDWEIGHTS hides
    under the moving phase."""
    k = -(-S // CHUNK_MAX)
    base = -(-S // k)
    base = -(-base // 4) * 4
    chunks = []
    t0 = 0
    while t0 < S:
        c = min(base, S - t0)
        chunks.append((t0, c))
        t0 += c
    return chunks


def _build_bass(slot_spec: tuple, D: int, Fh: int):
    """Bass program: sequence of expert slots, each (S tokens, ty) with
    ty 'b' (bf16) or 'f' (fp8 e4m3 DoubleRow). Per-slot inputs x{j}, w1{j},
    w2{j}; outputs y{j}."""
    import concourse.bass as bass
    import concourse.mybir as mybir
    import concourse.tile as tile

    ND = D // P           # 16 contraction tiles (phase 1)
    NF = Fh // P          # 14 f-tiles per half
    NPR = NF // 2         # 7 w1 pairs
    NDP = ND // 2         # 8 dt-pair groups
    assert NF % 2 == 0 and ND % 2 == 0
    S1b = max((S for S, ty in slot_spec if ty == "b"), default=8)
    S1f = max((S for S, ty in slot_spec if ty == "f"), default=8)

    bf16 = mybir.dt.bfloat16
    f8 = mybir.dt.float8e4
    f32 = mybir.dt.float32
    DR = mybir.MatmulPerfMode.DoubleRow

    nc = bass.Bass(name="moe_pair_ffn", num_swdge_queues=4)
    xs_d, w1s_d, w2s_d, ys_d = [], [], [], []
    for j, (S, ty) in enumerate(slot_spec):
        if ty == "b":
            xs_d.append(nc.dram_tensor(f"x{j}", [ND, P, S], bf16, kind="ExternalInput"))
            w1s_d.append(
                nc.dram_tensor(
                    f"w1{j}", [2, NPR, P, 2, ND, P], bf16, kind="ExternalInput"
                )
            )
            w2s_d.append(
                nc.dram_tensor(f"w2{j}", [NDP, P, 2, NF, P], bf16, kind="ExternalInput")
            )
        else:
            xs_d.append(nc.dram_tensor(f"x{j}", [ND, P, S], f8, kind="ExternalInput"))
            w1s_d.append(
                nc.dram_tensor(
                    f"w1{j}", [2, NPR, P, 2, NDP, 2, P], f8, kind="ExternalInput"
                )
            )
            w2s_d.append(
                nc.dram_tensor(
                    f"w2{j}", [NDP, P, 2, NPR, 2, P], f8, kind="ExternalInput"
                )
            )
        if ty == "b":
            ys_d.append(
                nc.dram_tensor(f"y{j}", [ND, P, S], bf16, kind="ExternalOutput")
            )
        else:
            ys_d.append(
                nc.dram_tensor(
                    f"y{j}", [ND // 4, P, 4, S], bf16, kind="ExternalOutput"
                )
            )

    act_silu = mybir.ActivationFunctionType.Silu
    act_copy = mybir.ActivationFunctionType.Copy

    with tile.TileContext(nc) as tc:
        with (
            tc.tile_pool(name="xp", bufs=1) as xpool,
            tc.tile_pool(name="ap", bufs=1) as apool,
            tc.tile_pool(name="w1p", bufs=3) as w1pool,
            tc.tile_pool(name="w2p", bufs=3) as w2pool,
            tc.tile_pool(name="hp", bufs=3) as hpool,
            tc.tile_pool(name="yp", bufs=3) as ypool,
            tc.tile_pool(name="x8p", bufs=1) as x8pool,
            tc.tile_pool(name="w18p", bufs=4) as w18pool,
            tc.tile_pool(name="w28p", bufs=4) as w28pool,
            tc.tile_pool(name="y4p", bufs=2) as y4pool,
            tc.tile_pool(name="ps", bufs=8, space="PSUM") as psum,
        ):
            # fp8 tiles live in their own pools, declared after the bf16
            # pools so the bf16 phase keeps the measured conflict-free
            # SBUF layout (shifting pool bases cost +35ns/matmul once).
            x_sb = xpool.tile([P, ND, S1b], bf16)
            aT = apool.tile([P, NF, S1b], bf16)
            has_f8 = any(ty == "f" for _, ty in slot_spec)
            if has_f8:
                x8_sb = x8pool.tile([P, ND, S1f], f8)
                aT8 = x8pool.tile([P, NF, S1f], f8)

            x8_hoisted: set = set()
            w18_hoisted: dict = {}
            for j, (S, ty) in enumerate(slot_spec):
                chunks = _chunk_list(S)
                x_d, w1_d, w2_d, y_d = xs_d[j], w1s_d[j], w2s_d[j], ys_d[j]

                if ty == "b":
                    # ---------------- bf16 slot ----------------
                    xsplit = -(-S // 8) * 4
                    startup = (
                        j == 0
                        and len(chunks) >= 3
                        and chunks[1][0] + chunks[1][1] <= xsplit
                    )
                    w1g0 = w1pool.tile([P, 2, ND, P], bf16, tag="w1", name="w1g0")
                    w1u0 = w1pool.tile([P, 2, ND, P], bf16, tag="w1", name="w1u0")
                    if startup:
                        # queue order: 0.5MB of w1 (gate/jj0), x chunk0
                        # dt-major, rest of pair0, x chunk1 — the dt-outer
                        # sub-phases below consume in exactly this order.
                        nc.gpsimd.dma_start(w1g0[:, 0], w1_d[0, 0, :, 0])
                        for dt in range(ND):
                            nc.gpsimd.dma_start(
                                x_sb[:, dt, :xsplit], x_d[dt, :, :xsplit]
                            )
                        nc.gpsimd.dma_start(w1g0[:, 1], w1_d[0, 0, :, 1])
                        nc.gpsimd.dma_start(w1u0[:, 0], w1_d[1, 0, :, 0])
                        nc.gpsimd.dma_start(w1u0[:, 1], w1_d[1, 0, :, 1])
                        for dt in range(ND):
                            nc.gpsimd.dma_start(
                                x_sb[:, dt, xsplit:S], x_d[dt, :, xsplit:]
                            )
                    else:
                        nc.gpsimd.dma_start(w1g0, w1_d[0, 0])
                        nc.gpsimd.dma_start(w1u0, w1_d[1, 0])
                        for dt in range(ND):
                            nc.gpsimd.dma_start(x_sb[:, dt, :S], x_d[dt])

                    def ph1_chain(w1t, jj, t0, csz, start, stop, ps):
                        for dt in range(ND):
                            nc.tensor.matmul(
                                ps[:, :csz],
                                w1t[:, jj, dt, :],
                                x_sb[:, dt, t0 : t0 + csz],
                                start=start and dt == 0,
                                stop=stop and dt == ND - 1,
                            )

                    def ph1_evac(fi, t0, csz, ps_g, ps_u):
                        hg = hpool.tile([P, CHUNK_MAX], bf16, tag="h", name="hg")
                        nc.scalar.activation(hg[:, :csz], ps_g[:, :csz], act_silu)
                        nc.vector.tensor_mul(
                            aT[:, fi, t0 : t0 + csz], hg[:, :csz], ps_u[:, :csz]
                        )

                    pr_start = 0
                    if startup:
                        sc = chunks[:2]
                        banks = {}
                        for gu in range(2):
                            for jj in range(2):
                                for ci in range(len(sc)):
                                    banks[(jj, ci, gu)] = psum.tile(
                                        [P, CHUNK_MAX], f32, tag="ps",
                                        name=f"ps_s{jj}{ci}{gu}",
                                    )
                        for gu, jj in ((0, 0), (0, 1), (1, 0), (1, 1)):
                            w1t = w1g0 if gu == 0 else w1u0
                            for dt in range(ND):
                                for ci, (t0, csz) in enumerate(sc):
                                    nc.tensor.matmul(
                                        banks[(jj, ci, gu)][:, :csz],
                                        w1t[:, jj, dt, :],
                                        x_sb[:, dt, t0 : t0 + csz],
                                        start=(dt == 0),
                                        stop=(dt == ND - 1),
                                    )
                            if gu == 1:
                                for ci, (t0, csz) in enumerate(sc):
                                    ph1_evac(
                                        jj, t0, csz,
                                        banks[(jj, ci, 0)], banks[(jj, ci, 1)],
                                    )
                        for jj in range(2):
                            for t0, csz in chunks[2:]:
                                ps_g = psum.tile(
                                    [P, CHUNK_MAX], f32, tag="ps", name="ps_g"
                                )
                                ph1_chain(w1g0, jj, t0, csz, True, True, ps_g)
                                ps_u = psum.tile(
                                    [P, CHUNK_MAX], f32, tag="ps", name="ps_u"
                                )
                                ph1_chain(w1u0, jj, t0, csz, True, True, ps_u)
                                ph1_evac(jj, t0, csz, ps_g, ps_u)
                        pr_start = 1

                    for pr in range(pr_start, NPR):
                        if pr == 0:
                            w1g, w1u = w1g0, w1u0
                        else:
                            w1g = w1pool.tile([P, 2, ND, P], bf16, tag="w1", name="w1g")
                            nc.gpsimd.dma_start(w1g, w1_d[0, pr])
                            w1u = w1pool.tile([P, 2, ND, P], bf16, tag="w1", name="w1u")
                            nc.gpsimd.dma_start(w1u, w1_d[1, pr])
                        for jj in range(2):
                            for t0, csz in chunks:
                                ps_g = psum.tile(
                                    [P, CHUNK_MAX], f32, tag="ps", name="ps_g"
                                )
                                ph1_chain(w1g, jj, t0, csz, True, True, ps_g)
                                ps_u = psum.tile(
                                    [P, CHUNK_MAX], f32, tag="ps", name="ps_u"
                                )
                                ph1_chain(w1u, jj, t0, csz, True, True, ps_u)
                                ph1_evac(2 * pr + jj, t0, csz, ps_g, ps_u)

                    for dp in range(NDP):
                        w2sb = w2pool.tile([P, 2, NF, P], bf16, tag="w2", name="w2sb")
                        nc.gpsimd.dma_start(w2sb, w2_d[dp])
                        for dj in range(2):
                            dt = 2 * dp + dj
                            y_st = ypool.tile([P, S1b], bf16, tag="y", name="y_st")
                            for t0, csz in chunks:
                                ps_y = psum.tile(
                                    [P, CHUNK_MAX], f32, tag="ps", name="ps_y"
                                )
                                for fi in range(NF):
                                    nc.tensor.matmul(
                                        ps_y[:, :csz],
                                        w2sb[:, dj, fi, :],
                                        aT[:, fi, t0 : t0 + csz],
                                        start=(fi == 0),
                                        stop=(fi == NF - 1),
                                    )
                                nc.scalar.activation(
                                    y_st[:, t0 : t0 + csz], ps_y[:, :csz], act_copy
                                )
                            (nc.sync if dt % 2 == 0 else nc.gpsimd).dma_start(
                                y_d[dt], y_st[:, :S]
                            )
                    # prefetch the next (fp8) slot's x8 + first w18 pair ahead
                    # of this ph3's throttled y DMAs so they aren't
                    # head-blocked on the SWDGE queue.
                    if j + 1 < len(slot_spec) and slot_spec[j + 1][1] == "f":
                        Sn = slot_spec[j + 1][0]
                        for dt in range(ND):
                            nc.gpsimd.dma_start(
                                x8_sb[:, dt, :Sn], xs_d[j + 1][dt]
                            )
                        x8_hoisted.add(j + 1)
                        hg0 = w18pool.tile(
                            [P, 2, NDP, 2, P], f8, tag="w18", name="w18hg"
                        )
                        nc.gpsimd.dma_start(hg0, w1s_d[j + 1][0, 0])
                        hu0 = w18pool.tile(
                            [P, 2, NDP, 2, P], f8, tag="w18", name="w18hu"
                        )
                        nc.gpsimd.dma_start(hu0, w1s_d[j + 1][1, 0])
                        hg1 = w18pool.tile(
                            [P, 2, NDP, 2, P], f8, tag="w18", name="w18hg1"
                        )
                        nc.gpsimd.dma_start(hg1, w1s_d[j + 1][0, 1])
                        hu1 = w18pool.tile(
                            [P, 2, NDP, 2, P], f8, tag="w18", name="w18hu1"
                        )
                        nc.gpsimd.dma_start(hu1, w1s_d[j + 1][1, 1])
                        w18_hoisted[j + 1] = ((hg0, hu0), (hg1, hu1))
                else:
                    # ---------------- fp8 slot (DoubleRow) ----------------
                    if j not in x8_hoisted:
                        for dt in range(ND):
                            nc.gpsimd.dma_start(x8_sb[:, dt, :S], x_d[dt])

                    for pr in range(NPR):
                        if pr < 2 and j in w18_hoisted:
                            w1g, w1u = w18_hoisted[j][pr]
                        else:
                            w1g = w18pool.tile(
                                [P, 2, NDP, 2, P], f8, tag="w18", name="w18g"
                            )
                            nc.gpsimd.dma_start(w1g, w1_d[0, pr])
                            w1u = w18pool.tile(
                                [P, 2, NDP, 2, P], f8, tag="w18", name="w18u"
                            )
                            nc.gpsimd.dma_start(w1u, w1_d[1, pr])
                        for jj in range(2):
                            fi = 2 * pr + jj
                            for t0, csz in chunks:
                                ps_g = psum.tile(
                                    [P, CHUNK_MAX], f32, tag="ps", name="ps_g"
                                )
                                for dp in range(NDP):
                                    nc.tensor.matmul(
                                        ps_g[:, :csz],
                                        w1g[:, jj, dp],
                                        x8_sb[:, 2 * dp : 2 * dp + 2, t0 : t0 + csz],
                                        start=(dp == 0),
                                        stop=(dp == NDP - 1),
                                        perf_mode=DR,
                                    )
                                ps_u = psum.tile(
                                    [P, CHUNK_MAX], f32, tag="ps", name="ps_u"
                                )
                                for dp in range(NDP):
                                    nc.tensor.matmul(
                                        ps_u[:, :csz],
                                        w1u[:, jj, dp],
                                        x8_sb[:, 2 * dp : 2 * dp + 2, t0 : t0 + csz],
                                        start=(dp == 0),
                                        stop=(dp == NDP - 1),
                                        perf_mode=DR,
                                    )
                                hg = hpool.tile([P, CHUNK_MAX], bf16, tag="h", name="hg")
                                nc.scalar.activation(
                                    hg[:, :csz], ps_g[:, :csz], act_silu,
                                    scale=1.0 / (X8S * W1S),
                                )
                                hu = hpool.tile([P, CHUNK_MAX], bf16, tag="h", name="hu")
                                nc.scalar.activation(
                                    hu[:, :csz], ps_u[:, :csz], act_copy,
                                    scale=X8S / (X8S * W1S),
                                )
                                nc.vector.tensor_mul(
                                    aT8[:, fi, t0 : t0 + csz], hg[:, :csz], hu[:, :csz]
                                )

                    y4 = None
                    for dp in range(NDP):
                        w2sb = w28pool.tile(
                            [P, 2, NPR, 2, P], f8, tag="w28", name="w28sb"
                        )
                        nc.gpsimd.dma_start(w2sb, w2_d[dp])
                        for dj in range(2):
                            dt = 2 * dp + dj
                            if dt % 4 == 0:
                                y4 = y4pool.tile(
                                    [P, 4, S1f], bf16, tag="y4", name="y4"
                                )
                            for t0, csz in chunks:
                                ps_y = psum.tile(
                                    [P, CHUNK_MAX], f32, tag="ps", name="ps_y"
                                )
                                for q in range(NPR):
                                    nc.tensor.matmul(
                                        ps_y[:, :csz],
                                        w2sb[:, dj, q],
                                        aT8[:, 2 * q : 2 * q + 2, t0 : t0 + csz],
                                        start=(q == 0),
                                        stop=(q == NPR - 1),
                                        perf_mode=DR,
                                    )
                                nc.scalar.activation(
                                    y4[:, dt % 4, t0 : t0 + csz],
                                    ps_y[:, :csz],
                                    act_copy,
                                )
                            if dt % 4 == 3:
                                nc.gpsimd.dma_start(
                                    y_d[dt // 4], y4[:, :, :S]
                                )

    _legalize_sync(nc)
    return nc


def _ensure_ntff_hook():
    """Register the axon NTFF-profile hook if the image's antenv lacks
    ``axon_hooks`` (the hook impl ships in trn_agent_boot). Best-effort."""
    import sys
    import types

    try:
        from antenv.axon_hooks import get_axon_ntff_profile_hook  # noqa: F401

        return
    except ImportError:
        pass
    try:
        import antenv

        mod = types.ModuleType("antenv.axon_hooks")
        mod._hook = None

        def set_axon_ntff_profile_hook(h):
            mod._hook = h

        def get_axon_ntff_profile_hook():
            return mod._hook

        mod.set_axon_ntff_profile_hook = set_axon_ntff_profile_hook
        mod.get_axon_ntff_profile_hook = get_axon_ntff_profile_hook
        sys.modules["antenv.axon_hooks"] = mod
        antenv.axon_hooks = mod

        from trn_agent_boot.trn_boot import _ntff_profile_via_ctypes

        so_path = "/opt/axon/libaxon_pjrt.so"
        hook = _ntff_profile_via_ctypes(so_path)
        if hook is not None:
            mod._hook = hook
    except Exception:
        pass


def _route(x, gate_w, top_k):
    """Replicates the reference router in numpy fp32 (renormalized top-k
    softmax == softmax over the top-k logits)."""
    logits = x.astype(np.float32) @ gate_w.astype(np.float32).T  # [T, E]
    k = int(top_k)
    idx = np.argpartition(-logits, k - 1, axis=1)[:, :k]
    lv = np.take_along_axis(logits, idx, axis=1)
    m = lv.max(axis=1, keepdims=True)
    ew = np.exp(lv - m)
    wts = ew / ew.sum(axis=1, keepdims=True)
    return idx, wts.astype(np.float32)


def _make_slots(hi_counts, lo_counts):
    """Cut experts into <=SLOT_CAP segments per precision; LPT-pack onto 4
    core pairs (fp8 cost 0.5/col); return per-pair slot lists
    [(expert, lo, hi, ty), ...] (bf16 slots first) and the shared slot-spec
    tuple ((S, ty), ...) with S = max across pairs, padded to mult of 4."""
    segs = []
    for e, c in enumerate(hi_counts):
        t = 0
        while t < c:
            s = min(SLOT_CAP, c - t)
            segs.append((s, 1.0, e, t, t + s, "b"))
            t += s
    for e, c in enumerate(lo_counts):
        base = hi_counts[e]
        t = 0
        while t < c:
            s = min(SLOT_CAP, c - t)
            segs.append((s, 0.5, e, base + t, base + t + s, "f"))
            t += s
    # bf16 first (descending), then fp8 (descending), onto least-loaded pair
    segs.sort(key=lambda g: (g[5], -g[0]))
    loads = [0.0] * 4
    pair_slots = [[] for _ in range(4)]
    for s, cost, e, lo, hi, ty in segs:
        p = min(range(4), key=lambda i: loads[i])
        loads[p] += s * cost
        pair_slots[p].append((e, lo, hi, ty))
    for sl in pair_slots:
        sl.sort(key=lambda t: (t[3], -(t[2] - t[1])))
    kb = max(sum(1 for t in sl if t[3] == "b") for sl in pair_slots)
    kf = max(sum(1 for t in sl if t[3] == "f") for sl in pair_slots)
    # normalize: every pair gets kb bf slots then kf f8 slots (dummies empty)
    spec = []
    norm = [[] for _ in range(4)]
    for ji in range(kb + kf):
        ty = "b" if ji < kb else "f"
        m = 8
        for p in range(4):
            mine = [t for t in pair_slots[p] if t[3] == ty]
            i = ji if ty == "b" else ji - kb
            if i < len(mine):
                norm[p].append(mine[i])
                m = max(m, mine[i][2] - mine[i][1])
            else:
                norm[p].append(None)
        spec.append((-(-m // 4) * 4, ty))
    return norm, tuple(spec)


def kernel(x, gate_w, wv1, w2, top_k):
    import ml_dtypes

    from concourse.bass_utils import run_bass_kernel_spmd

    x = np.asarray(x)
    gate_w = np.asarray(gate_w)
    wv1 = np.asarray(wv1)
    w2 = np.asarray(w2)

    T, D = x.shape
    E, F2, _ = wv1.shape
    F = F2 // 2
    Fh = F // 2
    ND = D // P
    NF = Fh // P
    NPR = NF // 2
    NDP = ND // 2
    n_cores = 8

    idx, wts = _route(x, gate_w, top_k)

    rows_l, w_l, hi_n = [], [], []
    for e in range(E):
        rows, cols = np.nonzero(idx == e)
        w_e = wts[rows, cols]
        order = np.argsort(-w_e, kind="stable")
        rows_l.append(rows[order])
        w_l.append(w_e[order])
        hi_n.append(int((w_e >= THETA).sum()))
    counts = [len(r) for r in rows_l]
    lo_n = [c - h for c, h in zip(counts, hi_n)]

    pair_slots, slot_spec = _make_slots(hi_n, lo_n)

    key = (slot_spec, D, Fh)
    if key not in _BASS_CACHE:
        _BASS_CACHE[key] = _build_bass(slot_spec, D, Fh)
    nc = _BASS_CACHE[key]

    bf16 = ml_dtypes.bfloat16
    f8 = ml_dtypes.float8_e4m3
    x_bf = x.astype(bf16)
    x_f8 = np.clip(x * X8S, -240, 240).astype(f8)
    w1_bf = wv1.astype(bf16)
    w2_bf = w2.astype(bf16)

    w1_cache: dict = {}
    w2_cache: dict = {}

    def w1_pack(e, h, ty):
        if (e, h, ty) not in w1_cache:
            if ty == "b":
                gsl = w1_bf[e][h * Fh : (h + 1) * Fh]
                usl = w1_bf[e][F + h * Fh : F + (h + 1) * Fh]
                both = np.stack([gsl, usl])  # [2, Fh, D]
                w1_cache[(e, h, ty)] = np.ascontiguousarray(
                    both.reshape(2, NPR, 2, P, ND, P).transpose(0, 1, 5, 2, 4, 3)
                )
            else:
                gsl = wv1[e][h * Fh : (h + 1) * Fh]
                usl = wv1[e][F + h * Fh : F + (h + 1) * Fh]
                both = np.clip(np.stack([gsl, usl]) * W1S, -240, 240).astype(f8)
                w1_cache[(e, h, ty)] = np.ascontiguousarray(
                    both.reshape(2, NPR, 2, P, NDP, 2, P).transpose(0, 1, 6, 2, 4, 5, 3)
                )
        return w1_cache[(e, h, ty)]

    def w2_pack(e, h, ty):
        if (e, h, ty) not in w2_cache:
            if ty == "b":
                sl = w2_bf[e][:, h * Fh : (h + 1) * Fh]  # [D, Fh]
                w2_cache[(e, h, ty)] = np.ascontiguousarray(
                    sl.reshape(NDP, 2, P, NF, P).transpose(0, 4, 1, 3, 2)
                )
            else:
                sl = np.clip(w2[e][:, h * Fh : (h + 1) * Fh] * W2S, -240, 240).astype(f8)
                w2_cache[(e, h, ty)] = np.ascontiguousarray(
                    sl.reshape(NDP, 2, P, NPR, 2, P).transpose(0, 5, 1, 3, 4, 2)
                )
        return w2_cache[(e, h, ty)]

    in_maps = []
    for p in range(4):
        slots = pair_slots[p]
        for h in range(2):
            im = {}
            for ji, (S, ty) in enumerate(slot_spec):
                slot = slots[ji]
                if ty == "b":
                    xq, w1z, w2z = (
                        np.zeros((D, S), dtype=bf16),
                        np.zeros((2, NPR, P, 2, ND, P), dtype=bf16),
                        np.zeros((NDP, P, 2, NF, P), dtype=bf16),
                    )
                else:
                    xq, w1z, w2z = (
                        np.zeros((D, S), dtype=f8),
                        np.zeros((2, NPR, P, 2, NDP, 2, P), dtype=f8),
                        np.zeros((NDP, P, 2, NPR, 2, P), dtype=f8),
                    )
                if slot is not None:
                    e, lo, hi, _ = slot
                    seg = rows_l[e][lo:hi]
                    src = x_bf if ty == "b" else x_f8
                    xq[:, : hi - lo] = src[seg].T
                    w1z = w1_pack(e, h, ty)
                    w2z = w2_pack(e, h, ty)
                im[f"x{ji}"] = np.ascontiguousarray(xq.reshape(ND, P, S))
                im[f"w1{ji}"] = w1z
                im[f"w2{ji}"] = w2z
            in_maps.append(im)

    _ensure_ntff_hook()
    res = run_bass_kernel_spmd(nc, in_maps, core_ids=list(range(n_cores)))
    global last_run
    last_run = res

    out = np.zeros((T, D), dtype=np.float32)
    for p in range(4):
        for ji, (S, ty) in enumerate(slot_spec):
            slot = pair_slots[p][ji]
            if slot is None:
                continue
            e, lo, hi, _ = slot
            n = hi - lo
            if ty == "b":
                y0 = res.results[2 * p][f"y{ji}"].reshape(D, -1)[:, :n]
                y1 = res.results[2 * p + 1][f"y{ji}"].reshape(D, -1)[:, :n]
            else:
                y0 = (
                    res.results[2 * p][f"y{ji}"]
                    .transpose(0, 2, 1, 3)
                    .reshape(D, -1)[:, :n]
                )
                y1 = (
                    res.results[2 * p + 1][f"y{ji}"]
                    .transpose(0, 2, 1, 3)
                    .reshape(D, -1)[:, :n]
                )
            ysum = y0.astype(np.float32) + y1.astype(np.float32)
            if ty == "f":
                ysum *= 1.0 / Y8S
            seg = rows_l[e][lo:hi]
            out[seg] += w_l[e][lo:hi, None] * ysum.T
    return out.astype(x.dtype, copy=False)


# revision 16
# speedup vs baseline: 1.3226x; 1.0033x over previous
"""Block-sparse MoE (softmax top-k routing + silu-gated FFN) on 8 Trainium2 cores.

Sharding: expert-pair x FFN-half. The router runs on host; each expert's
token list is sorted by router weight and split at THETA: high-weight
pairs run in bf16, low-weight pairs in fp8 e4m3 (DoubleRow, 2x tensor
rate) — the output error each fp8 token contributes is proportional to
its (small) router weight, keeping total rel err ~1.2e-2 vs the 2e-2
gate. Segments are LPT-packed onto 4 core-pairs (fp8 cost 0.5/col) and
each pair of cores splits the FFN dim in half (1792 of 3584), so all 8
cores run the same slot-size program (SPMD) with balanced cycles.

Per slot on a core (S tokens, ND=16 d-tiles, NFh=14 f-tiles):
  phase1: h.T [128 f, chunk] = w1h.T @ x (contract D); silu(g)*u -> aT
  phase3: y[d, tok] = sum_fi w2h[f,d].T @ aT[:, fi, :] (contract F/2)
          d on PSUM partitions, tokens moving => exact columns, no pad.
  fp8 slots: x*16, w1*32 -> psum = 512*h; silu via ACT scale 1/512;
  u16 = psum/32; a8 = e4m3(silu*u16) = 16a; w2*64 -> y*1024 (host /1024).

Moving chunks are equalized (~420-510) so LDWEIGHTS (~97-116ns) hides
under the moving phase. Slot0's x arrives dt-major in two column chunks
consumed dt-outer across 8 PSUM banks (PE starts ~12us in). w1 prefetch
depth 4 covers the ~6us pair-tile transfer. All bulk traffic rides the
SWDGE queue with >=2KB lines; later slots' x and w1 overlap prior ph3.
Host sums the two F-halves, applies router weight, scatter-adds (f32).
"""

import numpy as np


def _ensure_concourse_on_path():
    try:
        import concourse  # noqa: F401
    except ImportError:
        import sys

        for p in ("/opt/trn_rl_repo", "/root/.axon_site/_ro/trn_rl_repo"):
            if p not in sys.path:
                sys.path.insert(0, p)


_ensure_concourse_on_path()

P = 128
CHUNK_MAX = 512   # PSUM bank free-dim limit (f32)
SLOT_CAP = 2560   # max tokens per slot (SBUF-bound)
THETA = 0.37      # router-weight cutoff: below -> fp8 path
X8S, W1S, W2S = 16.0, 32.0, 64.0
Y8S = X8S * W2S   # fp8-slot output scale (host divides)

_BASS_CACHE: dict = {}
last_run = None  # BassKernelResults of the most recent kernel() call (for test.py)


def _legalize_sync(nc, max_waits: int = 1):
    """Split multi-wait sync_info into preceding EventSemaphore instructions.

    The walrus build in this container lowers every instruction with capacity
    for a single sync-wait command and errors with "Too many sync wait
    commands" otherwise, while Tile attaches up to 3 waits per instruction.
    A wait carried by an EventSemaphore on the same engine immediately before
    the instruction is semantically identical. For DMAs, keep the own-lane
    FIFO wait on the instruction itself so the in-queue wait doesn't stall
    the sequencer.
    """
    import concourse.mybir as mybir

    fn = nc.m.functions[0]
    for blk in fn.blocks:
        new_insts = []
        for inst in blk.instructions:
            si = inst.sync_info
            if si is not None and si.on_wait is not None and len(si.on_wait) > max_waits:
                ow = list(si.on_wait)
                upd_ids = {u.id for u in (si.on_update or [])}
                keep = [w for w in ow if w.id in upd_ids][:1]
                if not keep:
                    keep = [ow[-1]]
                for j, w in enumerate(ow):
                    if w is keep[0]:
                        continue
                    new_insts.append(
                        mybir.InstEventSemaphore(
                            name=f"{inst.name}-ws{j}",
                            opcode="EventSemaphore",
                            engine=inst.engine,
                            sync_info=mybir.SyncInfo(on_wait=[w], on_update=[]),
                        )
                    )
                si.on_wait = keep
            new_insts.append(inst)
        blk.instructions = new_insts


def _chunk_list(S: int):
    """Equalized moving chunks: k = ceil(S/512), base = ceil(S/k) rounded to
    a multiple of 4, so every chunk is large enough that LDWEIGHTS hides
    under the moving phase."""
    k = -(-S // CHUNK_MAX)
    base = -(-S // k)
    base = -(-base // 4) * 4
    chunks = []
    t0 = 0
    while t0 < S:
        c = min(base, S - t0)
        chunks.append((t0, c))
        t0 += c
    return chunks


def _build_bass(slot_spec: tuple, D: int, Fh: int):
    """Bass program: sequence of expert slots, each (S tokens, ty) with
    ty 'b' (bf16) or 'f' (fp8 e4m3 DoubleRow). Per-slot inputs x{j}, w1{j},
    w2{j}; outputs y{j}."""
    import concourse.bass as bass
    import concourse.mybir as mybir
    import concourse.tile as tile

    ND = D // P           # 16 contraction tiles (phase 1)
    NF = Fh // P          # 14 f-tiles per half
    NPR = NF // 2         # 7 w1 pairs
    NDP = ND // 2         # 8 dt-pair groups
    assert NF % 2 == 0 and ND % 2 == 0
    S1b = max((S for S, ty in slot_spec if ty == "b"), default=8)
    S1f = max((S for S, ty in slot_spec if ty == "f"), default=8)

    bf16 = mybir.dt.bfloat16
    f8 = mybir.dt.float8e4
    f32 = mybir.dt.float32
    DR = mybir.MatmulPerfMode.DoubleRow

    nc = bass.Bass(name="moe_pair_ffn", num_swdge_queues=4)
    xs_d, w1s_d, w2s_d, ys_d = [], [], [], []
    for j, (S, ty) in enumerate(slot_spec):
        if ty == "b":
            xs_d.append(nc.dram_tensor(f"x{j}", [ND, P, S], bf16, kind="ExternalInput"))
            w1s_d.append(
                nc.dram_tensor(
                    f"w1{j}", [2, NPR, P, 2, ND, P], bf16, kind="ExternalInput"
                )
            )
            w2s_d.append(
                nc.dram_tensor(f"w2{j}", [NDP, P, 2, NF, P], bf16, kind="ExternalInput")
            )
        else:
            xs_d.append(nc.dram_tensor(f"x{j}", [ND, P, S], f8, kind="ExternalInput"))
            w1s_d.append(
                nc.dram_tensor(
                    f"w1{j}", [2, NPR, P, 2, NDP, 2, P], f8, kind="ExternalInput"
                )
            )
            w2s_d.append(
                nc.dram_tensor(
                    f"w2{j}", [NDP, P, 2, NPR, 2, P], f8, kind="ExternalInput"
                )
            )
        if ty == "b":
            ys_d.append(
                nc.dram_tensor(f"y{j}", [ND, P, S], bf16, kind="ExternalOutput")
            )
        else:
            ys_d.append(
                nc.dram_tensor(
                    f"y{j}", [ND // 4, P, 4, S], bf16, kind="ExternalOutput"
                )
            )

    act_silu = mybir.ActivationFunctionType.Silu
    act_copy = mybir.ActivationFunctionType.Copy

    with tile.TileContext(nc) as tc:
        with (
            tc.tile_pool(name="xp", bufs=1) as xpool,
            tc.tile_pool(name="ap", bufs=1) as apool,
            tc.tile_pool(name="w1p", bufs=3) as w1pool,
            tc.tile_pool(name="w2p", bufs=3) as w2pool,
            tc.tile_pool(name="hp", bufs=3) as hpool,
            tc.tile_pool(name="yp", bufs=3) as ypool,
            tc.tile_pool(name="x8p", bufs=1) as x8pool,
            tc.tile_pool(name="w18p", bufs=4) as w18pool,
            tc.tile_pool(name="w28p", bufs=4) as w28pool,
            tc.tile_pool(name="y4p", bufs=2) as y4pool,
            tc.tile_pool(name="ps", bufs=8, space="PSUM") as psum,
        ):
            # fp8 tiles live in their own pools, declared after the bf16
            # pools so the bf16 phase keeps the measured conflict-free
            # SBUF layout (shifting pool bases cost +35ns/matmul once).
            x_sb = xpool.tile([P, ND, S1b], bf16)
            aT = apool.tile([P, NF, S1b], bf16)
            has_f8 = any(ty == "f" for _, ty in slot_spec)
            if has_f8:
                x8_sb = x8pool.tile([P, ND, S1f], f8)
                aT8 = x8pool.tile([P, NF, S1f], f8)

            x8_hoisted: set = set()
            w18_hoisted: dict = {}
            for j, (S, ty) in enumerate(slot_spec):
                chunks = _chunk_list(S)
                x_d, w1_d, w2_d, y_d = xs_d[j], w1s_d[j], w2s_d[j], ys_d[j]

                if ty == "b":
                    # ---------------- bf16 slot ----------------
                    w1_pre: dict = {}
                    xsplit = -(-S // 8) * 4
                    startup = (
                        j == 0
                        and len(chunks) >= 3
                        and chunks[1][0] + chunks[1][1] <= xsplit
                    )
                    w1g0 = w1pool.tile([P, 2, ND, P], bf16, tag="w1", name="w1g0")
                    w1u0 = w1pool.tile([P, 2, ND, P], bf16, tag="w1", name="w1u0")
                    if startup:
                        # queue order: 0.5MB of w1 (gate/jj0), x chunk0
                        # dt-major, rest of pair0, x chunk1 — the dt-outer
                        # sub-phases below consume in exactly this order.
                        nc.gpsimd.dma_start(w1g0[:, 0], w1_d[0, 0, :, 0])
                        for dt in range(ND):
                            nc.gpsimd.dma_start(
                                x_sb[:, dt, :xsplit], x_d[dt, :, :xsplit]
                            )
                        nc.gpsimd.dma_start(w1g0[:, 1], w1_d[0, 0, :, 1])
                        nc.gpsimd.dma_start(w1u0[:, 0], w1_d[1, 0, :, 0])
                        nc.gpsimd.dma_start(w1u0[:, 1], w1_d[1, 0, :, 1])
                        p1g = w1pool.tile([P, 2, ND, P], bf16, tag="w1", name="p1g")
                        nc.gpsimd.dma_start(p1g, w1_d[0, 1])
                        for dt in range(ND):
                            nc.gpsimd.dma_start(
                                x_sb[:, dt, xsplit:S], x_d[dt, :, xsplit:]
                            )
                        # 4th alloc in a 3-buf pool: its slot wait blocks the
                        # gpsimd engine until pair0 is consumed, so it must
                        # sit AFTER the x chunk1 DMAs in program order.
                        p1u = w1pool.tile([P, 2, ND, P], bf16, tag="w1", name="p1u")
                        nc.gpsimd.dma_start(p1u, w1_d[1, 1])
                        w1_pre[1] = (p1g, p1u)
                    else:
                        nc.gpsimd.dma_start(w1g0, w1_d[0, 0])
                        nc.gpsimd.dma_start(w1u0, w1_d[1, 0])
                        for dt in range(ND):
                            nc.gpsimd.dma_start(x_sb[:, dt, :S], x_d[dt])

                    def ph1_chain(w1t, jj, t0, csz, start, stop, ps):
                        for dt in range(ND):
                            nc.tensor.matmul(
                                ps[:, :csz],
                                w1t[:, jj, dt, :],
                                x_sb[:, dt, t0 : t0 + csz],
                                start=start and dt == 0,
                                stop=stop and dt == ND - 1,
                            )

                    def ph1_evac(fi, t0, csz, ps_g, ps_u):
                        hg = hpool.tile([P, CHUNK_MAX], bf16, tag="h", name="hg")
                        nc.scalar.activation(hg[:, :csz], ps_g[:, :csz], act_silu)
                        nc.vector.tensor_mul(
                            aT[:, fi, t0 : t0 + csz], hg[:, :csz], ps_u[:, :csz]
                        )

                    pr_start = 0
                    if startup:
                        sc = chunks[:2]
                        banks = {}
                        for gu in range(2):
                            for jj in range(2):
                                for ci in range(len(sc)):
                                    banks[(jj, ci, gu)] = psum.tile(
                                        [P, CHUNK_MAX], f32, tag="ps",
                                        name=f"ps_s{jj}{ci}{gu}",
                                    )
                        for gu, jj in ((0, 0), (0, 1), (1, 0), (1, 1)):
                            w1t = w1g0 if gu == 0 else w1u0
                            for dt in range(ND):
                                for ci, (t0, csz) in enumerate(sc):
                                    nc.tensor.matmul(
                                        banks[(jj, ci, gu)][:, :csz],
                                        w1t[:, jj, dt, :],
                                        x_sb[:, dt, t0 : t0 + csz],
                                        start=(dt == 0),
                                        stop=(dt == ND - 1),
                                    )
                            if gu == 1:
                                for ci, (t0, csz) in enumerate(sc):
                                    ph1_evac(
                                        jj, t0, csz,
                                        banks[(jj, ci, 0)], banks[(jj, ci, 1)],
                                    )
                        for jj in range(2):
                            for t0, csz in chunks[2:]:
                                ps_g = psum.tile(
                                    [P, CHUNK_MAX], f32, tag="ps", name="ps_g"
                                )
                                ph1_chain(w1g0, jj, t0, csz, True, True, ps_g)
                                ps_u = psum.tile(
                                    [P, CHUNK_MAX], f32, tag="ps", name="ps_u"
                                )
                                ph1_chain(w1u0, jj, t0, csz, True, True, ps_u)
                                ph1_evac(jj, t0, csz, ps_g, ps_u)
                        pr_start = 1

                    for pr in range(pr_start, NPR):
                        if pr == 0:
                            w1g, w1u = w1g0, w1u0
                        elif pr in w1_pre:
                            w1g, w1u = w1_pre.pop(pr)
                        else:
                            w1g = w1pool.tile([P, 2, ND, P], bf16, tag="w1", name="w1g")
                            for jj in range(2):
                                nc.gpsimd.dma_start(w1g[:, jj], w1_d[0, pr, :, jj])
                            w1u = w1pool.tile([P, 2, ND, P], bf16, tag="w1", name="w1u")
                            for jj in range(2):
                                nc.gpsimd.dma_start(w1u[:, jj], w1_d[1, pr, :, jj])
                        for jj in range(2):
                            for t0, csz in chunks:
                                ps_g = psum.tile(
                                    [P, CHUNK_MAX], f32, tag="ps", name="ps_g"
                                )
                                ph1_chain(w1g, jj, t0, csz, True, True, ps_g)
                                ps_u = psum.tile(
                                    [P, CHUNK_MAX], f32, tag="ps", name="ps_u"
                                )
                                ph1_chain(w1u, jj, t0, csz, True, True, ps_u)
                                ph1_evac(2 * pr + jj, t0, csz, ps_g, ps_u)

                    for dp in range(NDP):
                        w2sb = w2pool.tile([P, 2, NF, P], bf16, tag="w2", name="w2sb")
                        nc.gpsimd.dma_start(w2sb, w2_d[dp])
                        for dj in range(2):
                            dt = 2 * dp + dj
                            y_st = ypool.tile([P, S1b], bf16, tag="y", name="y_st")
                            for t0, csz in chunks:
                                ps_y = psum.tile(
                                    [P, CHUNK_MAX], f32, tag="ps", name="ps_y"
                                )
                                for fi in range(NF):
                                    nc.tensor.matmul(
                                        ps_y[:, :csz],
                                        w2sb[:, dj, fi, :],
                                        aT[:, fi, t0 : t0 + csz],
                                        start=(fi == 0),
                                        stop=(fi == NF - 1),
                                    )
                                nc.scalar.activation(
                                    y_st[:, t0 : t0 + csz], ps_y[:, :csz], act_copy
                                )
                            (nc.sync if dt % 2 == 0 else nc.gpsimd).dma_start(
                                y_d[dt], y_st[:, :S]
                            )
                    # prefetch the next (fp8) slot's x8 + first w18 pair ahead
                    # of this ph3's throttled y DMAs so they aren't
                    # head-blocked on the SWDGE queue.
                    if j + 1 < len(slot_spec) and slot_spec[j + 1][1] == "f":
                        Sn = slot_spec[j + 1][0]
                        for dt in range(ND):
                            nc.gpsimd.dma_start(
                                x8_sb[:, dt, :Sn], xs_d[j + 1][dt]
                            )
                        x8_hoisted.add(j + 1)
                        hg0 = w18pool.tile(
                            [P, 2, NDP, 2, P], f8, tag="w18", name="w18hg"
                        )
                        nc.gpsimd.dma_start(hg0, w1s_d[j + 1][0, 0])
                        hu0 = w18pool.tile(
                            [P, 2, NDP, 2, P], f8, tag="w18", name="w18hu"
                        )
                        nc.gpsimd.dma_start(hu0, w1s_d[j + 1][1, 0])
                        hg1 = w18pool.tile(
                            [P, 2, NDP, 2, P], f8, tag="w18", name="w18hg1"
                        )
                        nc.gpsimd.dma_start(hg1, w1s_d[j + 1][0, 1])
                        hu1 = w18pool.tile(
                            [P, 2, NDP, 2, P], f8, tag="w18", name="w18hu1"
                        )
                        nc.gpsimd.dma_start(hu1, w1s_d[j + 1][1, 1])
                        w18_hoisted[j + 1] = ((hg0, hu0), (hg1, hu1))
                else:
                    # ---------------- fp8 slot (DoubleRow) ----------------
                    if j not in x8_hoisted:
                        for dt in range(ND):
                            nc.gpsimd.dma_start(x8_sb[:, dt, :S], x_d[dt])

                    for pr in range(NPR):
                        if pr < 2 and j in w18_hoisted:
                            w1g, w1u = w18_hoisted[j][pr]
                        else:
                            w1g = w18pool.tile(
                                [P, 2, NDP, 2, P], f8, tag="w18", name="w18g"
                            )
                            for jj in range(2):
                                nc.gpsimd.dma_start(w1g[:, jj], w1_d[0, pr, :, jj])
                            w1u = w18pool.tile(
                                [P, 2, NDP, 2, P], f8, tag="w18", name="w18u"
                            )
                            for jj in range(2):
                                nc.gpsimd.dma_start(w1u[:, jj], w1_d[1, pr, :, jj])
                        for jj in range(2):
                            fi = 2 * pr + jj
                            for t0, csz in chunks:
                                ps_g = psum.tile(
                                    [P, CHUNK_MAX], f32, tag="ps", name="ps_g"
                                )
                                for dp in range(NDP):
                                    nc.tensor.matmul(
                                        ps_g[:, :csz],
                                        w1g[:, jj, dp],
                                        x8_sb[:, 2 * dp : 2 * dp + 2, t0 : t0 + csz],
                                        start=(dp == 0),
                                        stop=(dp == NDP - 1),
                                        perf_mode=DR,
                                    )
                                ps_u = psum.tile(
                                    [P, CHUNK_MAX], f32, tag="ps", name="ps_u"
                                )
                                for dp in range(NDP):
                                    nc.tensor.matmul(
                                        ps_u[:, :csz],
                                        w1u[:, jj, dp],
                                        x8_sb[:, 2 * dp : 2 * dp + 2, t0 : t0 + csz],
                                        start=(dp == 0),
                                        stop=(dp == NDP - 1),
                                        perf_mode=DR,
                                    )
                                hg = hpool.tile([P, CHUNK_MAX], bf16, tag="h", name="hg")
                                nc.scalar.activation(
                                    hg[:, :csz], ps_g[:, :csz], act_silu,
                                    scale=1.0 / (X8S * W1S),
                                )
                                hu = hpool.tile([P, CHUNK_MAX], bf16, tag="h", name="hu")
                                nc.scalar.activation(
                                    hu[:, :csz], ps_u[:, :csz], act_copy,
                                    scale=X8S / (X8S * W1S),
                                )
                                nc.vector.tensor_mul(
                                    aT8[:, fi, t0 : t0 + csz], hg[:, :csz], hu[:, :csz]
                                )

                    y4 = None
                    for dp in range(NDP):
                        w2sb = w28pool.tile(
                            [P, 2, NPR, 2, P], f8, tag="w28", name="w28sb"
                        )
                        nc.gpsimd.dma_start(w2sb, w2_d[dp])
                        for dj in range(2):
                            dt = 2 * dp + dj
                            if dt % 4 == 0:
                                y4 = y4pool.tile(
                                    [P, 4, S1f], bf16, tag="y4", name="y4"
                                )
                            for t0, csz in chunks:
                                ps_y = psum.tile(
                                    [P, CHUNK_MAX], f32, tag="ps", name="ps_y"
                                )
                                for q in range(NPR):
                                    nc.tensor.matmul(
                                        ps_y[:, :csz],
                                        w2sb[:, dj, q],
                                        aT8[:, 2 * q : 2 * q + 2, t0 : t0 + csz],
                                        start=(q == 0),
                                        stop=(q == NPR - 1),
                                        perf_mode=DR,
                                    )
                                nc.scalar.activation(
                                    y4[:, dt % 4, t0 : t0 + csz],
                                    ps_y[:, :csz],
                                    act_copy,
                                )
                            if dt % 4 == 3:
                                nc.gpsimd.dma_start(
                                    y_d[dt // 4], y4[:, :, :S]
                                )

    _legalize_sync(nc)
    return nc


def _ensure_ntff_hook():
    """Register the axon NTFF-profile hook if the image's antenv lacks
    ``axon_hooks`` (the hook impl ships in trn_agent_boot). Best-effort."""
    import sys
    import types

    try:
        from antenv.axon_hooks import get_axon_ntff_profile_hook  # noqa: F401

        return
    except ImportError:
        pass
    try:
        import antenv

        mod = types.ModuleType("antenv.axon_hooks")
        mod._hook = None

        def set_axon_ntff_profile_hook(h):
            mod._hook = h

        def get_axon_ntff_profile_hook():
            return mod._hook

        mod.set_axon_ntff_profile_hook = set_axon_ntff_profile_hook
        mod.get_axon_ntff_profile_hook = get_axon_ntff_profile_hook
        sys.modules["antenv.axon_hooks"] = mod
        antenv.axon_hooks = mod

        from trn_agent_boot.trn_boot import _ntff_profile_via_ctypes

        so_path = "/opt/axon/libaxon_pjrt.so"
        hook = _ntff_profile_via_ctypes(so_path)
        if hook is not None:
            mod._hook = hook
    except Exception:
        pass


def _route(x, gate_w, top_k):
    """Replicates the reference router in numpy fp32 (renormalized top-k
    softmax == softmax over the top-k logits)."""
    logits = x.astype(np.float32) @ gate_w.astype(np.float32).T  # [T, E]
    k = int(top_k)
    idx = np.argpartition(-logits, k - 1, axis=1)[:, :k]
    lv = np.take_along_axis(logits, idx, axis=1)
    m = lv.max(axis=1, keepdims=True)
    ew = np.exp(lv - m)
    wts = ew / ew.sum(axis=1, keepdims=True)
    return idx, wts.astype(np.float32)


def _make_slots(hi_counts, lo_counts):
    """Cut experts into <=SLOT_CAP segments per precision; LPT-pack onto 4
    core pairs (fp8 cost 0.5/col); return per-pair slot lists
    [(expert, lo, hi, ty), ...] (bf16 slots first) and the shared slot-spec
    tuple ((S, ty), ...) with S = max across pairs, padded to mult of 4."""
    segs = []
    for e, c in enumerate(hi_counts):
        t = 0
        while t < c:
            s = min(SLOT_CAP, c - t)
            segs.append((s, 1.0, e, t, t + s, "b"))
            t += s
    for e, c in enumerate(lo_counts):
        base = hi_counts[e]
        t = 0
        while t < c:
            s = min(SLOT_CAP, c - t)
            segs.append((s, 0.5, e, base + t, base + t + s, "f"))
            t += s
    # bf16 first (descending), then fp8 (descending), onto least-loaded pair
    segs.sort(key=lambda g: (g[5], -g[0]))
    loads = [0.0] * 4
    pair_slots = [[] for _ in range(4)]
    for s, cost, e, lo, hi, ty in segs:
        p = min(range(4), key=lambda i: loads[i])
        loads[p] += s * cost
        pair_slots[p].append((e, lo, hi, ty))
    for sl in pair_slots:
        sl.sort(key=lambda t: (t[3], -(t[2] - t[1])))
    kb = max(sum(1 for t in sl if t[3] == "b") for sl in pair_slots)
    kf = max(sum(1 for t in sl if t[3] == "f") for sl in pair_slots)
    # normalize: every pair gets kb bf slots then kf f8 slots (dummies empty)
    spec = []
    norm = [[] for _ in range(4)]
    for ji in range(kb + kf):
        ty = "b" if ji < kb else "f"
        m = 8
        for p in range(4):
            mine = [t for t in pair_slots[p] if t[3] == ty]
            i = ji if ty == "b" else ji - kb
            if i < len(mine):
                norm[p].append(mine[i])
                m = max(m, mine[i][2] - mine[i][1])
            else:
                norm[p].append(None)
        spec.append((-(-m // 4) * 4, ty))
    return norm, tuple(spec)


def kernel(x, gate_w, wv1, w2, top_k):
    import ml_dtypes

    from concourse.bass_utils import run_bass_kernel_spmd

    x = np.asarray(x)
    gate_w = np.asarray(gate_w)
    wv1 = np.asarray(wv1)
    w2 = np.asarray(w2)

    T, D = x.shape
    E, F2, _ = wv1.shape
    F = F2 // 2
    Fh = F // 2
    ND = D // P
    NF = Fh // P
    NPR = NF // 2
    NDP = ND // 2
    n_cores = 8

    idx, wts = _route(x, gate_w, top_k)

    rows_l, w_l, hi_n = [], [], []
    for e in range(E):
        rows, cols = np.nonzero(idx == e)
        w_e = wts[rows, cols]
        order = np.argsort(-w_e, kind="stable")
        rows_l.append(rows[order])
        w_l.append(w_e[order])
        hi_n.append(int((w_e >= THETA).sum()))
    counts = [len(r) for r in rows_l]
    lo_n = [c - h for c, h in zip(counts, hi_n)]

    pair_slots, slot_spec = _make_slots(hi_n, lo_n)

    key = (slot_spec, D, Fh)
    if key not in _BASS_CACHE:
        _BASS_CACHE[key] = _build_bass(slot_spec, D, Fh)
    nc = _BASS_CACHE[key]

    bf16 = ml_dtypes.bfloat16
    f8 = ml_dtypes.float8_e4m3
    x_bf = x.astype(bf16)
    x_f8 = np.clip(x * X8S, -240, 240).astype(f8)
    w1_bf = wv1.astype(bf16)
    w2_bf = w2.astype(bf16)

    w1_cache: dict = {}
    w2_cache: dict = {}

    def w1_pack(e, h, ty):
        if (e, h, ty) not in w1_cache:
            if ty == "b":
                gsl = w1_bf[e][h * Fh : (h + 1) * Fh]
                usl = w1_bf[e][F + h * Fh : F + (h + 1) * Fh]
                both = np.stack([gsl, usl])  # [2, Fh, D]
                w1_cache[(e, h, ty)] = np.ascontiguousarray(
                    both.reshape(2, NPR, 2, P, ND, P).transpose(0, 1, 5, 2, 4, 3)
                )
            else:
                gsl = wv1[e][h * Fh : (h + 1) * Fh]
                usl = wv1[e][F + h * Fh : F + (h + 1) * Fh]
                both = np.clip(np.stack([gsl, usl]) * W1S, -240, 240).astype(f8)
                w1_cache[(e, h, ty)] = np.ascontiguousarray(
                    both.reshape(2, NPR, 2, P, NDP, 2, P).transpose(0, 1, 6, 2, 4, 5, 3)
                )
        return w1_cache[(e, h, ty)]

    def w2_pack(e, h, ty):
        if (e, h, ty) not in w2_cache:
            if ty == "b":
                sl = w2_bf[e][:, h * Fh : (h + 1) * Fh]  # [D, Fh]
                w2_cache[(e, h, ty)] = np.ascontiguousarray(
                    sl.reshape(NDP, 2, P, NF, P).transpose(0, 4, 1, 3, 2)
                )
            else:
                sl = np.clip(w2[e][:, h * Fh : (h + 1) * Fh] * W2S, -240, 240).astype(f8)
                w2_cache[(e, h, ty)] = np.ascontiguousarray(
                    sl.reshape(NDP, 2, P, NPR, 2, P).transpose(0, 5, 1, 3, 4, 2)
                )
        return w2_cache[(e, h, ty)]

    in_maps = []
    for p in range(4):
        slots = pair_slots[p]
        for h in range(2):
            im = {}
            for ji, (S, ty) in enumerate(slot_spec):
                slot = slots[ji]
                if ty == "b":
                    xq, w1z, w2z = (
                        np.zeros((D, S), dtype=bf16),
                        np.zeros((2, NPR, P, 2, ND, P), dtype=bf16),
                        np.zeros((NDP, P, 2, NF, P), dtype=bf16),
                    )
                else:
                    xq, w1z, w2z = (
                        np.zeros((D, S), dtype=f8),
                        np.zeros((2, NPR, P, 2, NDP, 2, P), dtype=f8),
                        np.zeros((NDP, P, 2, NPR, 2, P), dtype=f8),
                    )
                if slot is not None:
                    e, lo, hi, _ = slot
                    seg = rows_l[e][lo:hi]
                    src = x_bf if ty == "b" else x_f8
                    xq[:, : hi - lo] = src[seg].T
                    w1z = w1_pack(e, h, ty)
                    w2z = w2_pack(e, h, ty)
                im[f"x{ji}"] = np.ascontiguousarray(xq.reshape(ND, P, S))
                im[f"w1{ji}"] = w1z
                im[f"w2{ji}"] = w2z
            in_maps.append(im)

    _ensure_ntff_hook()
    res = run_bass_kernel_spmd(nc, in_maps, core_ids=list(range(n_cores)))
    global last_run
    last_run = res

    out = np.zeros((T, D), dtype=np.float32)
    for p in range(4):
        for ji, (S, ty) in enumerate(slot_spec):
            slot = pair_slots[p][ji]
            if slot is None:
                continue
            e, lo, hi, _ = slot
            n = hi - lo
            if ty == "b":
                y0 = res.results[2 * p][f"y{ji}"].reshape(D, -1)[:, :n]
                y1 = res.results[2 * p + 1][f"y{ji}"].reshape(D, -1)[:, :n]
            else:
                y0 = (
                    res.results[2 * p][f"y{ji}"]
                    .transpose(0, 2, 1, 3)
                    .reshape(D, -1)[:, :n]
                )
                y1 = (
                    res.results[2 * p + 1][f"y{ji}"]
                    .transpose(0, 2, 1, 3)
                    .reshape(D, -1)[:, :n]
                )
            ysum = y0.astype(np.float32) + y1.astype(np.float32)
            if ty == "f":
                ysum *= 1.0 / Y8S
            seg = rows_l[e][lo:hi]
            out[seg] += w_l[e][lo:hi, None] * ysum.T
    return out.astype(x.dtype, copy=False)


# revision 17
# speedup vs baseline: 1.3309x; 1.0063x over previous
"""Block-sparse MoE (softmax top-k routing + silu-gated FFN) on 8 Trainium2 cores.

Sharding: expert-pair x FFN-half. The router runs on host; each expert's
token list is sorted by router weight and split at THETA: high-weight
pairs run in bf16, low-weight pairs in fp8 e4m3 (DoubleRow, 2x tensor
rate) — the output error each fp8 token contributes is proportional to
its (small) router weight, keeping total rel err ~1.2e-2 vs the 2e-2
gate. Segments are LPT-packed onto 4 core-pairs (fp8 cost 0.5/col) and
each pair of cores splits the FFN dim in half (1792 of 3584), so all 8
cores run the same slot-size program (SPMD) with balanced cycles.

Per slot on a core (S tokens, ND=16 d-tiles, NFh=14 f-tiles):
  phase1: h.T [128 f, chunk] = w1h.T @ x (contract D); silu(g)*u -> aT
  phase3: y[d, tok] = sum_fi w2h[f,d].T @ aT[:, fi, :] (contract F/2)
          d on PSUM partitions, tokens moving => exact columns, no pad.
  fp8 slots: x*16, w1*32 -> psum = 512*h; silu via ACT scale 1/512;
  u16 = psum/32; a8 = e4m3(silu*u16) = 16a; w2*64 -> y*1024 (host /1024).

Moving chunks are equalized (~420-510) so LDWEIGHTS (~97-116ns) hides
under the moving phase. Slot0's x arrives dt-major in two column chunks
consumed dt-outer across 8 PSUM banks (PE starts ~12us in). w1 prefetch
depth 4 covers the ~6us pair-tile transfer. All bulk traffic rides the
SWDGE queue with >=2KB lines; later slots' x and w1 overlap prior ph3.
Host sums the two F-halves, applies router weight, scatter-adds (f32).
"""

import numpy as np


def _ensure_concourse_on_path():
    try:
        import concourse  # noqa: F401
    except ImportError:
        import sys

        for p in ("/opt/trn_rl_repo", "/root/.axon_site/_ro/trn_rl_repo"):
            if p not in sys.path:
                sys.path.insert(0, p)


_ensure_concourse_on_path()

P = 128
CHUNK_MAX = 512   # PSUM bank free-dim limit (f32)
SLOT_CAP = 2560   # max tokens per slot (SBUF-bound)
THETA = 0.37      # router-weight cutoff: below -> fp8 path
X8S, W1S, W2S = 16.0, 32.0, 64.0
Y8S = X8S * W2S   # fp8-slot output scale (host divides)

_BASS_CACHE: dict = {}
last_run = None  # BassKernelResults of the most recent kernel() call (for test.py)


def _legalize_sync(nc, max_waits: int = 1):
    """Split multi-wait sync_info into preceding EventSemaphore instructions.

    The walrus build in this container lowers every instruction with capacity
    for a single sync-wait command and errors with "Too many sync wait
    commands" otherwise, while Tile attaches up to 3 waits per instruction.
    A wait carried by an EventSemaphore on the same engine immediately before
    the instruction is semantically identical. For DMAs, keep the own-lane
    FIFO wait on the instruction itself so the in-queue wait doesn't stall
    the sequencer.
    """
    import concourse.mybir as mybir

    fn = nc.m.functions[0]
    for blk in fn.blocks:
        new_insts = []
        for inst in blk.instructions:
            si = inst.sync_info
            if si is not None and si.on_wait is not None and len(si.on_wait) > max_waits:
                ow = list(si.on_wait)
                upd_ids = {u.id for u in (si.on_update or [])}
                keep = [w for w in ow if w.id in upd_ids][:1]
                if not keep:
                    keep = [ow[-1]]
                for j, w in enumerate(ow):
                    if w is keep[0]:
                        continue
                    new_insts.append(
                        mybir.InstEventSemaphore(
                            name=f"{inst.name}-ws{j}",
                            opcode="EventSemaphore",
                            engine=inst.engine,
                            sync_info=mybir.SyncInfo(on_wait=[w], on_update=[]),
                        )
                    )
                si.on_wait = keep
            new_insts.append(inst)
        blk.instructions = new_insts


def _chunk_list(S: int):
    """Equalized moving chunks: k = ceil(S/512), base = ceil(S/k) rounded to
    a multiple of 4, so every chunk is large enough that LDWEIGHTS hides
    under the moving phase."""
    k = -(-S // CHUNK_MAX)
    base = -(-S // k)
    base = -(-base // 4) * 4
    chunks = []
    t0 = 0
    while t0 < S:
        c = min(base, S - t0)
        chunks.append((t0, c))
        t0 += c
    return chunks


def _build_bass(slot_spec: tuple, D: int, Fh: int):
    """Bass program: sequence of expert slots, each (S tokens, ty) with
    ty 'b' (bf16) or 'f' (fp8 e4m3 DoubleRow). Per-slot inputs x{j}, w1{j},
    w2{j}; outputs y{j}."""
    import concourse.bass as bass
    import concourse.mybir as mybir
    import concourse.tile as tile

    ND = D // P           # 16 contraction tiles (phase 1)
    NF = Fh // P          # 14 f-tiles per half
    NPR = NF // 2         # 7 w1 pairs
    NDP = ND // 2         # 8 dt-pair groups
    assert NF % 2 == 0 and ND % 2 == 0
    S1b = max((S for S, ty in slot_spec if ty == "b"), default=8)
    S1f = max((S for S, ty in slot_spec if ty == "f"), default=8)

    bf16 = mybir.dt.bfloat16
    f8 = mybir.dt.float8e4
    f32 = mybir.dt.float32
    DR = mybir.MatmulPerfMode.DoubleRow

    nc = bass.Bass(name="moe_pair_ffn", num_swdge_queues=4)
    xs_d, w1s_d, w2s_d, ys_d = [], [], [], []
    for j, (S, ty) in enumerate(slot_spec):
        if ty == "b":
            xs_d.append(nc.dram_tensor(f"x{j}", [ND, P, S], bf16, kind="ExternalInput"))
            w1s_d.append(
                nc.dram_tensor(
                    f"w1{j}", [2, NPR, P, 2, ND, P], bf16, kind="ExternalInput"
                )
            )
            w2s_d.append(
                nc.dram_tensor(f"w2{j}", [NDP, P, 2, NF, P], bf16, kind="ExternalInput")
            )
        else:
            xs_d.append(nc.dram_tensor(f"x{j}", [ND, P, S], f8, kind="ExternalInput"))
            w1s_d.append(
                nc.dram_tensor(
                    f"w1{j}", [2, NPR, P, 2, NDP, 2, P], f8, kind="ExternalInput"
                )
            )
            w2s_d.append(
                nc.dram_tensor(
                    f"w2{j}", [NDP, P, 2, NPR, 2, P], f8, kind="ExternalInput"
                )
            )
        if ty == "b":
            ys_d.append(
                nc.dram_tensor(f"y{j}", [ND, P, S], bf16, kind="ExternalOutput")
            )
        else:
            ys_d.append(
                nc.dram_tensor(
                    f"y{j}", [ND // 4, P, 4, S], bf16, kind="ExternalOutput"
                )
            )

    act_silu = mybir.ActivationFunctionType.Silu
    act_copy = mybir.ActivationFunctionType.Copy

    with tile.TileContext(nc) as tc:
        with (
            tc.tile_pool(name="xp", bufs=1) as xpool,
            tc.tile_pool(name="ap", bufs=1) as apool,
            tc.tile_pool(name="w1p", bufs=3) as w1pool,
            tc.tile_pool(name="w2p", bufs=3) as w2pool,
            tc.tile_pool(name="hp", bufs=3) as hpool,
            tc.tile_pool(name="yp", bufs=3) as ypool,
            tc.tile_pool(name="x8p", bufs=1) as x8pool,
            tc.tile_pool(name="w18p", bufs=4) as w18pool,
            tc.tile_pool(name="w28p", bufs=4) as w28pool,
            tc.tile_pool(name="y4p", bufs=2) as y4pool,
            tc.tile_pool(name="ps", bufs=8, space="PSUM") as psum,
        ):
            # fp8 tiles live in their own pools, declared after the bf16
            # pools so the bf16 phase keeps the measured conflict-free
            # SBUF layout (shifting pool bases cost +35ns/matmul once).
            x_sb = xpool.tile([P, ND, S1b], bf16)
            aT = apool.tile([P, NF, S1b], bf16)
            has_f8 = any(ty == "f" for _, ty in slot_spec)
            if has_f8:
                x8_sb = x8pool.tile([P, ND, S1f], f8)
                aT8 = x8pool.tile([P, NF, S1f], f8)

            x8_hoisted: set = set()
            w18_hoisted: dict = {}
            for j, (S, ty) in enumerate(slot_spec):
                chunks = _chunk_list(S)
                x_d, w1_d, w2_d, y_d = xs_d[j], w1s_d[j], w2s_d[j], ys_d[j]

                if ty == "b":
                    # ---------------- bf16 slot ----------------
                    w1_pre: dict = {}
                    xsplit = -(-S // 8) * 4
                    startup = (
                        j == 0
                        and len(chunks) >= 3
                        and chunks[1][0] + chunks[1][1] <= xsplit
                    )
                    w1g0 = w1pool.tile([P, 2, ND, P], bf16, tag="w1", name="w1g0")
                    w1u0 = w1pool.tile([P, 2, ND, P], bf16, tag="w1", name="w1u0")
                    if startup:
                        # queue order: 0.5MB of w1 (gate/jj0), x chunk0
                        # dt-major, rest of pair0, x chunk1 — the dt-outer
                        # sub-phases below consume in exactly this order.
                        nc.gpsimd.dma_start(w1g0[:, 0], w1_d[0, 0, :, 0])
                        for dt in range(ND):
                            nc.gpsimd.dma_start(
                                x_sb[:, dt, :xsplit], x_d[dt, :, :xsplit]
                            )
                        nc.gpsimd.dma_start(w1g0[:, 1], w1_d[0, 0, :, 1])
                        nc.gpsimd.dma_start(w1u0[:, 0], w1_d[1, 0, :, 0])
                        nc.gpsimd.dma_start(w1u0[:, 1], w1_d[1, 0, :, 1])
                        p1g = w1pool.tile([P, 2, ND, P], bf16, tag="w1", name="p1g")
                        nc.gpsimd.dma_start(p1g, w1_d[0, 1])
                        for dt in range(ND):
                            nc.gpsimd.dma_start(
                                x_sb[:, dt, xsplit:S], x_d[dt, :, xsplit:]
                            )
                        # 4th alloc in a 3-buf pool: its slot wait blocks the
                        # gpsimd engine until pair0 is consumed, so it must
                        # sit AFTER the x chunk1 DMAs in program order.
                        p1u = w1pool.tile([P, 2, ND, P], bf16, tag="w1", name="p1u")
                        nc.gpsimd.dma_start(p1u, w1_d[1, 1])
                        w1_pre[1] = (p1g, p1u)
                    else:
                        nc.gpsimd.dma_start(w1g0, w1_d[0, 0])
                        nc.gpsimd.dma_start(w1u0, w1_d[1, 0])
                        for dt in range(ND):
                            nc.gpsimd.dma_start(x_sb[:, dt, :S], x_d[dt])

                    def ph1_chain(w1t, jj, t0, csz, start, stop, ps):
                        for dt in range(ND):
                            nc.tensor.matmul(
                                ps[:, :csz],
                                w1t[:, jj, dt, :],
                                x_sb[:, dt, t0 : t0 + csz],
                                start=start and dt == 0,
                                stop=stop and dt == ND - 1,
                            )

                    def ph1_evac(fi, t0, csz, ps_g, ps_u):
                        hg = hpool.tile([P, CHUNK_MAX], bf16, tag="h", name="hg")
                        nc.scalar.activation(hg[:, :csz], ps_g[:, :csz], act_silu)
                        nc.vector.tensor_mul(
                            aT[:, fi, t0 : t0 + csz], hg[:, :csz], ps_u[:, :csz]
                        )

                    pr_start = 0
                    if startup:
                        sc = chunks[:2]
                        banks = {}
                        for gu in range(2):
                            for jj in range(2):
                                for ci in range(len(sc)):
                                    banks[(jj, ci, gu)] = psum.tile(
                                        [P, CHUNK_MAX], f32, tag="ps",
                                        name=f"ps_s{jj}{ci}{gu}",
                                    )
                        for gu, jj in ((0, 0), (0, 1), (1, 0), (1, 1)):
                            w1t = w1g0 if gu == 0 else w1u0
                            for dt in range(ND):
                                for ci, (t0, csz) in enumerate(sc):
                                    nc.tensor.matmul(
                                        banks[(jj, ci, gu)][:, :csz],
                                        w1t[:, jj, dt, :],
                                        x_sb[:, dt, t0 : t0 + csz],
                                        start=(dt == 0),
                                        stop=(dt == ND - 1),
                                    )
                            if gu == 1:
                                for ci, (t0, csz) in enumerate(sc):
                                    ph1_evac(
                                        jj, t0, csz,
                                        banks[(jj, ci, 0)], banks[(jj, ci, 1)],
                                    )
                        for jj in range(2):
                            for t0, csz in chunks[2:]:
                                ps_g = psum.tile(
                                    [P, CHUNK_MAX], f32, tag="ps", name="ps_g"
                                )
                                ph1_chain(w1g0, jj, t0, csz, True, True, ps_g)
                                ps_u = psum.tile(
                                    [P, CHUNK_MAX], f32, tag="ps", name="ps_u"
                                )
                                ph1_chain(w1u0, jj, t0, csz, True, True, ps_u)
                                ph1_evac(jj, t0, csz, ps_g, ps_u)
                        pr_start = 1

                    for pr in range(pr_start, NPR):
                        if pr == 0:
                            w1g, w1u = w1g0, w1u0
                        elif pr in w1_pre:
                            w1g, w1u = w1_pre.pop(pr)
                        else:
                            w1g = w1pool.tile([P, 2, ND, P], bf16, tag="w1", name="w1g")
                            for jj in range(2):
                                nc.gpsimd.dma_start(w1g[:, jj], w1_d[0, pr, :, jj])
                            w1u = w1pool.tile([P, 2, ND, P], bf16, tag="w1", name="w1u")
                            for jj in range(2):
                                nc.gpsimd.dma_start(w1u[:, jj], w1_d[1, pr, :, jj])
                        for jj in range(2):
                            for t0, csz in chunks:
                                ps_g = psum.tile(
                                    [P, CHUNK_MAX], f32, tag="ps", name="ps_g"
                                )
                                ph1_chain(w1g, jj, t0, csz, True, True, ps_g)
                                ps_u = psum.tile(
                                    [P, CHUNK_MAX], f32, tag="ps", name="ps_u"
                                )
                                ph1_chain(w1u, jj, t0, csz, True, True, ps_u)
                                ph1_evac(2 * pr + jj, t0, csz, ps_g, ps_u)

                    for dp in range(NDP):
                        w2sb = w2pool.tile([P, 2, NF, P], bf16, tag="w2", name="w2sb")
                        nc.gpsimd.dma_start(w2sb, w2_d[dp])
                        for dj in range(2):
                            dt = 2 * dp + dj
                            y_st = ypool.tile([P, S1b], bf16, tag="y", name="y_st")
                            for t0, csz in chunks:
                                ps_y = psum.tile(
                                    [P, CHUNK_MAX], f32, tag="ps", name="ps_y"
                                )
                                for fi in range(NF):
                                    nc.tensor.matmul(
                                        ps_y[:, :csz],
                                        w2sb[:, dj, fi, :],
                                        aT[:, fi, t0 : t0 + csz],
                                        start=(fi == 0),
                                        stop=(fi == NF - 1),
                                    )
                                nc.scalar.activation(
                                    y_st[:, t0 : t0 + csz], ps_y[:, :csz], act_copy
                                )
                            (nc.sync if dt % 2 == 0 else nc.gpsimd).dma_start(
                                y_d[dt], y_st[:, :S]
                            )
                    # prefetch the next (fp8) slot's x8 + first w18 pair ahead
                    # of this ph3's throttled y DMAs so they aren't
                    # head-blocked on the SWDGE queue.
                    if j + 1 < len(slot_spec) and slot_spec[j + 1][1] == "f":
                        Sn = slot_spec[j + 1][0]
                        for dt in range(ND):
                            nc.gpsimd.dma_start(
                                x8_sb[:, dt, :Sn], xs_d[j + 1][dt]
                            )
                        x8_hoisted.add(j + 1)
                        hg0 = w18pool.tile(
                            [P, 2, NDP, 2, P], f8, tag="w18", name="w18hg"
                        )
                        for jj in range(2):
                            nc.gpsimd.dma_start(
                                hg0[:, jj], w1s_d[j + 1][0, 0, :, jj]
                            )
                        hu0 = w18pool.tile(
                            [P, 2, NDP, 2, P], f8, tag="w18", name="w18hu"
                        )
                        for jj in range(2):
                            nc.gpsimd.dma_start(
                                hu0[:, jj], w1s_d[j + 1][1, 0, :, jj]
                            )
                        hg1 = w18pool.tile(
                            [P, 2, NDP, 2, P], f8, tag="w18", name="w18hg1"
                        )
                        nc.gpsimd.dma_start(hg1, w1s_d[j + 1][0, 1])
                        hu1 = w18pool.tile(
                            [P, 2, NDP, 2, P], f8, tag="w18", name="w18hu1"
                        )
                        nc.gpsimd.dma_start(hu1, w1s_d[j + 1][1, 1])
                        w18_hoisted[j + 1] = ((hg0, hu0), (hg1, hu1))
                else:
                    # ---------------- fp8 slot (DoubleRow) ----------------
                    if j not in x8_hoisted:
                        for dt in range(ND):
                            nc.gpsimd.dma_start(x8_sb[:, dt, :S], x_d[dt])

                    for pr in range(NPR):
                        if j in w18_hoisted and pr < len(w18_hoisted[j]):
                            w1g, w1u = w18_hoisted[j][pr]
                        else:
                            w1g = w18pool.tile(
                                [P, 2, NDP, 2, P], f8, tag="w18", name="w18g"
                            )
                            for jj in range(2):
                                nc.gpsimd.dma_start(w1g[:, jj], w1_d[0, pr, :, jj])
                            w1u = w18pool.tile(
                                [P, 2, NDP, 2, P], f8, tag="w18", name="w18u"
                            )
                            for jj in range(2):
                                nc.gpsimd.dma_start(w1u[:, jj], w1_d[1, pr, :, jj])
                        for jj in range(2):
                            fi = 2 * pr + jj
                            for t0, csz in chunks:
                                ps_g = psum.tile(
                                    [P, CHUNK_MAX], f32, tag="ps", name="ps_g"
                                )
                                for dp in range(NDP):
                                    nc.tensor.matmul(
                                        ps_g[:, :csz],
                                        w1g[:, jj, dp],
                                        x8_sb[:, 2 * dp : 2 * dp + 2, t0 : t0 + csz],
                                        start=(dp == 0),
                                        stop=(dp == NDP - 1),
                                        perf_mode=DR,
                                    )
                                ps_u = psum.tile(
                                    [P, CHUNK_MAX], f32, tag="ps", name="ps_u"
                                )
                                for dp in range(NDP):
                                    nc.tensor.matmul(
                                        ps_u[:, :csz],
                                        w1u[:, jj, dp],
                                        x8_sb[:, 2 * dp : 2 * dp + 2, t0 : t0 + csz],
                                        start=(dp == 0),
                                        stop=(dp == NDP - 1),
                                        perf_mode=DR,
                                    )
                                hg = hpool.tile([P, CHUNK_MAX], bf16, tag="h", name="hg")
                                nc.scalar.activation(
                                    hg[:, :csz], ps_g[:, :csz], act_silu,
                                    scale=1.0 / (X8S * W1S),
                                )
                                hu = hpool.tile([P, CHUNK_MAX], bf16, tag="h", name="hu")
                                nc.scalar.activation(
                                    hu[:, :csz], ps_u[:, :csz], act_copy,
                                    scale=X8S / (X8S * W1S),
                                )
                                nc.vector.tensor_mul(
                                    aT8[:, fi, t0 : t0 + csz], hg[:, :csz], hu[:, :csz]
                                )

                    # pre-issue the whole w2 stream (flows during ph1; the
                    # deeper allocs' pool waits release just-in-time), then
                    # hoist the next fp8 slot's x8 + first w18 pair ahead of
                    # this ph3's throttled y4 DMAs.
                    w28_pre = []
                    for dp in range(NDP):
                        w2sb = w28pool.tile(
                            [P, 2, NPR, 2, P], f8, tag="w28", name="w28sb"
                        )
                        nc.gpsimd.dma_start(w2sb, w2_d[dp])
                        w28_pre.append(w2sb)
                    if j + 1 < len(slot_spec) and slot_spec[j + 1][1] == "f":
                        Sn = slot_spec[j + 1][0]
                        for dt in range(ND):
                            nc.gpsimd.dma_start(
                                x8_sb[:, dt, :Sn], xs_d[j + 1][dt]
                            )
                        x8_hoisted.add(j + 1)
                        fg0 = w18pool.tile(
                            [P, 2, NDP, 2, P], f8, tag="w18", name="w18fg0"
                        )
                        for jj in range(2):
                            nc.gpsimd.dma_start(
                                fg0[:, jj], w1s_d[j + 1][0, 0, :, jj]
                            )
                        fu0 = w18pool.tile(
                            [P, 2, NDP, 2, P], f8, tag="w18", name="w18fu0"
                        )
                        for jj in range(2):
                            nc.gpsimd.dma_start(
                                fu0[:, jj], w1s_d[j + 1][1, 0, :, jj]
                            )
                        w18_hoisted[j + 1] = ((fg0, fu0),)
                    y4 = None
                    for dp in range(NDP):
                        w2sb = w28_pre[dp]
                        for dj in range(2):
                            dt = 2 * dp + dj
                            if dt % 4 == 0:
                                y4 = y4pool.tile(
                                    [P, 4, S1f], bf16, tag="y4", name="y4"
                                )
                            for t0, csz in chunks:
                                ps_y = psum.tile(
                                    [P, CHUNK_MAX], f32, tag="ps", name="ps_y"
                                )
                                for q in range(NPR):
                                    nc.tensor.matmul(
                                        ps_y[:, :csz],
                                        w2sb[:, dj, q],
                                        aT8[:, 2 * q : 2 * q + 2, t0 : t0 + csz],
                                        start=(q == 0),
                                        stop=(q == NPR - 1),
                                        perf_mode=DR,
                                    )
                                nc.scalar.activation(
                                    y4[:, dt % 4, t0 : t0 + csz],
                                    ps_y[:, :csz],
                                    act_copy,
                                )
                            if dt % 4 == 3:
                                nc.gpsimd.dma_start(
                                    y_d[dt // 4], y4[:, :, :S]
                                )

    _legalize_sync(nc)
    return nc


def _ensure_ntff_hook():
    """Register the axon NTFF-profile hook if the image's antenv lacks
    ``axon_hooks`` (the hook impl ships in trn_agent_boot). Best-effort."""
    import sys
    import types

    try:
        from antenv.axon_hooks import get_axon_ntff_profile_hook  # noqa: F401

        return
    except ImportError:
        pass
    try:
        import antenv

        mod = types.ModuleType("antenv.axon_hooks")
        mod._hook = None

        def set_axon_ntff_profile_hook(h):
            mod._hook = h

        def get_axon_ntff_profile_hook():
            return mod._hook

        mod.set_axon_ntff_profile_hook = set_axon_ntff_profile_hook
        mod.get_axon_ntff_profile_hook = get_axon_ntff_profile_hook
        sys.modules["antenv.axon_hooks"] = mod
        antenv.axon_hooks = mod

        from trn_agent_boot.trn_boot import _ntff_profile_via_ctypes

        so_path = "/opt/axon/libaxon_pjrt.so"
        hook = _ntff_profile_via_ctypes(so_path)
        if hook is not None:
            mod._hook = hook
    except Exception:
        pass


def _route(x, gate_w, top_k):
    """Replicates the reference router in numpy fp32 (renormalized top-k
    softmax == softmax over the top-k logits)."""
    logits = x.astype(np.float32) @ gate_w.astype(np.float32).T  # [T, E]
    k = int(top_k)
    idx = np.argpartition(-logits, k - 1, axis=1)[:, :k]
    lv = np.take_along_axis(logits, idx, axis=1)
    m = lv.max(axis=1, keepdims=True)
    ew = np.exp(lv - m)
    wts = ew / ew.sum(axis=1, keepdims=True)
    return idx, wts.astype(np.float32)


def _make_slots(hi_counts, lo_counts):
    """Cut experts into <=SLOT_CAP segments per precision; LPT-pack onto 4
    core pairs (fp8 cost 0.5/col); return per-pair slot lists
    [(expert, lo, hi, ty), ...] (bf16 slots first) and the shared slot-spec
    tuple ((S, ty), ...) with S = max across pairs, padded to mult of 4."""
    segs = []
    for e, c in enumerate(hi_counts):
        t = 0
        while t < c:
            s = min(SLOT_CAP, c - t)
            segs.append((s, 1.0, e, t, t + s, "b"))
            t += s
    for e, c in enumerate(lo_counts):
        base = hi_counts[e]
        t = 0
        while t < c:
            s = min(SLOT_CAP, c - t)
            segs.append((s, 0.5, e, base + t, base + t + s, "f"))
            t += s
    # bf16 first (descending), then fp8 (descending), onto least-loaded pair
    segs.sort(key=lambda g: (g[5], -g[0]))
    loads = [0.0] * 4
    pair_slots = [[] for _ in range(4)]
    for s, cost, e, lo, hi, ty in segs:
        p = min(range(4), key=lambda i: loads[i])
        loads[p] += s * cost
        pair_slots[p].append((e, lo, hi, ty))
    for sl in pair_slots:
        sl.sort(key=lambda t: (t[3], -(t[2] - t[1])))
    kb = max(sum(1 for t in sl if t[3] == "b") for sl in pair_slots)
    kf = max(sum(1 for t in sl if t[3] == "f") for sl in pair_slots)
    # normalize: every pair gets kb bf slots then kf f8 slots (dummies empty)
    spec = []
    norm = [[] for _ in range(4)]
    for ji in range(kb + kf):
        ty = "b" if ji < kb else "f"
        m = 8
        for p in range(4):
            mine = [t for t in pair_slots[p] if t[3] == ty]
            i = ji if ty == "b" else ji - kb
            if i < len(mine):
                norm[p].append(mine[i])
                m = max(m, mine[i][2] - mine[i][1])
            else:
                norm[p].append(None)
        spec.append((-(-m // 4) * 4, ty))
    return norm, tuple(spec)


def kernel(x, gate_w, wv1, w2, top_k):
    import ml_dtypes

    from concourse.bass_utils import run_bass_kernel_spmd

    x = np.asarray(x)
    gate_w = np.asarray(gate_w)
    wv1 = np.asarray(wv1)
    w2 = np.asarray(w2)

    T, D = x.shape
    E, F2, _ = wv1.shape
    F = F2 // 2
    Fh = F // 2
    ND = D // P
    NF = Fh // P
    NPR = NF // 2
    NDP = ND // 2
    n_cores = 8

    idx, wts = _route(x, gate_w, top_k)

    rows_l, w_l, hi_n = [], [], []
    for e in range(E):
        rows, cols = np.nonzero(idx == e)
        w_e = wts[rows, cols]
        order = np.argsort(-w_e, kind="stable")
        rows_l.append(rows[order])
        w_l.append(w_e[order])
        hi_n.append(int((w_e >= THETA).sum()))
    counts = [len(r) for r in rows_l]
    lo_n = [c - h for c, h in zip(counts, hi_n)]

    pair_slots, slot_spec = _make_slots(hi_n, lo_n)

    key = (slot_spec, D, Fh)
    if key not in _BASS_CACHE:
        _BASS_CACHE[key] = _build_bass(slot_spec, D, Fh)
    nc = _BASS_CACHE[key]

    bf16 = ml_dtypes.bfloat16
    f8 = ml_dtypes.float8_e4m3
    x_bf = x.astype(bf16)
    x_f8 = np.clip(x * X8S, -240, 240).astype(f8)
    w1_bf = wv1.astype(bf16)
    w2_bf = w2.astype(bf16)

    w1_cache: dict = {}
    w2_cache: dict = {}

    def w1_pack(e, h, ty):
        if (e, h, ty) not in w1_cache:
            if ty == "b":
                gsl = w1_bf[e][h * Fh : (h + 1) * Fh]
                usl = w1_bf[e][F + h * Fh : F + (h + 1) * Fh]
                both = np.stack([gsl, usl])  # [2, Fh, D]
                w1_cache[(e, h, ty)] = np.ascontiguousarray(
                    both.reshape(2, NPR, 2, P, ND, P).transpose(0, 1, 5, 2, 4, 3)
                )
            else:
                gsl = wv1[e][h * Fh : (h + 1) * Fh]
                usl = wv1[e][F + h * Fh : F + (h + 1) * Fh]
                both = np.clip(np.stack([gsl, usl]) * W1S, -240, 240).astype(f8)
                w1_cache[(e, h, ty)] = np.ascontiguousarray(
                    both.reshape(2, NPR, 2, P, NDP, 2, P).transpose(0, 1, 6, 2, 4, 5, 3)
                )
        return w1_cache[(e, h, ty)]

    def w2_pack(e, h, ty):
        if (e, h, ty) not in w2_cache:
            if ty == "b":
                sl = w2_bf[e][:, h * Fh : (h + 1) * Fh]  # [D, Fh]
                w2_cache[(e, h, ty)] = np.ascontiguousarray(
                    sl.reshape(NDP, 2, P, NF, P).transpose(0, 4, 1, 3, 2)
                )
            else:
                sl = np.clip(w2[e][:, h * Fh : (h + 1) * Fh] * W2S, -240, 240).astype(f8)
                w2_cache[(e, h, ty)] = np.ascontiguousarray(
                    sl.reshape(NDP, 2, P, NPR, 2, P).transpose(0, 5, 1, 3, 4, 2)
                )
        return w2_cache[(e, h, ty)]

    in_maps = []
    for p in range(4):
        slots = pair_slots[p]
        for h in range(2):
            im = {}
            for ji, (S, ty) in enumerate(slot_spec):
                slot = slots[ji]
                if ty == "b":
                    xq, w1z, w2z = (
                        np.zeros((D, S), dtype=bf16),
                        np.zeros((2, NPR, P, 2, ND, P), dtype=bf16),
                        np.zeros((NDP, P, 2, NF, P), dtype=bf16),
                    )
                else:
                    xq, w1z, w2z = (
                        np.zeros((D, S), dtype=f8),
                        np.zeros((2, NPR, P, 2, NDP, 2, P), dtype=f8),
                        np.zeros((NDP, P, 2, NPR, 2, P), dtype=f8),
                    )
                if slot is not None:
                    e, lo, hi, _ = slot
                    seg = rows_l[e][lo:hi]
                    src = x_bf if ty == "b" else x_f8
                    xq[:, : hi - lo] = src[seg].T
                    w1z = w1_pack(e, h, ty)
                    w2z = w2_pack(e, h, ty)
                im[f"x{ji}"] = np.ascontiguousarray(xq.reshape(ND, P, S))
                im[f"w1{ji}"] = w1z
                im[f"w2{ji}"] = w2z
            in_maps.append(im)

    _ensure_ntff_hook()
    res = run_bass_kernel_spmd(nc, in_maps, core_ids=list(range(n_cores)))
    global last_run
    last_run = res

    out = np.zeros((T, D), dtype=np.float32)
    for p in range(4):
        for ji, (S, ty) in enumerate(slot_spec):
            slot = pair_slots[p][ji]
            if slot is None:
                continue
            e, lo, hi, _ = slot
            n = hi - lo
            if ty == "b":
                y0 = res.results[2 * p][f"y{ji}"].reshape(D, -1)[:, :n]
                y1 = res.results[2 * p + 1][f"y{ji}"].reshape(D, -1)[:, :n]
            else:
                y0 = (
                    res.results[2 * p][f"y{ji}"]
                    .transpose(0, 2, 1, 3)
                    .reshape(D, -1)[:, :n]
                )
                y1 = (
                    res.results[2 * p + 1][f"y{ji}"]
                    .transpose(0, 2, 1, 3)
                    .reshape(D, -1)[:, :n]
                )
            ysum = y0.astype(np.float32) + y1.astype(np.float32)
            if ty == "f":
                ysum *= 1.0 / Y8S
            seg = rows_l[e][lo:hi]
            out[seg] += w_l[e][lo:hi, None] * ysum.T
    return out.astype(x.dtype, copy=False)
